# revision 1
# baseline (speedup 1.0000x reference)
"""Trainium2 Bass kernel for nn_Attention_35734127903400.

Dense transformer attention block:
  xq = LN(x@wq); xk = LN(x@wk); xv = x@wv          (LN over full flattened head dim)
  rope(q, k); GQA self-attention (16 q heads, 8 kv heads, S=2048, full/non-causal)
  gated cross-attention with y (128 tokens); out = (self + tanh(gate)*cross) @ wo

Sharding (8 cores, no collectives): token-sharded. Core c handles batch
b=c//2, sequence half hf=c%2 (1024 q tokens). Each core computes K/V for
its batch's FULL 2048-token sequence (replicated within the pair; +19%
proj FLOPs but zero communication), Q only for its local 1024 tokens.
LN is over the feature dim so it is fully core-local under this sharding.

Numerics: bf16 matmul operands, f32 PSUM accumulation, LN/softmax math in
f32. Softmax skips max-subtraction (q/k are LN'd so |score*scale| stays
far inside exp's f32 range).

Structure: projection stages build Q^T/K^T (LN+rope, PE-transposed to
head-major) and V fully in SBUF, spill each with ONE DMA to DRAM scratch
and free the SBUF (the single-writer/single-reader round trip keeps every
DMA's wait list within the 2-command HWDGE limit, and lets stage-local
pools stack LIFO). The attention stage reloads all three, then per
(head, q-chunk): S^T = K Q^T per 128-key chunk, exp on ACT into bf16,
softmax denominator via ones-matmul (every PSUM partition gets the column
sum), O^T = V^T E so merged heads land feature-major — exactly the rhs
layout the output projection needs. Output is out^T, transposed on host.
"""

import numpy as np
import ml_dtypes

import concourse.bass as bass
import concourse.mybir as mybir
import concourse.tile as tile
from concourse.bass_utils import run_bass_kernel_spmd
from concourse.masks import make_identity

BF16 = ml_dtypes.bfloat16
F32 = mybir.dt.float32
BF = mybir.dt.bfloat16

P = 128
B, S, D = 4, 2048, 2048
H, KVH = 16, 8
HD = 128
NREP = 2
YL, YD = 128, 1024
EPS = 1e-5
S_LOC = S // 2
DC = D // P          # 16 contraction chunks for D
YDC = YD // P        # 8
TC = S // P          # 16 token chunks (full seq)
TCL = S_LOC // P     # 8 local token chunks
NQ = 512             # q-free chunk (one PSUM bank of f32)
QCN = S_LOC // NQ    # 2
KVD = KVH * HD       # 1024
SCALE = 1.0 / float(np.sqrt(np.float32(HD)))
AF = mybir.ActivationFunctionType
ALU = mybir.AluOpType

_CACHED = {}
LAST_EXEC_NS = None


def _ln_stats(nc, statp, ps_chunks):
    """bn_stats over a list of [P, 512] psum chunks -> mv [P, 2] (mean, var)."""
    nchunks = len(ps_chunks)
    stats = statp.tile([P, nchunks, 6], F32, tag="bnstats")
    for i, ps in enumerate(ps_chunks):
        nc.vector.bn_stats(out=stats[:, i, :], in_=ps[:])
    mv = statp.tile([P, 2], F32, tag="bnaggr")
    nc.vector.bn_aggr(out=mv, in_=stats)
    return mv


def _rope_inplace(nc, ropep, zn, nheads, cos_t, sin_t):
    """In-place rope on zn [P, nheads*HD] f32; cos/sin [P, 64] f32."""
    zv = zn.rearrange("p (h f two) -> p h f two", h=nheads, two=2)
    re = zv[:, :, :, 0]
    im = zv[:, :, :, 1]
    shp = (P, nheads, HD // 2)
    cb = cos_t[:, None, :].to_broadcast(shp)
    sb = sin_t[:, None, :].to_broadcast(shp)
    t1 = ropep.tile([P, nheads, HD // 2], F32, tag="rp1")
    t2 = ropep.tile([P, nheads, HD // 2], F32, tag="rp2")
    t3 = ropep.tile([P, nheads, HD // 2], F32, tag="rp3")
    nc.vector.tensor_mul(out=t1, in0=re, in1=cb)   # re*c
    nc.vector.tensor_mul(out=t2, in0=re, in1=sb)   # re*s
    nc.vector.tensor_mul(out=t3, in0=im, in1=sb)   # im*s
    nc.vector.tensor_sub(out=re, in0=t1, in1=t3)   # re' = re*c - im*s
    nc.vector.tensor_mul(out=t3, in0=im, in1=cb)   # im*c
    nc.vector.tensor_add(out=im, in0=t2, in1=t3)   # im' = re*s + im*c


def _split_dma_waits(nc, max_waits=1):
    """This walrus build's per-instruction structs have very few embedded
    sync-wait slots (1-2 depending on opcode). Hoist excess waits of ANY
    instruction onto preceding same-engine single-wait NoOps — the sequencer
    executes them in stream order before the instruction, so semantics are
    identical (marginally more conservative)."""
    n_split = 0
    for f in nc.m.functions:
        for blk in f.blocks:
            insts = list(blk.instructions)
            out = []
            changed = False
            for ins in insts:
                si = ins.sync_info
                if (si is not None and si.on_wait
                        and len(si.on_wait) > max_waits):
                    waits = list(si.on_wait)
                    for wi, w in enumerate(waits[:-max_waits]):
                        out.append(mybir.InstNoOp(
                            name=f"{ins.name}-wsplit{wi}", engine=ins.engine,
                            sync_info=mybir.SyncInfo(on_wait=[w],
                                                     on_update=[])))
                    ins.sync_info = mybir.SyncInfo(
                        on_wait=waits[-max_waits:],
                        on_update=list(si.on_update))
                    changed = True
                    n_split += 1
                out.append(ins)
            if changed:
                blk.instructions = out
    return n_split


def build_program():
    nc = bass.Bass()

    # ---- I/O ----
    xT = nc.declare_dram_parameter("xT", [D, S], BF, isOutput=False)
    xTq = nc.declare_dram_parameter("xTq", [D, S_LOC], BF, isOutput=False)
    yT = nc.declare_dram_parameter("yT", [YD, YL], BF, isOutput=False)
    wq_d = nc.declare_dram_parameter("wq", [D, D], BF, isOutput=False)
    wkv_d = nc.declare_dram_parameter("wkv", [D, 2 * KVD], BF, isOutput=False)
    wkvy_d = nc.declare_dram_parameter("wkvy", [YD, 2 * KVD], BF, isOutput=False)
    wo_d = nc.declare_dram_parameter("wo", [D, D], BF, isOutput=False)
    qw_d = nc.declare_dram_parameter("qw", [D], F32, isOutput=False)
    qb_d = nc.declare_dram_parameter("qb", [D], F32, isOutput=False)
    kw_d = nc.declare_dram_parameter("kw", [KVD], F32, isOutput=False)
    kb_d = nc.declare_dram_parameter("kb", [KVD], F32, isOutput=False)
    kyw_d = nc.declare_dram_parameter("kyw", [KVD], F32, isOutput=False)
    kyb_d = nc.declare_dram_parameter("kyb", [KVD], F32, isOutput=False)
    cosq_d = nc.declare_dram_parameter("cosq", [S_LOC, HD // 2], F32, isOutput=False)
    sinq_d = nc.declare_dram_parameter("sinq", [S_LOC, HD // 2], F32, isOutput=False)
    cosk_d = nc.declare_dram_parameter("cosk", [S, HD // 2], F32, isOutput=False)
    sink_d = nc.declare_dram_parameter("sink", [S, HD // 2], F32, isOutput=False)
    gates_d = nc.declare_dram_parameter("gates", [H], F32, isOutput=False)
    ymb_d = nc.declare_dram_parameter("ymb", [YL], F32, isOutput=False)
    outT = nc.declare_dram_parameter("outT", [D, S_LOC], F32, isOutput=True)

    with tile.TileContext(nc) as tc:
        from contextlib import ExitStack
        with ExitStack() as ctx:
            # ---- persistent pools ----
            cpool = ctx.enter_context(tc.tile_pool(name="consts", bufs=1))
            yp = ctx.enter_context(tc.tile_pool(name="ypool", bufs=1))
            dramp = ctx.enter_context(
                tc.tile_pool(name="dscratch", bufs=1, space="DRAM"))
            QT_dram = dramp.tile([P, H, S_LOC], BF)
            KT_dram = dramp.tile([P, KVH, S], BF)
            V_dram = dramp.tile([P, TC, KVD], BF)

            # projection-phase transient pools
            lnp = tc.alloc_tile_pool(name="lnparams", bufs=1)
            xs = tc.alloc_tile_pool(name="xstream", bufs=3)
            work = tc.alloc_tile_pool(name="work", bufs=3)
            ropep = tc.alloc_tile_pool(name="rope", bufs=1)
            statp = tc.alloc_tile_pool(name="stats", bufs=3)
            psA = tc.alloc_tile_pool(name="psA", bufs=1, space="PSUM")

            # ---- constants ----
            ident = cpool.tile([P, P], F32)
            make_identity(nc, ident)
            ones_t = cpool.tile([P, P], BF)
            nc.vector.memset(ones_t, 1.0)
            eps_t = cpool.tile([P, 1], F32)
            nc.vector.memset(eps_t, EPS)
            gates_t = cpool.tile([P, H], F32)
            nc.gpsimd.dma_start(
                out=gates_t,
                in_=bass.AP(tensor=gates_d, offset=0, ap=[[0, P], [1, H]]))
            ymb_t = cpool.tile([P, 1], F32)
            nc.gpsimd.dma_start(
                out=ymb_t,
                in_=bass.AP(tensor=ymb_d, offset=0, ap=[[1, P], [0, 1]]))

            def bcast_vec(dram_h, n):
                t = lnp.tile([P, n], F32, tag=f"lnp_{dram_h.name}", bufs=1)
                nc.gpsimd.dma_start(
                    out=t, in_=bass.AP(tensor=dram_h, offset=0, ap=[[0, P], [1, n]]))
                return t

            qw_t = bcast_vec(qw_d, D)
            qb_t = bcast_vec(qb_d, D)
            kw_t = bcast_vec(kw_d, KVD)
            kb_t = bcast_vec(kb_d, KVD)
            kyw_t = bcast_vec(kyw_d, KVD)
            kyb_t = bcast_vec(kyb_d, KVD)

            YKT = yp.tile([P, KVH, YL], BF)
            YV = yp.tile([P, KVH, HD], BF)

            def rstd_from_mv(mv):
                r = statp.tile([P, 1], F32, tag="rstd")
                nc.scalar.activation(out=r, in_=mv[:, 1:2], func=AF.Sqrt,
                                     bias=eps_t, scale=1.0)
                nc.vector.reciprocal(out=r, in_=r)
                return r

            def transpose_to(zn, nheads, sb_dst, tok0):
                """PE-transpose zn's heads ([P tok, nheads*HD] f32) into
                head-major bf16 SBUF dst[:, hg4, tok0:tok0+P]."""
                for hg in range(nheads // 4):
                    tp = psA.tile([P, 4, P], F32, tag="tr", bufs=2)
                    for j in range(4):
                        hh = hg * 4 + j
                        nc.tensor.transpose(
                            tp[:, j, :], zn[:, hh * HD:(hh + 1) * HD], ident)
                    nc.vector.tensor_copy(
                        out=sb_dst[:, hg * 4:(hg + 1) * 4, tok0:tok0 + P],
                        in_=tp)

            def ln_apply_sb(dst, nchunks, w_t, b_t):
                """In-place LN on dst [P, nchunks*NQ] f32 (already in SBUF).
                Spread across engines: stats DVE, normalize ACT, bias GPSIMD."""
                mv = _ln_stats(nc, statp,
                               [dst[:, n * NQ:(n + 1) * NQ]
                                for n in range(nchunks)])
                rstd = rstd_from_mv(mv)
                negmr = statp.tile([P, 1], F32, tag="negmr")
                nc.vector.tensor_scalar(
                    out=negmr, in0=mv[:, 0:1], scalar1=rstd, scalar2=-1.0,
                    op0=ALU.mult, op1=ALU.mult)
                n_tot = nchunks * NQ
                nc.scalar.activation(
                    out=dst[:, :n_tot], in_=dst[:, :n_tot], func=AF.Identity,
                    scale=rstd, bias=negmr)
                nc.vector.tensor_mul(out=dst[:, :n_tot], in0=dst[:, :n_tot],
                                     in1=w_t)
                nc.gpsimd.tensor_add(out=dst[:, :n_tot], in0=dst[:, :n_tot],
                                     in1=b_t)

            def load_cs_table(cos_d, sin_d, nchunks):
                ct = ropep.tile([P, TC, HD // 2], F32, tag="costab", bufs=1)
                st = ropep.tile([P, TC, HD // 2], F32, tag="sintab", bufs=1)
                nc.sync.dma_start(
                    out=ct[:, :nchunks, :],
                    in_=cos_d[:, :].rearrange("(t p) f -> p t f", p=P))
                nc.sync.dma_start(
                    out=st[:, :nchunks, :],
                    in_=sin_d[:, :].rearrange("(t p) f -> p t f", p=P))
                return ct, st

            # =========================================================
            # Stage B: Q proj + LN + rope + transpose -> QT_sb -> spill
            # =========================================================
            qtbp = tc.alloc_tile_pool(name="qtb", bufs=1)
            QT_sb = qtbp.tile([P, H, S_LOC], BF)
            wB = tc.alloc_tile_pool(name="wB", bufs=1)
            wq_sb = wB.tile([P, DC, D], BF, tag="w")
            xq_ap = xTq[:, :].rearrange("(dc p) s -> p dc s", p=P)
            xt_first = xs.tile([P, DC, P], BF, tag="xt", name="xt_first")
            nc.sync.dma_start(out=xt_first, in_=xq_ap[:, :, 0:P])
            wq_ap = wq_d[:, :].rearrange("(dc p) n -> p dc n", p=P)
            for dc in range(DC):
                nc.sync.dma_start(out=wq_sb[:, dc, :], in_=wq_ap[:, dc, :])
            cosq_t, sinq_t = load_cs_table(cosq_d, sinq_d, TCL)
            for tcl in range(TCL):
                tok0 = tcl * P
                if tcl == 0:
                    xt_t = xt_first
                else:
                    xt_t = xs.tile([P, DC, P], BF, tag="xt")
                    nc.sync.dma_start(out=xt_t, in_=xq_ap[:, :, tok0:tok0 + P])
                q_ps = [psA.tile([P, NQ], F32, tag=f"acc{n}", name=f"qps{n}",
                                 bufs=1) for n in range(4)]
                for dc in range(DC):
                    for n in range(4):
                        nc.tensor.matmul(
                            q_ps[n][:], lhsT=xt_t[:, dc, :],
                            rhs=wq_sb[:, dc, n * NQ:(n + 1) * NQ],
                            start=(dc == 0), stop=(dc == DC - 1))
                qn = work.tile([P, D], F32, tag="work")
                for n in range(4):
                    nc.scalar.copy(out=qn[:, n * NQ:(n + 1) * NQ],
                                   in_=q_ps[n][:])
                ln_apply_sb(qn, 4, qw_t, qb_t)
                _rope_inplace(nc, ropep, qn, H, cosq_t[:, tcl, :],
                              sinq_t[:, tcl, :])
                transpose_to(qn, H, QT_sb, tok0)
            for h in range(H):
                nc.sync.dma_start(out=QT_dram[:, h, :], in_=QT_sb[:, h, :])
            wB.release()
            qtbp.release()

            # =========================================================
            # Stage A-K: K projection (full seq) + LN + rope -> spill
            # =========================================================
            x_ap = xT[:, :].rearrange("(dc p) s -> p dc s", p=P)
            ktbp = tc.alloc_tile_pool(name="ktb", bufs=1)
            KT_sb = ktbp.tile([P, KVH, S], BF)
            wAk = tc.alloc_tile_pool(name="wAk", bufs=1)
            wk_sb = wAk.tile([P, DC, KVD], BF, tag="w")
            xt_firstk = xs.tile([P, DC, P], BF, tag="xt", name="xt_firstk")
            nc.sync.dma_start(out=xt_firstk, in_=x_ap[:, :, 0:P])
            wk_ap = wkv_d[:, :KVD].rearrange("(dc p) n -> p dc n", p=P)
            for dc in range(DC):
                nc.sync.dma_start(out=wk_sb[:, dc, :], in_=wk_ap[:, dc, :])
            cosk_t, sink_t = load_cs_table(cosk_d, sink_d, TC)
            for tci in range(TC):
                tok0 = tci * P
                if tci == 0:
                    xt_t = xt_firstk
                else:
                    xt_t = xs.tile([P, DC, P], BF, tag="xt")
                    nc.sync.dma_start(out=xt_t, in_=x_ap[:, :, tok0:tok0 + P])
                k_ps = [psA.tile([P, NQ], F32, tag=f"acc{n}", name=f"kps{n}",
                                 bufs=1) for n in range(2)]
                for dc in range(DC):
                    for n in range(2):
                        nc.tensor.matmul(
                            k_ps[n][:], lhsT=xt_t[:, dc, :],
                            rhs=wk_sb[:, dc, n * NQ:(n + 1) * NQ],
                            start=(dc == 0), stop=(dc == DC - 1))
                kn = work.tile([P, KVD], F32, tag="work")
                for n in range(2):
                    nc.scalar.copy(out=kn[:, n * NQ:(n + 1) * NQ],
                                   in_=k_ps[n][:])
                ln_apply_sb(kn, 2, kw_t, kb_t)
                _rope_inplace(nc, ropep, kn, KVH, cosk_t[:, tci, :],
                              sink_t[:, tci, :])
                transpose_to(kn, KVH, KT_sb, tok0)
            for kv in range(KVH):
                nc.sync.dma_start(out=KT_dram[:, kv, :], in_=KT_sb[:, kv, :])
            wAk.release()
            ktbp.release()

            # =========================================================
            # Stage A-V: V projection (full seq) -> spill
            # =========================================================
            vbp = tc.alloc_tile_pool(name="vb", bufs=1)
            V_sb = vbp.tile([P, TC, KVD], BF)
            wAv = tc.alloc_tile_pool(name="wAv", bufs=1)
            wv_sb = wAv.tile([P, DC, KVD], BF, tag="w")
            xt_firstv = xs.tile([P, DC, P], BF, tag="xt", name="xt_firstv")
            nc.sync.dma_start(out=xt_firstv, in_=x_ap[:, :, 0:P])
            wv_ap = wkv_d[:, KVD:].rearrange("(dc p) n -> p dc n", p=P)
            for dc in range(DC):
                nc.sync.dma_start(out=wv_sb[:, dc, :], in_=wv_ap[:, dc, :])
            for tci in range(TC):
                tok0 = tci * P
                if tci == 0:
                    xt_t = xt_firstv
                else:
                    xt_t = xs.tile([P, DC, P], BF, tag="xt")
                    nc.sync.dma_start(out=xt_t, in_=x_ap[:, :, tok0:tok0 + P])
                v_ps = [psA.tile([P, NQ], F32, tag=f"acc{n}", name=f"vps{n}",
                                 bufs=1) for n in range(2)]
                for dc in range(DC):
                    for n in range(2):
                        nc.tensor.matmul(
                            v_ps[n][:], lhsT=xt_t[:, dc, :],
                            rhs=wv_sb[:, dc, n * NQ:(n + 1) * NQ],
                            start=(dc == 0), stop=(dc == DC - 1))
                for n in range(2):
                    nc.scalar.copy(
                        out=V_sb[:, tci, n * NQ:(n + 1) * NQ], in_=v_ps[n][:])
            for kv in range(KVH):
                nc.sync.dma_start(
                    out=V_dram[:, :, kv * HD:(kv + 1) * HD],
                    in_=V_sb[:, :, kv * HD:(kv + 1) * HD])
            wAv.release()
            vbp.release()

            # =========================================================
            # Stage C: y projections -> YKT (LN, no rope), YV (SBUF)
            # =========================================================
            wC = tc.alloc_tile_pool(name="wC", bufs=1)
            wkvy_sb = wC.tile([P, YDC, 2 * KVD], BF, tag="w")
            nc.sync.dma_start(
                out=wkvy_sb, in_=wkvy_d[:, :].rearrange("(dc p) n -> p dc n", p=P))
            yt_t = xs.tile([P, YDC, YL], BF, tag="yt", bufs=1)
            nc.sync.dma_start(
                out=yt_t, in_=yT[:, :].rearrange("(dc p) s -> p dc s", p=P))
            yk_ps = [psA.tile([P, NQ], F32, tag=f"acc{n}", name=f"ykps{n}",
                              bufs=1) for n in range(2)]
            yv_ps = [psA.tile([P, NQ], F32, tag=f"acc{n+2}", name=f"yvps{n}",
                              bufs=1) for n in range(2)]
            for dc in range(YDC):
                for n in range(2):
                    nc.tensor.matmul(
                        yk_ps[n][:], lhsT=yt_t[:, dc, :],
                        rhs=wkvy_sb[:, dc, n * NQ:(n + 1) * NQ],
                        start=(dc == 0), stop=(dc == YDC - 1))
                for n in range(2):
                    nc.tensor.matmul(
                        yv_ps[n][:], lhsT=yt_t[:, dc, :],
                        rhs=wkvy_sb[:, dc, KVD + n * NQ:KVD + (n + 1) * NQ],
                        start=(dc == 0), stop=(dc == YDC - 1))
            for n in range(2):
                nc.scalar.copy(
                    out=YV[:, 4 * n:4 * (n + 1), :], in_=yv_ps[n][:])
            ykn = work.tile([P, KVD], F32, tag="work")
            for n in range(2):
                nc.scalar.copy(out=ykn[:, n * NQ:(n + 1) * NQ],
                               in_=yk_ps[n][:])
            ln_apply_sb(ykn, 2, kyw_t, kyb_t)
            for hg in range(2):
                tp = psA.tile([P, 4, P], F32, tag="tr", bufs=2)
                for j in range(4):
                    kv = hg * 4 + j
                    nc.tensor.transpose(
                        tp[:, j, :], ykn[:, kv * HD:(kv + 1) * HD], ident)
                nc.vector.tensor_copy(
                    out=YKT[:, hg * 4:(hg + 1) * 4, :], in_=tp)
            wC.release()
            statp.release()
            ropep.release()
            work.release()
            xs.release()
            lnp.release()
            psA.release()

            # =========================================================
            # Stage D: attention per (head, q-chunk)
            # =========================================================
            mgp = tc.alloc_tile_pool(name="merged", bufs=1)
            ktrp = tc.alloc_tile_pool(name="ktr", bufs=1)
            vrp = tc.alloc_tile_pool(name="vr", bufs=1)
            qtrp = tc.alloc_tile_pool(name="qtr", bufs=1)
            ep = tc.alloc_tile_pool(name="escores", bufs=2)
            eyp = tc.alloc_tile_pool(name="eyscores", bufs=2)
            rcp = tc.alloc_tile_pool(name="recips", bufs=2)
            psD = tc.alloc_tile_pool(name="psD", bufs=2, space="PSUM")
            merged = mgp.tile([P, H, S_LOC], BF)      # merged^T feature-major
            KTr = ktrp.tile([P, KVH, S], BF)
            Vr = vrp.tile([P, TC, KVD], BF)
            QTr = qtrp.tile([P, H, S_LOC], BF)
            for kv in range(KVH):
                nc.sync.dma_start(out=KTr[:, kv, :], in_=KT_dram[:, kv, :])
                nc.sync.dma_start(
                    out=Vr[:, :, kv * HD:(kv + 1) * HD],
                    in_=V_dram[:, :, kv * HD:(kv + 1) * HD])
            for h in range(H):
                nc.sync.dma_start(out=QTr[:, h, :], in_=QT_dram[:, h, :])
            for h in range(H):
                kv = h // NREP
                for qc in range(QCN):
                    q0 = qc * NQ
                    qt_t = QTr[:, h, q0:q0 + NQ]
                    # cross-attention first: short chain, overlaps the
                    # self-attention pipeline instead of serializing its tail
                    sy_ps = psD.tile([P, NQ], F32, tag="sy", bufs=1, name="sy_ps")
                    nc.tensor.matmul(
                        sy_ps[:], lhsT=YKT[:, kv, :], rhs=qt_t,
                        start=True, stop=True, skip_group_check=True)
                    Ey_t = eyp.tile([P, NQ], BF, tag="Ey")
                    nc.scalar.activation(
                        out=Ey_t, in_=sy_ps[:], func=AF.Exp, scale=SCALE,
                        bias=ymb_t)
                    dy_ps = psD.tile([P, NQ], F32, tag="cross", bufs=1, name="dy_ps")
                    nc.tensor.matmul(
                        dy_ps[:], lhsT=ones_t, rhs=Ey_t,
                        start=True, stop=True, skip_group_check=True)
                    oy_ps = psD.tile([P, NQ], F32, tag="cross", bufs=1, name="oy_ps")
                    nc.tensor.matmul(
                        oy_ps[:], lhsT=YV[:, kv, :], rhs=Ey_t,
                        start=True, stop=True, skip_group_check=True)
                    rec_y = rcp.tile([P, NQ], F32, tag="recy")
                    nc.vector.reciprocal(out=rec_y, in_=dy_ps[:])
                    t1 = rcp.tile([P, NQ], F32, tag="t1")
                    nc.vector.scalar_tensor_tensor(
                        out=t1, in0=oy_ps[:], scalar=gates_t[:, h:h + 1],
                        in1=rec_y, op0=ALU.mult, op1=ALU.mult)
                    o_ps = psD.tile([P, NQ], F32, tag="o", bufs=1)
                    d_ps = psD.tile([P, NQ], F32, tag="d", bufs=1)
                    E_t = ep.tile([P, TC, NQ], BF, tag="E", bufs=3)
                    for kp in range(TC // 2):
                        s_ps = psD.tile([P, 2, NQ], F32, tag="s", bufs=2)
                        for j in range(2):
                            kc = kp * 2 + j
                            nc.tensor.matmul(
                                s_ps[:, j, :],
                                lhsT=KTr[:, kv, kc * P:(kc + 1) * P],
                                rhs=qt_t, start=True, stop=True,
                                skip_group_check=True)
                        nc.scalar.activation(
                            out=E_t[:, kp * 2:kp * 2 + 2, :], in_=s_ps[:],
                            func=AF.Exp, scale=SCALE)
                        # pairwise E sum on DVE halves the denominator matmuls
                        esum = eyp.tile([P, NQ], BF, tag="es", bufs=4,
                                        name="esum")
                        nc.vector.tensor_add(
                            out=esum, in0=E_t[:, kp * 2, :],
                            in1=E_t[:, kp * 2 + 1, :])
                        nc.tensor.matmul(
                            d_ps[:], lhsT=ones_t, rhs=esum,
                            start=(kp == 0), stop=(kp == TC // 2 - 1),
                            skip_group_check=True)
                        for j in range(2):
                            kc = kp * 2 + j
                            nc.tensor.matmul(
                                o_ps[:], lhsT=Vr[:, kc, kv * HD:(kv + 1) * HD],
                                rhs=E_t[:, kc, :],
                                start=(kc == 0), stop=(kc == TC - 1),
                                skip_group_check=True)
                    # merge: O/denom + tanh(gate)*Oy/denom_y (t1 ready above)
                    rec = rcp.tile([P, NQ], F32, tag="rec")
                    nc.vector.reciprocal(out=rec, in_=d_ps[:])
                    t0 = rcp.tile([P, NQ], F32, tag="t0")
                    nc.vector.tensor_mul(out=t0, in0=o_ps[:], in1=rec)
                    nc.vector.tensor_add(
                        out=merged[:, h, q0:q0 + NQ], in0=t0, in1=t1)
            rcp.release()
            eyp.release()
            ep.release()
            qtrp.release()
            vrp.release()
            ktrp.release()
            psD.release()

            # =========================================================
            # Stage E: output projection out^T = wo^T @ merged^T
            # =========================================================
            wE = tc.alloc_tile_pool(name="wE", bufs=1)
            outp = tc.alloc_tile_pool(name="outtiles", bufs=3)
            psE = tc.alloc_tile_pool(name="psE", bufs=2, space="PSUM")
            wo_sb = wE.tile([P, DC, D], BF, tag="w")
            wo_ap = wo_d[:, :].rearrange("(hc p) n -> p hc n", p=P)
            for hc in range(DC):
                nc.sync.dma_start(out=wo_sb[:, hc, :], in_=wo_ap[:, hc, :])
            for qc in range(QCN):
                q0 = qc * NQ
                for oc in range(DC):
                    out_ps = psE.tile([P, NQ], F32, tag="oout")
                    for hc in range(DC):
                        nc.tensor.matmul(
                            out_ps[:],
                            lhsT=wo_sb[:, hc, oc * P:(oc + 1) * P],
                            rhs=merged[:, hc, q0:q0 + NQ],
                            start=(hc == 0), stop=(hc == DC - 1))
                    out_t = outp.tile([P, NQ], F32, tag="outt")
                    nc.vector.tensor_copy(out=out_t, in_=out_ps[:])
                    nc.sync.dma_start(
                        out=outT[oc * P:(oc + 1) * P, q0:q0 + NQ],
                        in_=out_t)
            psE.release()
            outp.release()
            wE.release()
            mgp.release()

    _split_dma_waits(nc)
    return nc


def _prep_shared(x, y, freqs_cos, freqs_sin, y_mask, wq, wk, wv, wk_y, wv_y,
                 wo, q_w, q_b, k_w, k_b, ky_w, ky_b, gate):
    f32 = np.float32
    shared = {
        "wq": np.ascontiguousarray(np.asarray(wq, f32).astype(BF16)),
        "wkv": np.ascontiguousarray(
            np.concatenate([np.asarray(wk, f32), np.asarray(wv, f32)],
                           axis=1).astype(BF16)),
        "wkvy": np.ascontiguousarray(
            np.concatenate([np.asarray(wk_y, f32), np.asarray(wv_y, f32)],
                           axis=1).astype(BF16)),
        "wo": np.ascontiguousarray(np.asarray(wo, f32).astype(BF16)),
        "qw": np.ascontiguousarray(np.asarray(q_w, f32)),
        "qb": np.ascontiguousarray(np.asarray(q_b, f32)),
        "kw": np.ascontiguousarray(np.asarray(k_w, f32)),
        "kb": np.ascontiguousarray(np.asarray(k_b, f32)),
        "kyw": np.ascontiguousarray(np.asarray(ky_w, f32)),
        "kyb": np.ascontiguousarray(np.asarray(ky_b, f32)),
        "cosk": np.ascontiguousarray(np.asarray(freqs_cos, f32)),
        "sink": np.ascontiguousarray(np.asarray(freqs_sin, f32)),
        "gates": np.ascontiguousarray(np.tanh(np.asarray(gate, f32))),
    }
    per_core = []
    for c in range(8):
        b, hf = c // 2, c % 2
        sl = slice(hf * S_LOC, (hf + 1) * S_LOC)
        xTb = np.asarray(x[b], f32).T.astype(BF16)
        m = dict(shared)
        m["xT"] = np.ascontiguousarray(xTb)
        m["xTq"] = np.ascontiguousarray(xTb[:, sl])
        m["yT"] = np.ascontiguousarray(np.asarray(y[b], f32).T.astype(BF16))
        m["cosq"] = np.ascontiguousarray(np.asarray(freqs_cos, f32)[sl])
        m["sinq"] = np.ascontiguousarray(np.asarray(freqs_sin, f32)[sl])
        m["ymb"] = np.where(np.asarray(y_mask[b]), 0.0, -1e9).astype(f32)
        per_core.append(m)
    return per_core


def kernel(**inputs):
    if "nc" not in _CACHED:
        _CACHED["nc"] = build_program()
    nc = _CACHED["nc"]
    in_maps = _prep_shared(
        inputs["x"], inputs["y"], inputs["freqs_cos"], inputs["freqs_sin"],
        inputs["y_mask"], inputs["wq"], inputs["wk"], inputs["wv"],
        inputs["wk_y"], inputs["wv_y"], inputs["wo"], inputs["q_w"],
        inputs["q_b"], inputs["k_w"], inputs["k_b"], inputs["ky_w"],
        inputs["ky_b"], inputs["gate"])
    res = run_bass_kernel_spmd(nc, in_maps, core_ids=list(range(8)))
    global LAST_EXEC_NS
    LAST_EXEC_NS = res.exec_time_ns
    out = np.zeros((B, S, D), np.float32)
    for c in range(8):
        b, hf = c // 2, c % 2
        out[b, hf * S_LOC:(hf + 1) * S_LOC, :] = res.results[c]["outT"].T
    return out


if __name__ == "__main__":
    nc = build_program()
    print("program built OK")



# revision 3
# speedup vs baseline: 1.2508x; 1.2508x over previous
"""Trainium2 Bass kernel for nn_Attention_35734127903400 — v2.

Token-sharded (core c: batch c//2, seq half c%2) with pair-AllGather K/V
dedup: each core projects K/V only for its local 1024 tokens, exchanges
halves with its pair core via two HBM AllGathers (K first, then V) that
run on the collective cores, overlapped with Q/y projection. Key order
in KTr/Vr is global (gather slot r = token half r); on odd cores the
slot-0 DMA overwrites the locally-projected half with the peer's data
and the local half lands in the upper columns — same program on every
core, no divergence.

K^T/V/Q^T stay SBUF-resident (no DRAM spill). Attention is software-
pipelined: out-matmuls trail scores by one head (tolerating the late V
gather); softmax denominators fold 4-wide on DVE into 4 ones-matmuls;
output projection for q-chunk 0 interleaves with attention q-chunk 1,
with wo streamed in eighths; merged output aliases into dead Q^T
slices. Collectives issue from Pool (gpsimd), whose sequencer blocks on
the bounce-DMA waits — so LN bias-adds run on DVE/Pool split such that
nothing downstream queues behind a blocked Pool sequencer.
"""

import numpy as np
import ml_dtypes

import concourse.bass as bass
import concourse.mybir as mybir
import concourse.tile as tile
from concourse.bass_utils import run_bass_kernel_spmd
from concourse.masks import make_identity

BF16 = ml_dtypes.bfloat16
F32 = mybir.dt.float32
BF = mybir.dt.bfloat16

P = 128
B, S, D = 4, 2048, 2048
H, KVH = 16, 8
HD = 128
NREP = 2
YL, YD = 128, 1024
EPS = 1e-5
S_LOC = S // 2
DC = D // P          # 16 contraction chunks for D
YDC = YD // P        # 8
TC = S // P          # 16 key chunks (full seq)
TCL = S_LOC // P     # 8 local token chunks
NQ = 512             # q-free chunk (one PSUM bank of f32)
QCN = S_LOC // NQ    # 2
KVD = KVH * HD       # 1024
SCALE = 1.0 / float(np.sqrt(np.float32(HD)))
AF = mybir.ActivationFunctionType
ALU = mybir.AluOpType
REP_GROUPS = [[0, 1], [2, 3], [4, 5], [6, 7]]

_CACHED = {}
LAST_EXEC_NS = None


def _ln_stats(nc, statp, ps_chunks):
    """bn_stats over a list of [P, 512] chunks -> mv [P, 2] (mean, var)."""
    nchunks = len(ps_chunks)
    stats = statp.tile([P, nchunks, 6], F32, tag="bnstats")
    for i, ps in enumerate(ps_chunks):
        nc.vector.bn_stats(out=stats[:, i, :], in_=ps[:])
    mv = statp.tile([P, 2], F32, tag="bnaggr")
    nc.vector.bn_aggr(out=mv, in_=stats)
    return mv


def _rope_inplace(nc, ropep, zn, nheads, cos_t, sin_t, mul_eng):
    """In-place rope on zn [P, nheads*HD] f32; cos/sin [P, 64] f32.
    The three products run on mul_eng (DVE or Pool), sub/add on DVE."""
    zv = zn.rearrange("p (h f two) -> p h f two", h=nheads, two=2)
    re = zv[:, :, :, 0]
    im = zv[:, :, :, 1]
    shp = (P, nheads, HD // 2)
    cb = cos_t[:, None, :].to_broadcast(shp)
    sb = sin_t[:, None, :].to_broadcast(shp)
    t1 = ropep.tile([P, nheads, HD // 2], F32, tag="rp1")
    t2 = ropep.tile([P, nheads, HD // 2], F32, tag="rp2")
    t3 = ropep.tile([P, nheads, HD // 2], F32, tag="rp3")
    mul_eng.tensor_mul(out=t1, in0=re, in1=cb)     # re*c
    mul_eng.tensor_mul(out=t2, in0=re, in1=sb)     # re*s
    nc.vector.tensor_mul(out=t3, in0=im, in1=sb)   # im*s
    nc.vector.tensor_sub(out=re, in0=t1, in1=t3)   # re' = re*c - im*s
    mul_eng.tensor_mul(out=t3, in0=im, in1=cb)     # im*c
    nc.vector.tensor_add(out=im, in0=t2, in1=t3)   # im' = re*s + im*c


def _split_dma_waits(nc, max_waits=1):
    """Hoist excess sync waits onto preceding same-engine single-wait NoOps
    (walrus per-instruction structs have 1-2 wait slots)."""
    n_split = 0
    for f in nc.m.functions:
        for blk in f.blocks:
            insts = list(blk.instructions)
            out = []
            changed = False
            for ins in insts:
                si = ins.sync_info
                if (si is not None and si.on_wait
                        and len(si.on_wait) > max_waits):
                    waits = list(si.on_wait)
                    for wi, w in enumerate(waits[:-max_waits]):
                        out.append(mybir.InstNoOp(
                            name=f"{ins.name}-wsplit{wi}", engine=ins.engine,
                            sync_info=mybir.SyncInfo(on_wait=[w],
                                                     on_update=[])))
                    ins.sync_info = mybir.SyncInfo(
                        on_wait=waits[-max_waits:],
                        on_update=list(si.on_update))
                    changed = True
                    n_split += 1
                out.append(ins)
            if changed:
                blk.instructions = out
    return n_split


def build_program():
    nc = bass.Bass()

    # ---- I/O (all per-core local; key order handled host-side) ----
    xTq = nc.declare_dram_parameter("xTq", [D, S_LOC], BF, isOutput=False)
    yT = nc.declare_dram_parameter("yT", [YD, YL], BF, isOutput=False)
    wq_d = nc.declare_dram_parameter("wq", [D, D], BF, isOutput=False)
    wkv_d = nc.declare_dram_parameter("wkv", [D, 2 * KVD], BF, isOutput=False)
    wkvy_d = nc.declare_dram_parameter("wkvy", [YD, 2 * KVD], BF, isOutput=False)
    wo_d = nc.declare_dram_parameter("wo", [D, D], BF, isOutput=False)
    qw_d = nc.declare_dram_parameter("qw", [D], F32, isOutput=False)
    qb_d = nc.declare_dram_parameter("qb", [D], F32, isOutput=False)
    kw_d = nc.declare_dram_parameter("kw", [KVD], F32, isOutput=False)
    kb_d = nc.declare_dram_parameter("kb", [KVD], F32, isOutput=False)
    kyw_d = nc.declare_dram_parameter("kyw", [KVD], F32, isOutput=False)
    kyb_d = nc.declare_dram_parameter("kyb", [KVD], F32, isOutput=False)
    cosq_d = nc.declare_dram_parameter("cosq", [S_LOC, HD // 2], F32, isOutput=False)
    sinq_d = nc.declare_dram_parameter("sinq", [S_LOC, HD // 2], F32, isOutput=False)
    gates_d = nc.declare_dram_parameter("gates", [H], F32, isOutput=False)
    ymb_d = nc.declare_dram_parameter("ymb", [YL], F32, isOutput=False)
    outT = nc.declare_dram_parameter("outT", [D, S_LOC], F32, isOutput=True)

    with tile.TileContext(nc) as tc:
        from contextlib import ExitStack
        with ExitStack() as ctx:
            # ---- persistent pools ----
            cpool = ctx.enter_context(tc.tile_pool(name="consts", bufs=1))
            yp = ctx.enter_context(tc.tile_pool(name="ypool", bufs=1))
            ktp = ctx.enter_context(tc.tile_pool(name="ktpool", bufs=1))
            qtp = ctx.enter_context(tc.tile_pool(name="qtpool", bufs=1))
            dramp = ctx.enter_context(
                tc.tile_pool(name="dscratch", bufs=1, space="DRAM"))

            KTr = ktp.tile([P, KVH, S], BF)       # key-major K^T, global order
            QT = qtp.tile([P, H, S_LOC], BF)      # Q^T; merged aliases in later
            YKT = yp.tile([P, KVH, YL], BF)
            YV = yp.tile([P, KVH, HD], BF)

            # one K gather; V gather split by feature half (kv-heads 0-3 /
            # 4-7) so each half lands just ahead of the heads needing it
            kin = dramp.tile([P, KVH, S_LOC], BF)
            kout = dramp.tile([2, P, KVH, S_LOC], BF)
            vin1 = dramp.tile([P, TCL, KVD // 2], BF)
            vin2 = dramp.tile([P, TCL, KVD // 2], BF)
            vout1 = dramp.tile([2, P, TCL, KVD // 2], BF)
            vout2 = dramp.tile([2, P, TCL, KVD // 2], BF)

            # projection-phase transient pools
            xs = tc.alloc_tile_pool(name="xstream", bufs=3)
            work = tc.alloc_tile_pool(name="work", bufs=3)
            ropep = tc.alloc_tile_pool(name="rope", bufs=1)
            statp = tc.alloc_tile_pool(name="stats", bufs=3)
            psA = tc.alloc_tile_pool(name="psA", bufs=1, space="PSUM")

            # ---- constants ----
            ident = cpool.tile([P, P], F32)
            make_identity(nc, ident)
            ones_t = cpool.tile([P, P], BF)
            nc.vector.memset(ones_t, 1.0)
            eps_t = cpool.tile([P, 1], F32)
            nc.vector.memset(eps_t, EPS)
            gates_t = cpool.tile([P, H], F32)
            nc.gpsimd.dma_start(
                out=gates_t,
                in_=bass.AP(tensor=gates_d, offset=0, ap=[[0, P], [1, H]]))
            ymb_t = cpool.tile([P, 1], F32)
            nc.gpsimd.dma_start(
                out=ymb_t,
                in_=bass.AP(tensor=ymb_d, offset=0, ap=[[1, P], [0, 1]]))

            cosq_t = ropep.tile([P, TCL, HD // 2], F32, tag="costab", bufs=1)
            sinq_t = ropep.tile([P, TCL, HD // 2], F32, tag="sintab", bufs=1)
            nc.sync.dma_start(
                out=cosq_t,
                in_=cosq_d[:, :].rearrange("(t p) f -> p t f", p=P))
            nc.sync.dma_start(
                out=sinq_t,
                in_=sinq_d[:, :].rearrange("(t p) f -> p t f", p=P))

            def bcast_vec(pool, dram_h, n):
                t = pool.tile([P, n], F32, tag=f"lnp_{dram_h.name}", bufs=1)
                nc.gpsimd.dma_start(
                    out=t, in_=bass.AP(tensor=dram_h, offset=0, ap=[[0, P], [1, n]]))
                return t

            def rstd_from_mv(mv):
                r = statp.tile([P, 1], F32, tag="rstd")
                nc.scalar.activation(out=r, in_=mv[:, 1:2], func=AF.Sqrt,
                                     bias=eps_t, scale=1.0)
                nc.vector.reciprocal(out=r, in_=r)
                return r

            def transpose_to(zn, nheads, sb_dst, tok0):
                """PE-transpose zn's heads into head-major bf16 dst."""
                for hg in range(nheads // 4):
                    tp = psA.tile([P, 4, P], F32, tag="tr", bufs=2)
                    for j in range(4):
                        hh = hg * 4 + j
                        nc.tensor.transpose(
                            tp[:, j, :], zn[:, hh * HD:(hh + 1) * HD], ident)
                    nc.scalar.copy(
                        out=sb_dst[:, hg * 4:(hg + 1) * 4, tok0:tok0 + P],
                        in_=tp)

            def ln_apply_mv(dst, nchunks, mv, w_t, b_t, badd_eng):
                rstd = rstd_from_mv(mv)
                negmr = statp.tile([P, 1], F32, tag="negmr")
                nc.gpsimd.tensor_scalar(
                    out=negmr, in0=mv[:, 0:1], scalar1=rstd, scalar2=-1.0,
                    op0=ALU.mult, op1=ALU.mult)
                n_tot = nchunks * NQ
                nc.scalar.activation(
                    out=dst[:, :n_tot], in_=dst[:, :n_tot], func=AF.Identity,
                    scale=rstd, bias=negmr)
                nc.vector.tensor_mul(out=dst[:, :n_tot], in0=dst[:, :n_tot],
                                     in1=w_t)
                badd_eng.tensor_add(out=dst[:, :n_tot], in0=dst[:, :n_tot],
                                    in1=b_t)

            def ln_apply_sb(dst, nchunks, w_t, b_t, badd_eng):
                """In-place LN on dst [P, nchunks*NQ] f32 (SBUF).
                stats DVE, normalize ACT, w-mul DVE, bias on badd_eng."""
                mv = _ln_stats(nc, statp,
                               [dst[:, n * NQ:(n + 1) * NQ]
                                for n in range(nchunks)])
                rstd = rstd_from_mv(mv)
                negmr = statp.tile([P, 1], F32, tag="negmr")
                nc.vector.tensor_scalar(
                    out=negmr, in0=mv[:, 0:1], scalar1=rstd, scalar2=-1.0,
                    op0=ALU.mult, op1=ALU.mult)
                n_tot = nchunks * NQ
                nc.scalar.activation(
                    out=dst[:, :n_tot], in_=dst[:, :n_tot], func=AF.Identity,
                    scale=rstd, bias=negmr)
                nc.vector.tensor_mul(out=dst[:, :n_tot], in0=dst[:, :n_tot],
                                     in1=w_t)
                badd_eng.tensor_add(out=dst[:, :n_tot], in0=dst[:, :n_tot],
                                    in1=b_t)

            x_ap = xTq[:, :].rearrange("(dc p) s -> p dc s", p=P)

            # =========================================================
            # Stage K: local-half K proj + LN + rope + transpose, then
            # one pair-AllGather into KTr (global key order). wq's first
            # half prefetches during the K loop (DMA slack).
            # =========================================================
            wBa = tc.alloc_tile_pool(name="wBa", bufs=1)
            wqA = wBa.tile([P, DC // 4, D], BF, tag="w")
            wq_ap = wq_d[:, :].rearrange("(dc p) n -> p dc n", p=P)
            lnpK = tc.alloc_tile_pool(name="lnpK", bufs=1)
            kw_t = bcast_vec(lnpK, kw_d, KVD)
            kb_t = bcast_vec(lnpK, kb_d, KVD)
            wAk = tc.alloc_tile_pool(name="wAk", bufs=1)
            wk_sb = wAk.tile([P, DC, KVD], BF, tag="w")
            wk_ap = wkv_d[:, :KVD].rearrange("(dc p) n -> p dc n", p=P)
            # queue order: 2 wk chunks, first x tile, rest of wk, wq half
            for dc in range(2):
                nc.sync.dma_start(out=wk_sb[:, dc, :], in_=wk_ap[:, dc, :])
            xt_first = xs.tile([P, DC, P], BF, tag="xt", name="xt_firstk")
            nc.sync.dma_start(out=xt_first, in_=x_ap[:, :, 0:P])
            for dc in range(2, DC):
                nc.sync.dma_start(out=wk_sb[:, dc, :], in_=wk_ap[:, dc, :])
            for dc in range(DC // 4):
                nc.sync.dma_start(out=wqA[:, dc, :], in_=wq_ap[:, dc, :])
            kns = []
            for tci in range(TCL):
                tok0 = tci * P
                if tci == 0:
                    xt_t = xt_first
                else:
                    xt_t = xs.tile([P, DC, P], BF, tag="xt")
                    nc.sync.dma_start(out=xt_t, in_=x_ap[:, :, tok0:tok0 + P])
                k_ps = [psA.tile([P, NQ], F32, tag=f"acc{n}", name=f"kps{n}",
                                 bufs=2) for n in range(2)]
                for dc in range(DC):
                    for n in range(2):
                        nc.tensor.matmul(
                            k_ps[n][:], lhsT=xt_t[:, dc, :],
                            rhs=wk_sb[:, dc, n * NQ:(n + 1) * NQ],
                            start=(dc == 0), stop=(dc == DC - 1))
                kn = work.tile([P, KVD], F32, tag="work")
                for n in range(2):
                    nc.scalar.copy(out=kn[:, n * NQ:(n + 1) * NQ],
                                   in_=k_ps[n][:])
                ln_apply_sb(kn, 2, kw_t, kb_t, nc.vector)
                _rope_inplace(nc, ropep, kn, KVH, cosq_t[:, tci, :],
                              sinq_t[:, tci, :], nc.gpsimd)
                # transposes trail by two chunks so PE never waits on the
                # LN/rope chain (its latency exceeds one PE iteration)
                if tci > 1:
                    transpose_to(kns[tci - 2], KVH, KTr, tok0 - 2 * P)
                kns.append(kn)
            for i in (TCL - 2, TCL - 1):
                transpose_to(kns[i], KVH, KTr, i * P)
            nc.gpsimd.dma_start(out=kin[:, :, :], in_=KTr[:, :, 0:S_LOC])
            nc.gpsimd.collective_compute(
                "AllGather", ALU.bypass, replica_groups=REP_GROUPS,
                ins=[kin[:, :, :].opt()], outs=[kout[:, :, :, :].opt()])
            wAk.release()
            lnpK.release()

            # =========================================================
            # Stage V: local-half V proj, streamed to the bounce buffers
            # per token chunk; two V-half gathers follow the K gather.
            # =========================================================
            wAv = tc.alloc_tile_pool(name="wAv", bufs=1)
            wC = tc.alloc_tile_pool(name="wC", bufs=1)
            vst = tc.alloc_tile_pool(name="vstream", bufs=2)
            wv_sb = wAv.tile([P, DC, KVD], BF, tag="w")
            wv_ap = wkv_d[:, KVD:].rearrange("(dc p) n -> p dc n", p=P)
            for dc in range(2):
                nc.sync.dma_start(out=wv_sb[:, dc, :], in_=wv_ap[:, dc, :])
            xt_firstv = xs.tile([P, DC, P], BF, tag="xt", name="xt_firstv")
            nc.sync.dma_start(out=xt_firstv, in_=x_ap[:, :, 0:P])
            for dc in range(2, DC):
                nc.sync.dma_start(out=wv_sb[:, dc, :], in_=wv_ap[:, dc, :])
            # y-projection weights stream during the V loop; y-proj runs at
            # the end of this stage so attention can start right after Q
            wkvy_sb = wC.tile([P, YDC, 2 * KVD], BF, tag="w")
            wkvy_ap = wkvy_d[:, :].rearrange("(dc p) n -> p dc n", p=P)
            yt_t = xs.tile([P, YDC, YL], BF, tag="yt", bufs=1)
            nc.sync.dma_start(
                out=yt_t, in_=yT[:, :].rearrange("(dc p) s -> p dc s", p=P))
            for dc in range(YDC):
                nc.sync.dma_start(out=wkvy_sb[:, dc, :], in_=wkvy_ap[:, dc, :])
            for tci in range(TCL):
                tok0 = tci * P
                if tci == 0:
                    xt_t = xt_firstv
                else:
                    xt_t = xs.tile([P, DC, P], BF, tag="xt")
                    nc.sync.dma_start(out=xt_t, in_=x_ap[:, :, tok0:tok0 + P])
                v_ps = [psA.tile([P, NQ], F32, tag=f"acc{n}", name=f"vps{n}",
                                 bufs=2) for n in range(2)]
                for dc in range(DC):
                    for n in range(2):
                        nc.tensor.matmul(
                            v_ps[n][:], lhsT=xt_t[:, dc, :],
                            rhs=wv_sb[:, dc, n * NQ:(n + 1) * NQ],
                            start=(dc == 0), stop=(dc == DC - 1))
                vt = vst.tile([P, KVD], BF, tag="vt", bufs=4)
                for n in range(2):
                    nc.scalar.copy(out=vt[:, n * NQ:(n + 1) * NQ],
                                   in_=v_ps[n][:])
                nc.gpsimd.dma_start(out=vin1[:, tci, :],
                                     in_=vt[:, 0:KVD // 2])
                nc.gpsimd.dma_start(out=vin2[:, tci, :],
                                    in_=vt[:, KVD // 2:KVD])
            nc.gpsimd.collective_compute(
                "AllGather", ALU.bypass, replica_groups=REP_GROUPS,
                ins=[vin1[:, :, :].opt()], outs=[vout1[:, :, :, :].opt()])
            nc.gpsimd.collective_compute(
                "AllGather", ALU.bypass, replica_groups=REP_GROUPS,
                ins=[vin2[:, :, :].opt()], outs=[vout2[:, :, :, :].opt()])
            # ---- y projections -> YKT (LN, no rope), YV ----
            vst.release()
            lnpY = tc.alloc_tile_pool(name="lnpY", bufs=1)
            kyw_t = bcast_vec(lnpY, kyw_d, KVD)
            kyb_t = bcast_vec(lnpY, kyb_d, KVD)
            yk_ps = [psA.tile([P, NQ], F32, tag=f"acc{n}", name=f"ykps{n}",
                              bufs=2) for n in range(2)]
            yv_ps = [psA.tile([P, NQ], F32, tag=f"acc{n+2}", name=f"yvps{n}",
                              bufs=1) for n in range(2)]
            for dc in range(YDC):
                for n in range(2):
                    nc.tensor.matmul(
                        yk_ps[n][:], lhsT=yt_t[:, dc, :],
                        rhs=wkvy_sb[:, dc, n * NQ:(n + 1) * NQ],
                        start=(dc == 0), stop=(dc == YDC - 1))
                for n in range(2):
                    nc.tensor.matmul(
                        yv_ps[n][:], lhsT=yt_t[:, dc, :],
                        rhs=wkvy_sb[:, dc, KVD + n * NQ:KVD + (n + 1) * NQ],
                        start=(dc == 0), stop=(dc == YDC - 1))
            for n in range(2):
                nc.scalar.copy(
                    out=YV[:, 4 * n:4 * (n + 1), :], in_=yv_ps[n][:])
            ykn = work.tile([P, KVD], F32, tag="work")
            for n in range(2):
                nc.scalar.copy(out=ykn[:, n * NQ:(n + 1) * NQ],
                               in_=yk_ps[n][:])
            ln_apply_sb(ykn, 2, kyw_t, kyb_t, nc.gpsimd)
            for hg in range(2):
                tp = psA.tile([P, 4, P], F32, tag="tr", bufs=2)
                for j in range(4):
                    kv = hg * 4 + j
                    nc.tensor.transpose(
                        tp[:, j, :], ykn[:, kv * HD:(kv + 1) * HD], ident)
                nc.scalar.copy(
                    out=YKT[:, hg * 4:(hg + 1) * 4, :], in_=tp)
            lnpY.release()
            wC.release()
            wAv.release()

            # =========================================================
            # Stage Q: Q proj + LN + rope + transpose -> QT. First wq
            # half is already resident; second half streams in now.
            # =========================================================
            lnpQ = tc.alloc_tile_pool(name="lnpQ", bufs=1)
            qw_t = bcast_vec(lnpQ, qw_d, D)
            qb_t = bcast_vec(lnpQ, qb_d, D)
            wBb = tc.alloc_tile_pool(name="wBb", bufs=1)
            wqB = wBb.tile([P, 3 * DC // 4, D], BF, tag="w")
            for dc in range(3 * DC // 4):
                nc.sync.dma_start(out=wqB[:, dc, :],
                                  in_=wq_ap[:, DC // 4 + dc, :])
            qns = []
            for tcl in range(TCL):
                tok0 = tcl * P
                xt_t = xs.tile([P, DC, P], BF, tag="xt")
                nc.sync.dma_start(out=xt_t, in_=x_ap[:, :, tok0:tok0 + P])
                q_ps = [psA.tile([P, NQ], F32,
                                 tag=f"acc{n}", name=f"qps{n}",
                                 bufs=(2 if n < 2 else 1)) for n in range(4)]
                for dc in range(DC):
                    if dc < DC // 4:
                        wq_half, dci = wqA, dc
                    else:
                        wq_half, dci = wqB, dc - DC // 4
                    for n in range(4):
                        nc.tensor.matmul(
                            q_ps[n][:], lhsT=xt_t[:, dc, :],
                            rhs=wq_half[:, dci, n * NQ:(n + 1) * NQ],
                            start=(dc == 0), stop=(dc == DC - 1))
                qn = work.tile([P, D], F32, tag="work")
                for n in range(4):
                    nc.scalar.copy(out=qn[:, n * NQ:(n + 1) * NQ],
                                   in_=q_ps[n][:])
                ln_apply_sb(qn, 4, qw_t, qb_t, nc.vector)
                for hh in range(2):
                    _rope_inplace(nc, ropep, qn[:, hh * KVD:(hh + 1) * KVD],
                                  H // 2, cosq_t[:, tcl, :],
                                  sinq_t[:, tcl, :], nc.gpsimd)
                if tcl > 1:
                    transpose_to(qns[tcl - 2], H, QT, tok0 - 2 * P)
                qns.append(qn)
            for i in (TCL - 2, TCL - 1):
                transpose_to(qns[i], H, QT, i * P)
            # land K slots via the Pool queue (slot r = token half r) so
            # stage C's weight stream on the sync queue is not blocked
            for kv in range(KVH):
                nc.gpsimd.dma_start(out=KTr[:, kv, 0:S_LOC],
                                    in_=kout[0, :, kv, :])
                nc.gpsimd.dma_start(out=KTr[:, kv, S_LOC:S],
                                    in_=kout[1, :, kv, :])
            wBb.release()
            lnpQ.release()
            wBa.release()

            statp.release()
            ropep.release()
            work.release()
            xs.release()
            psA.release()

            # =========================================================
            # Stage D+E: pipelined attention + interleaved out-proj.
            # V lands via the otherwise-idle Pool queue (the landing DMAs
            # wait on the V gathers; nothing else queues behind them).
            # =========================================================
            vrp = tc.alloc_tile_pool(name="vrpool", bufs=1)
            Vr = vrp.tile([P, TC, KVD], BF)       # token-major V, global order
            for vo, c0 in ((vout1, 0), (vout2, KVD // 2)):
                nc.gpsimd.dma_start(out=Vr[:, 0:TCL, c0:c0 + KVD // 2],
                                    in_=vo[0, :, :, :])
                nc.gpsimd.dma_start(out=Vr[:, TCL:TC, c0:c0 + KVD // 2],
                                    in_=vo[1, :, :, :])
            ep = tc.alloc_tile_pool(name="escores", bufs=2)
            esp = tc.alloc_tile_pool(name="espairs", bufs=4)
            esq = tc.alloc_tile_pool(name="esquads", bufs=8)
            eyp = tc.alloc_tile_pool(name="eyscores", bufs=2)
            rcp = tc.alloc_tile_pool(name="recips", bufs=2)
            wop = tc.alloc_tile_pool(name="wostream", bufs=2)
            outp = tc.alloc_tile_pool(name="outtiles", bufs=3)
            psD = tc.alloc_tile_pool(name="psD", bufs=1, space="PSUM")

            wo_ap = wo_d[:, :].rearrange("(hc p) n -> p hc n", p=P)

            def load_wo8(j):
                t = wop.tile([P, DC, 2 * P], BF, tag="wo8", bufs=2,
                             name=f"wo8_{j}")
                nc.sync.dma_start(out=t, in_=wo_ap[:, :, j * 256:(j + 1) * 256])
                return t

            def attn_scores(h, qc):
                kv = h // NREP
                q0 = qc * NQ
                qt_t = QT[:, h, q0:q0 + NQ]
                # self-attention scores for all 16 key chunks
                E_t = ep.tile([P, TC, NQ], BF, tag="E", bufs=2)
                quads = []
                pairs = []
                for kp in range(TC // 2):
                    s_ps = psD.tile([P, 2, NQ], F32, tag="s", bufs=2)
                    for j in range(2):
                        kc = kp * 2 + j
                        nc.tensor.matmul(
                            s_ps[:, j, :],
                            lhsT=KTr[:, kv, kc * P:(kc + 1) * P],
                            rhs=qt_t, start=True, stop=True,
                            skip_group_check=True)
                    nc.scalar.activation(
                        out=E_t[:, kp * 2:kp * 2 + 2, :], in_=s_ps[:],
                        func=AF.Exp, scale=SCALE)
                    pr = esp.tile([P, NQ], BF, tag="esp", bufs=4,
                                  name="espair")
                    nc.vector.tensor_add(
                        out=pr, in0=E_t[:, kp * 2, :],
                        in1=E_t[:, kp * 2 + 1, :])
                    pairs.append(pr)
                    if kp % 2 == 1:
                        qd = esq.tile([P, NQ], BF, tag="esq", bufs=8,
                                      name="esquad")
                        nc.vector.tensor_add(
                            out=qd, in0=pairs[-2], in1=pairs[-1])
                        quads.append(qd)
                # cross-attention (needs y-stage outputs, so issued after
                # the self scores): sy -> Ey -> dy -> oy -> t1
                sy_ps = psD.tile([P, NQ], F32, tag="cross", bufs=1,
                                 name="sy_ps")
                nc.tensor.matmul(
                    sy_ps[:], lhsT=YKT[:, kv, :], rhs=qt_t,
                    start=True, stop=True, skip_group_check=True)
                Ey_t = eyp.tile([P, NQ], BF, tag="Ey", bufs=2)
                nc.scalar.activation(
                    out=Ey_t, in_=sy_ps[:], func=AF.Exp, scale=SCALE,
                    bias=ymb_t)
                dy_ps = psD.tile([P, NQ], F32, tag="cross", bufs=1,
                                 name="dy_ps")
                nc.tensor.matmul(
                    dy_ps[:], lhsT=ones_t, rhs=Ey_t,
                    start=True, stop=True, skip_group_check=True)
                rec_y = rcp.tile([P, NQ], F32, tag="recy", bufs=2)
                nc.vector.reciprocal(out=rec_y, in_=dy_ps[:])
                oy_ps = psD.tile([P, NQ], F32, tag="cross", bufs=1,
                                 name="oy_ps")
                nc.tensor.matmul(
                    oy_ps[:], lhsT=YV[:, kv, :], rhs=Ey_t,
                    start=True, stop=True, skip_group_check=True)
                t1 = rcp.tile([P, NQ], F32, tag="t1", bufs=3)
                nc.vector.scalar_tensor_tensor(
                    out=t1, in0=oy_ps[:], scalar=gates_t[:, h:h + 1],
                    in1=rec_y, op0=ALU.mult, op1=ALU.mult)
                return dict(h=h, kv=kv, q0=q0, E_t=E_t, quads=quads, t1=t1)

            def attn_out(st):
                h, kv, q0 = st["h"], st["kv"], st["q0"]
                E_t, quads, t1 = st["E_t"], st["quads"], st["t1"]
                d_ps = psD.tile([P, NQ], F32, tag="d", bufs=1, name="d_ps")
                for i, qd in enumerate(quads):
                    nc.tensor.matmul(
                        d_ps[:], lhsT=ones_t, rhs=qd,
                        start=(i == 0), stop=(i == len(quads) - 1),
                        skip_group_check=True)
                o_ps = psD.tile([P, NQ], F32, tag="o", bufs=1, name="o_ps")
                for kc in range(TC):
                    nc.tensor.matmul(
                        o_ps[:], lhsT=Vr[:, kc, kv * HD:(kv + 1) * HD],
                        rhs=E_t[:, kc, :],
                        start=(kc == 0), stop=(kc == TC - 1),
                        skip_group_check=True)
                rec = rcp.tile([P, NQ], F32, tag="rec", bufs=2)
                nc.vector.reciprocal(out=rec, in_=d_ps[:])
                t0 = rcp.tile([P, NQ], F32, tag="t0", bufs=2)
                nc.vector.tensor_mul(out=t0, in0=o_ps[:], in1=rec)
                # merged output aliases into the (now dead) Q^T slice
                nc.vector.tensor_add(
                    out=QT[:, h, q0:q0 + NQ], in0=t0, in1=t1)

            def outproj(qc, oc, wo8_t):
                q0 = qc * NQ
                out_ps = psD.tile([P, NQ], F32, tag="oout", bufs=1,
                                  name="out_ps")
                sub = (oc % 2) * P
                for hc in range(DC):
                    nc.tensor.matmul(
                        out_ps[:],
                        lhsT=wo8_t[:, hc, sub:sub + P],
                        rhs=QT[:, hc, q0:q0 + NQ],
                        start=(hc == 0), stop=(hc == DC - 1),
                        skip_group_check=True)
                out_t = outp.tile([P, NQ], F32, tag="outt")
                nc.vector.tensor_copy(out=out_t, in_=out_ps[:])
                nc.sync.dma_start(
                    out=outT[oc * P:(oc + 1) * P, q0:q0 + NQ],
                    in_=out_t)

            pending = []
            wo8_t = None
            for qc in range(QCN):
                for h in range(H):
                    st = attn_scores(h, qc)
                    pending.append(st)
                    if len(pending) > 1:
                        attn_out(pending.pop(0))
                    if qc == 1:
                        oc = h
                        if oc % 2 == 0:
                            wo8_t = load_wo8(oc // 2)
                        outproj(0, oc, wo8_t)
            for st in pending:
                attn_out(st)
            for oc in range(DC):
                if oc % 2 == 0:
                    wo8_t = load_wo8(oc // 2)
                outproj(1, oc, wo8_t)

            psD.release()
            outp.release()
            wop.release()
            rcp.release()
            eyp.release()
            esq.release()
            esp.release()
            ep.release()
            vrp.release()

    _split_dma_waits(nc)
    return nc


def _prep_shared(x, y, freqs_cos, freqs_sin, y_mask, wq, wk, wv, wk_y, wv_y,
                 wo, q_w, q_b, k_w, k_b, ky_w, ky_b, gate):
    f32 = np.float32
    shared = {
        "wq": np.ascontiguousarray(np.asarray(wq, f32).astype(BF16)),
        "wkv": np.ascontiguousarray(
            np.concatenate([np.asarray(wk, f32), np.asarray(wv, f32)],
                           axis=1).astype(BF16)),
        "wkvy": np.ascontiguousarray(
            np.concatenate([np.asarray(wk_y, f32), np.asarray(wv_y, f32)],
                           axis=1).astype(BF16)),
        "wo": np.ascontiguousarray(np.asarray(wo, f32).astype(BF16)),
        "qw": np.ascontiguousarray(np.asarray(q_w, f32)),
        "qb": np.ascontiguousarray(np.asarray(q_b, f32)),
        "kw": np.ascontiguousarray(np.asarray(k_w, f32)),
        "kb": np.ascontiguousarray(np.asarray(k_b, f32)),
        "kyw": np.ascontiguousarray(np.asarray(ky_w, f32)),
        "kyb": np.ascontiguousarray(np.asarray(ky_b, f32)),
        "gates": np.ascontiguousarray(np.tanh(np.asarray(gate, f32))),
    }
    per_core = []
    for c in range(8):
        b, hf = c // 2, c % 2
        sl = slice(hf * S_LOC, (hf + 1) * S_LOC)
        xTb = np.asarray(x[b], f32).T.astype(BF16)
        m = dict(shared)
        m["xTq"] = np.ascontiguousarray(xTb[:, sl])
        m["yT"] = np.ascontiguousarray(np.asarray(y[b], f32).T.astype(BF16))
        m["cosq"] = np.ascontiguousarray(np.asarray(freqs_cos, f32)[sl])
        m["sinq"] = np.ascontiguousarray(np.asarray(freqs_sin, f32)[sl])
        m["ymb"] = np.where(np.asarray(y_mask[b]), 0.0, -1e9).astype(f32)
        per_core.append(m)
    return per_core


def kernel(**inputs):
    if "nc" not in _CACHED:
        _CACHED["nc"] = build_program()
    nc = _CACHED["nc"]
    in_maps = _prep_shared(
        inputs["x"], inputs["y"], inputs["freqs_cos"], inputs["freqs_sin"],
        inputs["y_mask"], inputs["wq"], inputs["wk"], inputs["wv"],
        inputs["wk_y"], inputs["wv_y"], inputs["wo"], inputs["q_w"],
        inputs["q_b"], inputs["k_w"], inputs["k_b"], inputs["ky_w"],
        inputs["ky_b"], inputs["gate"])
    res = run_bass_kernel_spmd(nc, in_maps, core_ids=list(range(8)))
    global LAST_EXEC_NS
    LAST_EXEC_NS = res.exec_time_ns
    out = np.zeros((B, S, D), np.float32)
    for c in range(8):
        b, hf = c // 2, c % 2
        out[b, hf * S_LOC:(hf + 1) * S_LOC, :] = res.results[c]["outT"].T
    return out


if __name__ == "__main__":
    nc = build_program()
    print("program built OK")


# revision 4
# speedup vs baseline: 1.3141x; 1.0506x over previous
"""Trainium2 Bass kernel for nn_Attention_35734127903400 — v2.

Token-sharded (core c: batch c//2, seq half c%2) with pair-AllGather K/V
dedup: each core projects K/V only for its local 1024 tokens, exchanges
halves with its pair core via two HBM AllGathers (K first, then V) that
run on the collective cores, overlapped with Q/y projection. Key order
in KTr/Vr is global (gather slot r = token half r); on odd cores the
slot-0 DMA overwrites the locally-projected half with the peer's data
and the local half lands in the upper columns — same program on every
core, no divergence.

K^T/V/Q^T stay SBUF-resident (no DRAM spill). Attention is software-
pipelined: out-matmuls trail scores by one head (tolerating the late V
gather); softmax denominators fold 4-wide on DVE into 4 ones-matmuls;
output projection for q-chunk 0 interleaves with attention q-chunk 1,
with wo streamed in eighths; merged output aliases into dead Q^T
slices. Collectives issue from Pool (gpsimd), whose sequencer blocks on
the bounce-DMA waits — so LN bias-adds run on DVE/Pool split such that
nothing downstream queues behind a blocked Pool sequencer.
"""

import numpy as np
import ml_dtypes

import concourse.bass as bass
import concourse.mybir as mybir
import concourse.tile as tile
from concourse.bass_utils import run_bass_kernel_spmd
from concourse.masks import make_identity

BF16 = ml_dtypes.bfloat16
E4 = ml_dtypes.float8_e4m3fn
F32 = mybir.dt.float32
BF = mybir.dt.bfloat16
F8 = mybir.dt.float8e4
WSCALE = 64.0
DR = mybir.MatmulPerfMode.DoubleRow

P = 128
B, S, D = 4, 2048, 2048
H, KVH = 16, 8
HD = 128
NREP = 2
YL, YD = 128, 1024
EPS = 1e-5
S_LOC = S // 2
DC = D // P          # 16 contraction chunks for D
YDC = YD // P        # 8
TC = S // P          # 16 key chunks (full seq)
TCL = S_LOC // P     # 8 local token chunks
NQ = 512             # q-free chunk (one PSUM bank of f32)
QCN = S_LOC // NQ    # 2
KVD = KVH * HD       # 1024
SCALE = 1.0 / float(np.sqrt(np.float32(HD)))
AF = mybir.ActivationFunctionType
ALU = mybir.AluOpType
REP_GROUPS = [[0, 1], [2, 3], [4, 5], [6, 7]]

_CACHED = {}
LAST_EXEC_NS = None


def _ln_stats(nc, statp, ps_chunks):
    """bn_stats over a list of [P, 512] chunks -> mv [P, 2] (mean, var)."""
    nchunks = len(ps_chunks)
    stats = statp.tile([P, nchunks, 6], F32, tag="bnstats")
    for i, ps in enumerate(ps_chunks):
        nc.vector.bn_stats(out=stats[:, i, :], in_=ps[:])
    mv = statp.tile([P, 2], F32, tag="bnaggr")
    nc.vector.bn_aggr(out=mv, in_=stats)
    return mv


def _rope_inplace(nc, ropep, zn, nheads, cos_t, sin_t, mul_eng):
    """In-place rope on zn [P, nheads*HD] f32; cos/sin [P, 64] f32.
    The three products run on mul_eng (DVE or Pool), sub/add on DVE."""
    zv = zn.rearrange("p (h f two) -> p h f two", h=nheads, two=2)
    re = zv[:, :, :, 0]
    im = zv[:, :, :, 1]
    shp = (P, nheads, HD // 2)
    cb = cos_t[:, None, :].to_broadcast(shp)
    sb = sin_t[:, None, :].to_broadcast(shp)
    t1 = ropep.tile([P, nheads, HD // 2], F32, tag="rp1")
    t2 = ropep.tile([P, nheads, HD // 2], F32, tag="rp2")
    t3 = ropep.tile([P, nheads, HD // 2], F32, tag="rp3")
    mul_eng.tensor_mul(out=t1, in0=re, in1=cb)     # re*c
    mul_eng.tensor_mul(out=t2, in0=re, in1=sb)     # re*s
    nc.vector.tensor_mul(out=t3, in0=im, in1=sb)   # im*s
    nc.vector.tensor_sub(out=re, in0=t1, in1=t3)   # re' = re*c - im*s
    mul_eng.tensor_mul(out=t3, in0=im, in1=cb)     # im*c
    nc.vector.tensor_add(out=im, in0=t2, in1=t3)   # im' = re*s + im*c


def _split_dma_waits(nc, max_waits=1):
    """Hoist excess sync waits onto preceding same-engine single-wait NoOps
    (walrus per-instruction structs have 1-2 wait slots)."""
    n_split = 0
    for f in nc.m.functions:
        for blk in f.blocks:
            insts = list(blk.instructions)
            out = []
            changed = False
            for ins in insts:
                si = ins.sync_info
                if (si is not None and si.on_wait
                        and len(si.on_wait) > max_waits):
                    waits = list(si.on_wait)
                    for wi, w in enumerate(waits[:-max_waits]):
                        out.append(mybir.InstNoOp(
                            name=f"{ins.name}-wsplit{wi}", engine=ins.engine,
                            sync_info=mybir.SyncInfo(on_wait=[w],
                                                     on_update=[])))
                    ins.sync_info = mybir.SyncInfo(
                        on_wait=waits[-max_waits:],
                        on_update=list(si.on_update))
                    changed = True
                    n_split += 1
                out.append(ins)
            if changed:
                blk.instructions = out
    return n_split


def build_program():
    nc = bass.Bass()

    # ---- I/O (all per-core local; key order handled host-side) ----
    xh_d = nc.declare_dram_parameter("xh", [D, S_LOC], F8, isOutput=False)
    xl_d = nc.declare_dram_parameter("xl", [D, S_LOC], F8, isOutput=False)
    yT = nc.declare_dram_parameter("yT", [YD, YL], BF, isOutput=False)
    wqh_d = nc.declare_dram_parameter("wqh", [D, D], F8, isOutput=False)
    wql_d = nc.declare_dram_parameter("wql", [D, D], F8, isOutput=False)
    wkvh_d = nc.declare_dram_parameter("wkvh", [D, 2 * KVD], F8, isOutput=False)
    wkvl_d = nc.declare_dram_parameter("wkvl", [D, 2 * KVD], F8, isOutput=False)
    wkvy_d = nc.declare_dram_parameter("wkvy", [YD, 2 * KVD], BF, isOutput=False)
    wo_d = nc.declare_dram_parameter("wo", [D, D], BF, isOutput=False)
    qw_d = nc.declare_dram_parameter("qw", [D], F32, isOutput=False)
    qb_d = nc.declare_dram_parameter("qb", [D], F32, isOutput=False)
    kw_d = nc.declare_dram_parameter("kw", [KVD], F32, isOutput=False)
    kb_d = nc.declare_dram_parameter("kb", [KVD], F32, isOutput=False)
    kyw_d = nc.declare_dram_parameter("kyw", [KVD], F32, isOutput=False)
    kyb_d = nc.declare_dram_parameter("kyb", [KVD], F32, isOutput=False)
    cosq_d = nc.declare_dram_parameter("cosq", [S_LOC, HD // 2], F32, isOutput=False)
    sinq_d = nc.declare_dram_parameter("sinq", [S_LOC, HD // 2], F32, isOutput=False)
    gates_d = nc.declare_dram_parameter("gates", [H], F32, isOutput=False)
    ymb_d = nc.declare_dram_parameter("ymb", [YL], F32, isOutput=False)
    outT = nc.declare_dram_parameter("outT", [D, S_LOC], F32, isOutput=True)

    with tile.TileContext(nc) as tc:
        from contextlib import ExitStack
        with ExitStack() as ctx:
            # ---- persistent pools ----
            cpool = ctx.enter_context(tc.tile_pool(name="consts", bufs=1))
            yp = ctx.enter_context(tc.tile_pool(name="ypool", bufs=1))
            ktp = ctx.enter_context(tc.tile_pool(name="ktpool", bufs=1))
            qtp = ctx.enter_context(tc.tile_pool(name="qtpool", bufs=1))
            dramp = ctx.enter_context(
                tc.tile_pool(name="dscratch", bufs=1, space="DRAM"))

            KTr = ktp.tile([P, KVH, S], BF)       # key-major K^T, global order
            QT = qtp.tile([P, H, S_LOC], BF)      # Q^T; merged aliases in later
            YKT = yp.tile([P, KVH, YL], BF)
            YV = yp.tile([P, KVH, HD], BF)

            # one K gather; V gather split by feature half (kv-heads 0-3 /
            # 4-7) so each half lands just ahead of the heads needing it
            kin = dramp.tile([P, KVH, S_LOC], BF)
            kout = dramp.tile([2, P, KVH, S_LOC], BF)
            vin1 = dramp.tile([P, TCL, KVD // 2], BF)
            vin2 = dramp.tile([P, TCL, KVD // 2], BF)
            vout1 = dramp.tile([2, P, TCL, KVD // 2], BF)
            vout2 = dramp.tile([2, P, TCL, KVD // 2], BF)

            # projection-phase transient pools
            xs = tc.alloc_tile_pool(name="xstream", bufs=3)
            work = tc.alloc_tile_pool(name="work", bufs=3)
            ropep = tc.alloc_tile_pool(name="rope", bufs=1)
            statp = tc.alloc_tile_pool(name="stats", bufs=3)
            psA = tc.alloc_tile_pool(name="psA", bufs=1, space="PSUM")

            # ---- constants ----
            ident = cpool.tile([P, P], F32)
            make_identity(nc, ident)
            ones_t = cpool.tile([P, P], BF)
            nc.vector.memset(ones_t, 1.0)
            eps_t = cpool.tile([P, 1], F32)
            nc.vector.memset(eps_t, EPS)
            gates_t = cpool.tile([P, H], F32)
            nc.gpsimd.dma_start(
                out=gates_t,
                in_=bass.AP(tensor=gates_d, offset=0, ap=[[0, P], [1, H]]))
            ymb_t = cpool.tile([P, 1], F32)
            nc.gpsimd.dma_start(
                out=ymb_t,
                in_=bass.AP(tensor=ymb_d, offset=0, ap=[[1, P], [0, 1]]))

            cosq_t = ropep.tile([P, TCL, HD // 2], F32, tag="costab", bufs=1)
            sinq_t = ropep.tile([P, TCL, HD // 2], F32, tag="sintab", bufs=1)

            def bcast_vec(pool, dram_h, n):
                t = pool.tile([P, n], F32, tag=f"lnp_{dram_h.name}", bufs=1)
                nc.gpsimd.dma_start(
                    out=t, in_=bass.AP(tensor=dram_h, offset=0, ap=[[0, P], [1, n]]))
                return t

            def rstd_from_mv(mv):
                r = statp.tile([P, 1], F32, tag="rstd")
                nc.scalar.activation(out=r, in_=mv[:, 1:2], func=AF.Sqrt,
                                     bias=eps_t, scale=1.0)
                nc.vector.reciprocal(out=r, in_=r)
                return r

            def transpose_to(zn, nheads, sb_dst, tok0):
                """PE-transpose zn's heads into head-major bf16 dst."""
                for hg in range(nheads // 4):
                    tp = psA.tile([P, 4, P], F32, tag="tr", bufs=2)
                    for j in range(4):
                        hh = hg * 4 + j
                        nc.tensor.transpose(
                            tp[:, j, :], zn[:, hh * HD:(hh + 1) * HD], ident)
                    nc.scalar.copy(
                        out=sb_dst[:, hg * 4:(hg + 1) * 4, tok0:tok0 + P],
                        in_=tp)

            def ln_apply_mv(dst, nchunks, mv, w_t, b_t, badd_eng):
                rstd = rstd_from_mv(mv)
                negmr = statp.tile([P, 1], F32, tag="negmr")
                nc.gpsimd.tensor_scalar(
                    out=negmr, in0=mv[:, 0:1], scalar1=rstd, scalar2=-1.0,
                    op0=ALU.mult, op1=ALU.mult)
                n_tot = nchunks * NQ
                nc.scalar.activation(
                    out=dst[:, :n_tot], in_=dst[:, :n_tot], func=AF.Identity,
                    scale=rstd, bias=negmr)
                nc.vector.tensor_mul(out=dst[:, :n_tot], in0=dst[:, :n_tot],
                                     in1=w_t)
                badd_eng.tensor_add(out=dst[:, :n_tot], in0=dst[:, :n_tot],
                                    in1=b_t)

            def ln_apply_sb(dst, nchunks, w_t, b_t, badd_eng):
                """In-place LN on dst [P, nchunks*NQ] f32 (SBUF).
                stats DVE, normalize ACT, w-mul DVE, bias on badd_eng."""
                mv = _ln_stats(nc, statp,
                               [dst[:, n * NQ:(n + 1) * NQ]
                                for n in range(nchunks)])
                rstd = rstd_from_mv(mv)
                negmr = statp.tile([P, 1], F32, tag="negmr")
                nc.vector.tensor_scalar(
                    out=negmr, in0=mv[:, 0:1], scalar1=rstd, scalar2=-1.0,
                    op0=ALU.mult, op1=ALU.mult)
                n_tot = nchunks * NQ
                nc.scalar.activation(
                    out=dst[:, :n_tot], in_=dst[:, :n_tot], func=AF.Identity,
                    scale=rstd, bias=negmr)
                nc.vector.tensor_mul(out=dst[:, :n_tot], in0=dst[:, :n_tot],
                                     in1=w_t)
                badd_eng.tensor_add(out=dst[:, :n_tot], in0=dst[:, :n_tot],
                                    in1=b_t)

            xh_ap = xh_d[:, :].rearrange("(dc p) s -> p dc s", p=P)
            xl_ap = xl_d[:, :].rearrange("(dc p) s -> p dc s", p=P)

            def load_x(tok0, name=None):
                xh_t = xs.tile([P, DC, P], F8, tag="xh",
                               name=name and name + "h")
                xl_t = xs.tile([P, DC, P], F8, tag="xl",
                               name=name and name + "l")
                nc.sync.dma_start(out=xh_t, in_=xh_ap[:, :, tok0:tok0 + P])
                nc.sync.dma_start(out=xl_t, in_=xl_ap[:, :, tok0:tok0 + P])
                return xh_t, xl_t

            def dr_proj(ps_banks, xh_t, xl_t, wh, wl, nslices):
                """3-term fp8 DoubleRow accumulation over all DC chunks."""
                npair = DC // 2
                for dcp in range(npair):
                    sl = slice(2 * dcp, 2 * dcp + 2)
                    for bi, nsl in enumerate(nslices):
                        for ti, (lt, rt) in enumerate(
                                ((xh_t, wh), (xh_t, wl), (xl_t, wh))):
                            nc.tensor.matmul(
                                ps_banks[bi][:], lhsT=lt[:, sl, :],
                                rhs=rt[:, sl, nsl],
                                start=(dcp == 0 and ti == 0),
                                stop=(dcp == npair - 1 and ti == 2),
                                perf_mode=DR)

            # =========================================================
            # Stage K: local-half K proj + LN + rope + transpose, then
            # one pair-AllGather into KTr (global key order). wq's first
            # half prefetches during the K loop (DMA slack).
            # =========================================================
            wBa = tc.alloc_tile_pool(name="wBa", bufs=1)
            wqAh = wBa.tile([P, DC // 4, D], F8, tag="wh")
            wqAl = wBa.tile([P, DC // 4, D], F8, tag="wl")
            wqh_ap = wqh_d[:, :].rearrange("(dc p) n -> p dc n", p=P)
            wql_ap = wql_d[:, :].rearrange("(dc p) n -> p dc n", p=P)
            lnpK = tc.alloc_tile_pool(name="lnpK", bufs=1)
            kw_t = bcast_vec(lnpK, kw_d, KVD)
            kb_t = bcast_vec(lnpK, kb_d, KVD)
            wAk = tc.alloc_tile_pool(name="wAk", bufs=1)
            wkh_sb = wAk.tile([P, DC, KVD], F8, tag="wh")
            wkl_sb = wAk.tile([P, DC, KVD], F8, tag="wl")
            wkh_ap = wkvh_d[:, :KVD].rearrange("(dc p) n -> p dc n", p=P)
            wkl_ap = wkvl_d[:, :KVD].rearrange("(dc p) n -> p dc n", p=P)
            # queue order: 2 wk chunks, first x tile, rest of wk, wq quarter
            for dc in range(2):
                nc.sync.dma_start(out=wkh_sb[:, dc, :], in_=wkh_ap[:, dc, :])
                nc.sync.dma_start(out=wkl_sb[:, dc, :], in_=wkl_ap[:, dc, :])
            xt_first = load_x(0, name="xt_firstk")
            for dc in range(2, DC):
                nc.sync.dma_start(out=wkh_sb[:, dc, :], in_=wkh_ap[:, dc, :])
                nc.sync.dma_start(out=wkl_sb[:, dc, :], in_=wkl_ap[:, dc, :])
            nc.sync.dma_start(
                out=cosq_t,
                in_=cosq_d[:, :].rearrange("(t p) f -> p t f", p=P))
            nc.sync.dma_start(
                out=sinq_t,
                in_=sinq_d[:, :].rearrange("(t p) f -> p t f", p=P))
            for dc in range(DC // 4):
                nc.sync.dma_start(out=wqAh[:, dc, :], in_=wqh_ap[:, dc, :])
                nc.sync.dma_start(out=wqAl[:, dc, :], in_=wql_ap[:, dc, :])
            kns = []
            for tci in range(TCL):
                tok0 = tci * P
                if tci == 0:
                    xh_t, xl_t = xt_first
                else:
                    xh_t, xl_t = load_x(tok0)
                k_ps = [psA.tile([P, NQ], F32, tag=f"acc{n}", name=f"kps{n}",
                                 bufs=2) for n in range(2)]
                dr_proj(k_ps, xh_t, xl_t, wkh_sb, wkl_sb,
                        [slice(n * NQ, (n + 1) * NQ) for n in range(2)])
                kn = work.tile([P, KVD], F32, tag="work")
                for n in range(2):
                    nc.scalar.copy(out=kn[:, n * NQ:(n + 1) * NQ],
                                   in_=k_ps[n][:])
                ln_apply_sb(kn, 2, kw_t, kb_t, nc.vector)
                _rope_inplace(nc, ropep, kn, KVH, cosq_t[:, tci, :],
                              sinq_t[:, tci, :], nc.gpsimd)
                # transposes trail by two chunks so PE never waits on the
                # LN/rope chain (its latency exceeds one PE iteration)
                if tci > 1:
                    transpose_to(kns[tci - 2], KVH, KTr, tok0 - 2 * P)
                kns.append(kn)
            for i in (TCL - 2, TCL - 1):
                transpose_to(kns[i], KVH, KTr, i * P)
            nc.gpsimd.dma_start(out=kin[:, :, :], in_=KTr[:, :, 0:S_LOC])
            nc.gpsimd.collective_compute(
                "AllGather", ALU.bypass, replica_groups=REP_GROUPS,
                ins=[kin[:, :, :].opt()], outs=[kout[:, :, :, :].opt()])
            wAk.release()
            lnpK.release()

            # =========================================================
            # Stage V: local-half V proj, streamed to the bounce buffers
            # per token chunk; two V-half gathers follow the K gather.
            # =========================================================
            wAv = tc.alloc_tile_pool(name="wAv", bufs=1)
            wC = tc.alloc_tile_pool(name="wC", bufs=1)
            vst = tc.alloc_tile_pool(name="vstream", bufs=2)
            wvh_sb = wAv.tile([P, DC, KVD], F8, tag="wh")
            wvl_sb = wAv.tile([P, DC, KVD], F8, tag="wl")
            wvh_ap = wkvh_d[:, KVD:].rearrange("(dc p) n -> p dc n", p=P)
            wvl_ap = wkvl_d[:, KVD:].rearrange("(dc p) n -> p dc n", p=P)
            for dc in range(2):
                nc.sync.dma_start(out=wvh_sb[:, dc, :], in_=wvh_ap[:, dc, :])
                nc.sync.dma_start(out=wvl_sb[:, dc, :], in_=wvl_ap[:, dc, :])
            xt_firstv = load_x(0, name="xt_firstv")
            for dc in range(2, DC):
                nc.sync.dma_start(out=wvh_sb[:, dc, :], in_=wvh_ap[:, dc, :])
                nc.sync.dma_start(out=wvl_sb[:, dc, :], in_=wvl_ap[:, dc, :])
            # y-projection weights stream during the V loop; y-proj runs at
            # the end of this stage so attention can start right after Q
            wkvy_sb = wC.tile([P, YDC, 2 * KVD], BF, tag="w")
            wkvy_ap = wkvy_d[:, :].rearrange("(dc p) n -> p dc n", p=P)
            yt_t = xs.tile([P, YDC, YL], BF, tag="yt", bufs=1)
            nc.sync.dma_start(
                out=yt_t, in_=yT[:, :].rearrange("(dc p) s -> p dc s", p=P))
            for dc in range(YDC):
                nc.sync.dma_start(out=wkvy_sb[:, dc, :], in_=wkvy_ap[:, dc, :])
            for tci in range(TCL):
                tok0 = tci * P
                if tci == 0:
                    xh_t, xl_t = xt_firstv
                else:
                    xh_t, xl_t = load_x(tok0)
                v_ps = [psA.tile([P, NQ], F32, tag=f"acc{n}", name=f"vps{n}",
                                 bufs=2) for n in range(2)]
                dr_proj(v_ps, xh_t, xl_t, wvh_sb, wvl_sb,
                        [slice(n * NQ, (n + 1) * NQ) for n in range(2)])
                vt = vst.tile([P, KVD], BF, tag="vt", bufs=4)
                for n in range(2):
                    nc.scalar.activation(
                        out=vt[:, n * NQ:(n + 1) * NQ], in_=v_ps[n][:],
                        func=AF.Identity, scale=1.0 / WSCALE)
                nc.gpsimd.dma_start(out=vin1[:, tci, :],
                                     in_=vt[:, 0:KVD // 2])
                nc.gpsimd.dma_start(out=vin2[:, tci, :],
                                    in_=vt[:, KVD // 2:KVD])
            nc.gpsimd.collective_compute(
                "AllGather", ALU.bypass, replica_groups=REP_GROUPS,
                ins=[vin1[:, :, :].opt()], outs=[vout1[:, :, :, :].opt()])
            nc.gpsimd.collective_compute(
                "AllGather", ALU.bypass, replica_groups=REP_GROUPS,
                ins=[vin2[:, :, :].opt()], outs=[vout2[:, :, :, :].opt()])
            # ---- y projections -> YKT (LN, no rope), YV ----
            vst.release()
            lnpY = tc.alloc_tile_pool(name="lnpY", bufs=1)
            kyw_t = bcast_vec(lnpY, kyw_d, KVD)
            kyb_t = bcast_vec(lnpY, kyb_d, KVD)
            yk_ps = [psA.tile([P, NQ], F32, tag=f"acc{n}", name=f"ykps{n}",
                              bufs=2) for n in range(2)]
            yv_ps = [psA.tile([P, NQ], F32, tag=f"acc{n+2}", name=f"yvps{n}",
                              bufs=1) for n in range(2)]
            for dc in range(YDC):
                for n in range(2):
                    nc.tensor.matmul(
                        yk_ps[n][:], lhsT=yt_t[:, dc, :],
                        rhs=wkvy_sb[:, dc, n * NQ:(n + 1) * NQ],
                        start=(dc == 0), stop=(dc == YDC - 1))
                for n in range(2):
                    nc.tensor.matmul(
                        yv_ps[n][:], lhsT=yt_t[:, dc, :],
                        rhs=wkvy_sb[:, dc, KVD + n * NQ:KVD + (n + 1) * NQ],
                        start=(dc == 0), stop=(dc == YDC - 1))
            for n in range(2):
                nc.scalar.copy(
                    out=YV[:, 4 * n:4 * (n + 1), :], in_=yv_ps[n][:])
            ykn = work.tile([P, KVD], F32, tag="work")
            for n in range(2):
                nc.scalar.copy(out=ykn[:, n * NQ:(n + 1) * NQ],
                               in_=yk_ps[n][:])
            ln_apply_sb(ykn, 2, kyw_t, kyb_t, nc.gpsimd)
            for hg in range(2):
                tp = psA.tile([P, 4, P], F32, tag="tr", bufs=2)
                for j in range(4):
                    kv = hg * 4 + j
                    nc.tensor.transpose(
                        tp[:, j, :], ykn[:, kv * HD:(kv + 1) * HD], ident)
                nc.scalar.copy(
                    out=YKT[:, hg * 4:(hg + 1) * 4, :], in_=tp)
            lnpY.release()
            wC.release()
            wAv.release()

            # =========================================================
            # Stage Q: Q proj + LN + rope + transpose -> QT. First wq
            # half is already resident; second half streams in now.
            # =========================================================
            lnpQ = tc.alloc_tile_pool(name="lnpQ", bufs=1)
            qw_t = bcast_vec(lnpQ, qw_d, D)
            qb_t = bcast_vec(lnpQ, qb_d, D)
            wBb = tc.alloc_tile_pool(name="wBb", bufs=1)
            wqBh = wBb.tile([P, 3 * DC // 4, D], F8, tag="wh")
            wqBl = wBb.tile([P, 3 * DC // 4, D], F8, tag="wl")
            for dc in range(3 * DC // 4):
                nc.sync.dma_start(out=wqBh[:, dc, :],
                                  in_=wqh_ap[:, DC // 4 + dc, :])
                nc.sync.dma_start(out=wqBl[:, dc, :],
                                  in_=wql_ap[:, DC // 4 + dc, :])
            qns = []
            for tcl in range(TCL):
                tok0 = tcl * P
                xh_t, xl_t = load_x(tok0)
                q_ps = [psA.tile([P, NQ], F32,
                                 tag=f"acc{n}", name=f"qps{n}",
                                 bufs=(2 if n < 2 else 1)) for n in range(4)]
                npair = DC // 2
                for dcp in range(npair):
                    if dcp < DC // 8:
                        wh, wl = wqAh, wqAl
                        sl = slice(2 * dcp, 2 * dcp + 2)
                    else:
                        wh, wl = wqBh, wqBl
                        sl = slice(2 * dcp - DC // 4, 2 * dcp - DC // 4 + 2)
                    xsl = slice(2 * dcp, 2 * dcp + 2)
                    for n in range(4):
                        for ti, (lt, rt) in enumerate(
                                ((xh_t, wh), (xh_t, wl), (xl_t, wh))):
                            nc.tensor.matmul(
                                q_ps[n][:], lhsT=lt[:, xsl, :],
                                rhs=rt[:, sl, n * NQ:(n + 1) * NQ],
                                start=(dcp == 0 and ti == 0),
                                stop=(dcp == npair - 1 and ti == 2),
                                perf_mode=DR)
                qn = work.tile([P, D], F32, tag="work")
                for n in range(4):
                    nc.scalar.copy(out=qn[:, n * NQ:(n + 1) * NQ],
                                   in_=q_ps[n][:])
                ln_apply_sb(qn, 4, qw_t, qb_t, nc.vector)
                for hh in range(2):
                    _rope_inplace(nc, ropep, qn[:, hh * KVD:(hh + 1) * KVD],
                                  H // 2, cosq_t[:, tcl, :],
                                  sinq_t[:, tcl, :], nc.gpsimd)
                if tcl > 1:
                    transpose_to(qns[tcl - 2], H, QT, tok0 - 2 * P)
                qns.append(qn)
            for i in (TCL - 2, TCL - 1):
                transpose_to(qns[i], H, QT, i * P)
            # land K slots via the Pool queue (slot r = token half r) so
            # stage C's weight stream on the sync queue is not blocked
            for kv in range(KVH):
                nc.gpsimd.dma_start(out=KTr[:, kv, 0:S_LOC],
                                    in_=kout[0, :, kv, :])
                nc.gpsimd.dma_start(out=KTr[:, kv, S_LOC:S],
                                    in_=kout[1, :, kv, :])
            wBb.release()
            lnpQ.release()
            wBa.release()

            statp.release()
            ropep.release()
            work.release()
            xs.release()
            psA.release()

            # =========================================================
            # Stage D+E: pipelined attention + interleaved out-proj.
            # V lands via the otherwise-idle Pool queue (the landing DMAs
            # wait on the V gathers; nothing else queues behind them).
            # =========================================================
            vrp = tc.alloc_tile_pool(name="vrpool", bufs=1)
            Vr = vrp.tile([P, TC, KVD], BF)       # token-major V, global order
            for vo, c0 in ((vout1, 0), (vout2, KVD // 2)):
                nc.gpsimd.dma_start(out=Vr[:, 0:TCL, c0:c0 + KVD // 2],
                                    in_=vo[0, :, :, :])
                nc.gpsimd.dma_start(out=Vr[:, TCL:TC, c0:c0 + KVD // 2],
                                    in_=vo[1, :, :, :])
            ep = tc.alloc_tile_pool(name="escores", bufs=2)
            esp = tc.alloc_tile_pool(name="espairs", bufs=4)
            esq = tc.alloc_tile_pool(name="esquads", bufs=8)
            eyp = tc.alloc_tile_pool(name="eyscores", bufs=2)
            rcp = tc.alloc_tile_pool(name="recips", bufs=2)
            wop = tc.alloc_tile_pool(name="wostream", bufs=2)
            outp = tc.alloc_tile_pool(name="outtiles", bufs=3)
            psD = tc.alloc_tile_pool(name="psD", bufs=1, space="PSUM")

            wo_ap = wo_d[:, :].rearrange("(hc p) n -> p hc n", p=P)

            def load_wo8(j):
                t = wop.tile([P, DC, 2 * P], BF, tag="wo8", bufs=2,
                             name=f"wo8_{j}")
                nc.sync.dma_start(out=t, in_=wo_ap[:, :, j * 256:(j + 1) * 256])
                return t

            def attn_scores(h, qc):
                kv = h // NREP
                q0 = qc * NQ
                qt_t = QT[:, h, q0:q0 + NQ]
                # self-attention scores for all 16 key chunks
                E_t = ep.tile([P, TC, NQ], BF, tag="E", bufs=2)
                quads = []
                octs = []
                pairs = []
                for kp in range(TC // 2):
                    s_ps = psD.tile([P, 2, NQ], F32, tag="s", bufs=2)
                    for j in range(2):
                        kc = kp * 2 + j
                        nc.tensor.matmul(
                            s_ps[:, j, :],
                            lhsT=KTr[:, kv, kc * P:(kc + 1) * P],
                            rhs=qt_t, start=True, stop=True,
                            skip_group_check=True)
                    nc.scalar.activation(
                        out=E_t[:, kp * 2:kp * 2 + 2, :], in_=s_ps[:],
                        func=AF.Exp, scale=SCALE)
                    pr = esp.tile([P, NQ], BF, tag="esp", bufs=4,
                                  name="espair")
                    nc.vector.tensor_add(
                        out=pr, in0=E_t[:, kp * 2, :],
                        in1=E_t[:, kp * 2 + 1, :])
                    pairs.append(pr)
                    if kp % 2 == 1:
                        qd = esq.tile([P, NQ], BF, tag="esq", bufs=6,
                                      name="esquad")
                        nc.vector.tensor_add(
                            out=qd, in0=pairs[-2], in1=pairs[-1])
                        quads.append(qd)
                        if len(quads) % 2 == 0:
                            oc8 = esq.tile([P, NQ], BF, tag="eso", bufs=4,
                                           name="esoct")
                            nc.vector.tensor_add(
                                out=oc8, in0=quads[-2], in1=quads[-1])
                            octs.append(oc8)
                # cross-attention (needs y-stage outputs, so issued after
                # the self scores): sy -> Ey -> dy -> oy -> t1
                sy_ps = psD.tile([P, NQ], F32, tag="cross", bufs=1,
                                 name="sy_ps")
                nc.tensor.matmul(
                    sy_ps[:], lhsT=YKT[:, kv, :], rhs=qt_t,
                    start=True, stop=True, skip_group_check=True)
                Ey_t = eyp.tile([P, NQ], BF, tag="Ey", bufs=2)
                nc.scalar.activation(
                    out=Ey_t, in_=sy_ps[:], func=AF.Exp, scale=SCALE,
                    bias=ymb_t)
                dy_ps = psD.tile([P, NQ], F32, tag="cross", bufs=1,
                                 name="dy_ps")
                nc.tensor.matmul(
                    dy_ps[:], lhsT=ones_t, rhs=Ey_t,
                    start=True, stop=True, skip_group_check=True)
                rec_y = rcp.tile([P, NQ], F32, tag="recy", bufs=2)
                nc.vector.reciprocal(out=rec_y, in_=dy_ps[:])
                oy_ps = psD.tile([P, NQ], F32, tag="cross", bufs=1,
                                 name="oy_ps")
                nc.tensor.matmul(
                    oy_ps[:], lhsT=YV[:, kv, :], rhs=Ey_t,
                    start=True, stop=True, skip_group_check=True)
                t1 = rcp.tile([P, NQ], F32, tag="t1", bufs=3)
                nc.vector.scalar_tensor_tensor(
                    out=t1, in0=oy_ps[:], scalar=gates_t[:, h:h + 1],
                    in1=rec_y, op0=ALU.mult, op1=ALU.mult)
                return dict(h=h, kv=kv, q0=q0, E_t=E_t, quads=octs, t1=t1)

            def attn_out(st):
                h, kv, q0 = st["h"], st["kv"], st["q0"]
                E_t, quads, t1 = st["E_t"], st["quads"], st["t1"]
                d_ps = psD.tile([P, NQ], F32, tag="d", bufs=1, name="d_ps")
                for i, qd in enumerate(quads):
                    nc.tensor.matmul(
                        d_ps[:], lhsT=ones_t, rhs=qd,
                        start=(i == 0), stop=(i == len(quads) - 1),
                        skip_group_check=True)
                o_ps = psD.tile([P, NQ], F32, tag="o", bufs=1, name="o_ps")
                for kc in range(TC):
                    nc.tensor.matmul(
                        o_ps[:], lhsT=Vr[:, kc, kv * HD:(kv + 1) * HD],
                        rhs=E_t[:, kc, :],
                        start=(kc == 0), stop=(kc == TC - 1),
                        skip_group_check=True)
                rec = rcp.tile([P, NQ], F32, tag="rec", bufs=2)
                nc.vector.reciprocal(out=rec, in_=d_ps[:])
                t0 = rcp.tile([P, NQ], F32, tag="t0", bufs=2)
                nc.vector.tensor_mul(out=t0, in0=o_ps[:], in1=rec)
                # merged output aliases into the (now dead) Q^T slice
                nc.vector.tensor_add(
                    out=QT[:, h, q0:q0 + NQ], in0=t0, in1=t1)

            def outproj(qc, oc, wo8_t):
                q0 = qc * NQ
                out_ps = psD.tile([P, NQ], F32, tag="oout", bufs=1,
                                  name="out_ps")
                sub = (oc % 2) * P
                for hc in range(DC):
                    nc.tensor.matmul(
                        out_ps[:],
                        lhsT=wo8_t[:, hc, sub:sub + P],
                        rhs=QT[:, hc, q0:q0 + NQ],
                        start=(hc == 0), stop=(hc == DC - 1),
                        skip_group_check=True)
                out_t = outp.tile([P, NQ], F32, tag="outt")
                nc.vector.tensor_copy(out=out_t, in_=out_ps[:])
                nc.sync.dma_start(
                    out=outT[oc * P:(oc + 1) * P, q0:q0 + NQ],
                    in_=out_t)

            pending = []
            wo8_t = None
            for qc in range(QCN):
                for h in range(H):
                    st = attn_scores(h, qc)
                    pending.append(st)
                    if len(pending) > 1:
                        attn_out(pending.pop(0))
                    if qc == 1:
                        oc = h
                        if oc % 2 == 0:
                            wo8_t = load_wo8(oc // 2)
                        outproj(0, oc, wo8_t)
            for st in pending:
                attn_out(st)
            for oc in range(DC):
                if oc % 2 == 0:
                    wo8_t = load_wo8(oc // 2)
                outproj(1, oc, wo8_t)

            psD.release()
            outp.release()
            wop.release()
            rcp.release()
            eyp.release()
            esq.release()
            esp.release()
            ep.release()
            vrp.release()

    _split_dma_waits(nc)
    return nc


def _prep_shared(x, y, freqs_cos, freqs_sin, y_mask, wq, wk, wv, wk_y, wv_y,
                 wo, q_w, q_b, k_w, k_b, ky_w, ky_b, gate):
    f32 = np.float32

    def hilo(a, scale):
        s = np.asarray(a, f32) * scale
        hi = s.astype(E4)
        lo = (s - hi.astype(f32)).astype(E4)
        return np.ascontiguousarray(hi), np.ascontiguousarray(lo)

    wq64 = hilo(wq, WSCALE)
    wkv64 = hilo(np.concatenate([np.asarray(wk, f32), np.asarray(wv, f32)],
                                axis=1), WSCALE)
    shared = {
        "wqh": wq64[0], "wql": wq64[1],
        "wkvh": wkv64[0], "wkvl": wkv64[1],
        "wkvy": np.ascontiguousarray(
            np.concatenate([np.asarray(wk_y, f32), np.asarray(wv_y, f32)],
                           axis=1).astype(BF16)),
        "wo": np.ascontiguousarray(np.asarray(wo, f32).astype(BF16)),
        "qw": np.ascontiguousarray(np.asarray(q_w, f32)),
        "qb": np.ascontiguousarray(np.asarray(q_b, f32)),
        "kw": np.ascontiguousarray(np.asarray(k_w, f32)),
        "kb": np.ascontiguousarray(np.asarray(k_b, f32)),
        "kyw": np.ascontiguousarray(np.asarray(ky_w, f32)),
        "kyb": np.ascontiguousarray(np.asarray(ky_b, f32)),
        "gates": np.ascontiguousarray(np.tanh(np.asarray(gate, f32))),
    }
    per_core = []
    for c in range(8):
        b, hf = c // 2, c % 2
        sl = slice(hf * S_LOC, (hf + 1) * S_LOC)
        xTb = np.asarray(x[b], f32).T
        m = dict(shared)
        m["xh"], m["xl"] = hilo(xTb[:, sl], 1.0)
        m["yT"] = np.ascontiguousarray(np.asarray(y[b], f32).T.astype(BF16))
        m["cosq"] = np.ascontiguousarray(np.asarray(freqs_cos, f32)[sl])
        m["sinq"] = np.ascontiguousarray(np.asarray(freqs_sin, f32)[sl])
        m["ymb"] = np.where(np.asarray(y_mask[b]), 0.0, -1e9).astype(f32)
        per_core.append(m)
    return per_core


def kernel(**inputs):
    if "nc" not in _CACHED:
        _CACHED["nc"] = build_program()
    nc = _CACHED["nc"]
    in_maps = _prep_shared(
        inputs["x"], inputs["y"], inputs["freqs_cos"], inputs["freqs_sin"],
        inputs["y_mask"], inputs["wq"], inputs["wk"], inputs["wv"],
        inputs["wk_y"], inputs["wv_y"], inputs["wo"], inputs["q_w"],
        inputs["q_b"], inputs["k_w"], inputs["k_b"], inputs["ky_w"],
        inputs["ky_b"], inputs["gate"])
    res = run_bass_kernel_spmd(nc, in_maps, core_ids=list(range(8)))
    global LAST_EXEC_NS
    LAST_EXEC_NS = res.exec_time_ns
    out = np.zeros((B, S, D), np.float32)
    for c in range(8):
        b, hf = c // 2, c % 2
        out[b, hf * S_LOC:(hf + 1) * S_LOC, :] = res.results[c]["outT"].T
    return out


if __name__ == "__main__":
    nc = build_program()
    print("program built OK")


# revision 5
# speedup vs baseline: 1.3162x; 1.0016x over previous
"""Trainium2 Bass kernel for nn_Attention_35734127903400 — v2.

Token-sharded (core c: batch c//2, seq half c%2) with pair-AllGather K/V
dedup: each core projects K/V only for its local 1024 tokens, exchanges
halves with its pair core via two HBM AllGathers (K first, then V) that
run on the collective cores, overlapped with Q/y projection. Key order
in KTr/Vr is global (gather slot r = token half r); on odd cores the
slot-0 DMA overwrites the locally-projected half with the peer's data
and the local half lands in the upper columns — same program on every
core, no divergence.

K^T/V/Q^T stay SBUF-resident (no DRAM spill). Attention is software-
pipelined: out-matmuls trail scores by one head (tolerating the late V
gather); softmax denominators fold 4-wide on DVE into 4 ones-matmuls;
output projection for q-chunk 0 interleaves with attention q-chunk 1,
with wo streamed in eighths; merged output aliases into dead Q^T
slices. Collectives issue from Pool (gpsimd), whose sequencer blocks on
the bounce-DMA waits — so LN bias-adds run on DVE/Pool split such that
nothing downstream queues behind a blocked Pool sequencer.
"""

import numpy as np
import ml_dtypes

import concourse.bass as bass
import concourse.mybir as mybir
import concourse.tile as tile
from concourse.bass_utils import run_bass_kernel_spmd
from concourse.masks import make_identity

BF16 = ml_dtypes.bfloat16
E4 = ml_dtypes.float8_e4m3fn
F32 = mybir.dt.float32
BF = mybir.dt.bfloat16
F8 = mybir.dt.float8e4
WSCALE = 64.0
DR = mybir.MatmulPerfMode.DoubleRow

P = 128
B, S, D = 4, 2048, 2048
H, KVH = 16, 8
HD = 128
NREP = 2
YL, YD = 128, 1024
EPS = 1e-5
S_LOC = S // 2
DC = D // P          # 16 contraction chunks for D
YDC = YD // P        # 8
TC = S // P          # 16 key chunks (full seq)
TCL = S_LOC // P     # 8 local token chunks
NQ = 512             # q-free chunk (one PSUM bank of f32)
QCN = S_LOC // NQ    # 2
KVD = KVH * HD       # 1024
SCALE = 1.0 / float(np.sqrt(np.float32(HD)))
AF = mybir.ActivationFunctionType
ALU = mybir.AluOpType
REP_GROUPS = [[0, 1], [2, 3], [4, 5], [6, 7]]

_CACHED = {}
LAST_EXEC_NS = None


def _ln_stats(nc, statp, ps_chunks):
    """bn_stats over a list of [P, 512] chunks -> mv [P, 2] (mean, var)."""
    nchunks = len(ps_chunks)
    stats = statp.tile([P, nchunks, 6], F32, tag="bnstats")
    for i, ps in enumerate(ps_chunks):
        nc.vector.bn_stats(out=stats[:, i, :], in_=ps[:])
    mv = statp.tile([P, 2], F32, tag="bnaggr")
    nc.vector.bn_aggr(out=mv, in_=stats)
    return mv


def _rope_inplace(nc, ropep, zn, nheads, cos_t, sin_t, mul_eng):
    """In-place rope on zn [P, nheads*HD] f32; cos/sin [P, 64] f32.
    The three products run on mul_eng (DVE or Pool), sub/add on DVE."""
    zv = zn.rearrange("p (h f two) -> p h f two", h=nheads, two=2)
    re = zv[:, :, :, 0]
    im = zv[:, :, :, 1]
    shp = (P, nheads, HD // 2)
    cb = cos_t[:, None, :].to_broadcast(shp)
    sb = sin_t[:, None, :].to_broadcast(shp)
    t1 = ropep.tile([P, nheads, HD // 2], F32, tag="rp1")
    t2 = ropep.tile([P, nheads, HD // 2], F32, tag="rp2")
    t3 = ropep.tile([P, nheads, HD // 2], F32, tag="rp3")
    mul_eng.tensor_mul(out=t1, in0=re, in1=cb)     # re*c
    mul_eng.tensor_mul(out=t2, in0=re, in1=sb)     # re*s
    nc.vector.tensor_mul(out=t3, in0=im, in1=sb)   # im*s
    nc.vector.tensor_sub(out=re, in0=t1, in1=t3)   # re' = re*c - im*s
    mul_eng.tensor_mul(out=t3, in0=im, in1=cb)     # im*c
    nc.vector.tensor_add(out=im, in0=t2, in1=t3)   # im' = re*s + im*c


def _split_dma_waits(nc, max_waits=1):
    """Hoist excess sync waits onto preceding same-engine single-wait NoOps
    (walrus per-instruction structs have 1-2 wait slots)."""
    n_split = 0
    for f in nc.m.functions:
        for blk in f.blocks:
            insts = list(blk.instructions)
            out = []
            changed = False
            for ins in insts:
                si = ins.sync_info
                if (si is not None and si.on_wait
                        and len(si.on_wait) > max_waits):
                    waits = list(si.on_wait)
                    for wi, w in enumerate(waits[:-max_waits]):
                        out.append(mybir.InstNoOp(
                            name=f"{ins.name}-wsplit{wi}", engine=ins.engine,
                            sync_info=mybir.SyncInfo(on_wait=[w],
                                                     on_update=[])))
                    ins.sync_info = mybir.SyncInfo(
                        on_wait=waits[-max_waits:],
                        on_update=list(si.on_update))
                    changed = True
                    n_split += 1
                out.append(ins)
            if changed:
                blk.instructions = out
    return n_split


def build_program():
    nc = bass.Bass()

    # ---- I/O (all per-core local; key order handled host-side) ----
    xh_d = nc.declare_dram_parameter("xh", [D, S_LOC], F8, isOutput=False)
    xl_d = nc.declare_dram_parameter("xl", [D, S_LOC], F8, isOutput=False)
    yT = nc.declare_dram_parameter("yT", [YD, YL], BF, isOutput=False)
    wqh_d = nc.declare_dram_parameter("wqh", [D, D], F8, isOutput=False)
    wql_d = nc.declare_dram_parameter("wql", [D, D], F8, isOutput=False)
    wkvh_d = nc.declare_dram_parameter("wkvh", [D, 2 * KVD], F8, isOutput=False)
    wkvl_d = nc.declare_dram_parameter("wkvl", [D, 2 * KVD], F8, isOutput=False)
    wkvy_d = nc.declare_dram_parameter("wkvy", [YD, 2 * KVD], BF, isOutput=False)
    wo_d = nc.declare_dram_parameter("wo", [D, D], BF, isOutput=False)
    qw_d = nc.declare_dram_parameter("qw", [D], F32, isOutput=False)
    qb_d = nc.declare_dram_parameter("qb", [D], F32, isOutput=False)
    kw_d = nc.declare_dram_parameter("kw", [KVD], F32, isOutput=False)
    kb_d = nc.declare_dram_parameter("kb", [KVD], F32, isOutput=False)
    kyw_d = nc.declare_dram_parameter("kyw", [KVD], BF, isOutput=False)
    kyb_d = nc.declare_dram_parameter("kyb", [KVD], BF, isOutput=False)
    cosq_d = nc.declare_dram_parameter("cosq", [S_LOC, HD // 2], F32, isOutput=False)
    sinq_d = nc.declare_dram_parameter("sinq", [S_LOC, HD // 2], F32, isOutput=False)
    gates_d = nc.declare_dram_parameter("gates", [H], F32, isOutput=False)
    ymb_d = nc.declare_dram_parameter("ymb", [YL], F32, isOutput=False)
    outT = nc.declare_dram_parameter("outT", [D, S_LOC], F32, isOutput=True)

    with tile.TileContext(nc) as tc:
        from contextlib import ExitStack
        with ExitStack() as ctx:
            # ---- persistent pools ----
            cpool = ctx.enter_context(tc.tile_pool(name="consts", bufs=1))
            yp = ctx.enter_context(tc.tile_pool(name="ypool", bufs=1))
            ktp = ctx.enter_context(tc.tile_pool(name="ktpool", bufs=1))
            qtp = ctx.enter_context(tc.tile_pool(name="qtpool", bufs=1))
            dramp = ctx.enter_context(
                tc.tile_pool(name="dscratch", bufs=1, space="DRAM"))

            KTr = ktp.tile([P, KVH, S], BF)       # key-major K^T, global order
            QT = qtp.tile([P, H, S_LOC], BF)      # Q^T; merged aliases in later
            YKT = yp.tile([P, KVH, YL], BF)
            YV = yp.tile([P, KVH, HD], BF)

            # one K gather; V gather split by feature half (kv-heads 0-3 /
            # 4-7) so each half lands just ahead of the heads needing it
            kin = dramp.tile([P, KVH, S_LOC], BF)
            kout = dramp.tile([2, P, KVH, S_LOC], BF)
            vin1 = dramp.tile([P, TCL, KVD // 2], BF)
            vin2 = dramp.tile([P, TCL, KVD // 2], BF)
            vout1 = dramp.tile([2, P, TCL, KVD // 2], BF)
            vout2 = dramp.tile([2, P, TCL, KVD // 2], BF)

            # projection-phase transient pools
            xs = tc.alloc_tile_pool(name="xstream", bufs=3)
            work = tc.alloc_tile_pool(name="work", bufs=3)
            ropep = tc.alloc_tile_pool(name="rope", bufs=1)
            statp = tc.alloc_tile_pool(name="stats", bufs=3)
            psA = tc.alloc_tile_pool(name="psA", bufs=1, space="PSUM")

            # ---- constants ----
            ident = cpool.tile([P, P], F32)
            make_identity(nc, ident)
            ones_t = cpool.tile([P, P], BF)
            nc.vector.memset(ones_t, 1.0)
            eps_t = cpool.tile([P, 1], F32)
            nc.vector.memset(eps_t, EPS)
            gates_t = cpool.tile([P, H], F32)
            nc.gpsimd.dma_start(
                out=gates_t,
                in_=bass.AP(tensor=gates_d, offset=0, ap=[[0, P], [1, H]]))
            ymb_t = cpool.tile([P, 1], F32)
            nc.gpsimd.dma_start(
                out=ymb_t,
                in_=bass.AP(tensor=ymb_d, offset=0, ap=[[1, P], [0, 1]]))

            cosq_t = ropep.tile([P, TCL, HD // 2], F32, tag="costab", bufs=1)
            sinq_t = ropep.tile([P, TCL, HD // 2], F32, tag="sintab", bufs=1)

            def bcast_vec(pool, dram_h, n, dt=F32):
                t = pool.tile([P, n], dt, tag=f"lnp_{dram_h.name}", bufs=1)
                nc.gpsimd.dma_start(
                    out=t, in_=bass.AP(tensor=dram_h, offset=0, ap=[[0, P], [1, n]]))
                return t

            def rstd_from_mv(mv):
                r = statp.tile([P, 1], F32, tag="rstd")
                nc.scalar.activation(out=r, in_=mv[:, 1:2], func=AF.Sqrt,
                                     bias=eps_t, scale=1.0)
                nc.vector.reciprocal(out=r, in_=r)
                return r

            def transpose_to(zn, nheads, sb_dst, tok0):
                """PE-transpose zn's heads into head-major bf16 dst."""
                for hg in range(nheads // 4):
                    tp = psA.tile([P, 4, P], F32, tag="tr", bufs=2)
                    for j in range(4):
                        hh = hg * 4 + j
                        nc.tensor.transpose(
                            tp[:, j, :], zn[:, hh * HD:(hh + 1) * HD], ident)
                    nc.scalar.copy(
                        out=sb_dst[:, hg * 4:(hg + 1) * 4, tok0:tok0 + P],
                        in_=tp)

            def ln_apply_mv(dst, nchunks, mv, w_t, b_t, badd_eng,
                            wmul_eng=None):
                wmul_eng = wmul_eng or nc.vector
                rstd = rstd_from_mv(mv)
                negmr = statp.tile([P, 1], F32, tag="negmr")
                nc.vector.tensor_scalar(
                    out=negmr, in0=mv[:, 0:1], scalar1=rstd, scalar2=-1.0,
                    op0=ALU.mult, op1=ALU.mult)
                n_tot = nchunks * NQ
                nc.scalar.activation(
                    out=dst[:, :n_tot], in_=dst[:, :n_tot], func=AF.Identity,
                    scale=rstd, bias=negmr)
                wmul_eng.tensor_mul(out=dst[:, :n_tot], in0=dst[:, :n_tot],
                                    in1=w_t)
                badd_eng.tensor_add(out=dst[:, :n_tot], in0=dst[:, :n_tot],
                                    in1=b_t)

            def ln_apply_sb(dst, nchunks, w_t, b_t, badd_eng,
                            wmul_eng=None):
                wmul_eng = wmul_eng or nc.vector
                """In-place LN on dst [P, nchunks*NQ] f32 (SBUF).
                stats DVE, normalize ACT, w-mul DVE, bias on badd_eng."""
                mv = _ln_stats(nc, statp,
                               [dst[:, n * NQ:(n + 1) * NQ]
                                for n in range(nchunks)])
                rstd = rstd_from_mv(mv)
                negmr = statp.tile([P, 1], F32, tag="negmr")
                nc.vector.tensor_scalar(
                    out=negmr, in0=mv[:, 0:1], scalar1=rstd, scalar2=-1.0,
                    op0=ALU.mult, op1=ALU.mult)
                n_tot = nchunks * NQ
                nc.scalar.activation(
                    out=dst[:, :n_tot], in_=dst[:, :n_tot], func=AF.Identity,
                    scale=rstd, bias=negmr)
                nc.vector.tensor_mul(out=dst[:, :n_tot], in0=dst[:, :n_tot],
                                     in1=w_t)
                badd_eng.tensor_add(out=dst[:, :n_tot], in0=dst[:, :n_tot],
                                    in1=b_t)

            xh_ap = xh_d[:, :].rearrange("(dc p) s -> p dc s", p=P)
            xl_ap = xl_d[:, :].rearrange("(dc p) s -> p dc s", p=P)

            def load_x(tok0, name=None):
                xh_t = xs.tile([P, DC, P], F8, tag="xh",
                               name=name and name + "h")
                xl_t = xs.tile([P, DC, P], F8, tag="xl",
                               name=name and name + "l")
                nc.sync.dma_start(out=xh_t, in_=xh_ap[:, :, tok0:tok0 + P])
                nc.sync.dma_start(out=xl_t, in_=xl_ap[:, :, tok0:tok0 + P])
                return xh_t, xl_t

            def dr_proj(ps_banks, xh_t, xl_t, wh, wl, nslices):
                """3-term fp8 DoubleRow accumulation over all DC chunks."""
                npair = DC // 2
                for dcp in range(npair):
                    sl = slice(2 * dcp, 2 * dcp + 2)
                    for bi, nsl in enumerate(nslices):
                        for ti, (lt, rt) in enumerate(
                                ((xh_t, wh), (xh_t, wl), (xl_t, wh))):
                            nc.tensor.matmul(
                                ps_banks[bi][:], lhsT=lt[:, sl, :],
                                rhs=rt[:, sl, nsl],
                                start=(dcp == 0 and ti == 0),
                                stop=(dcp == npair - 1 and ti == 2),
                                perf_mode=DR)

            # =========================================================
            # Stage K: local-half K proj + LN + rope + transpose, then
            # one pair-AllGather into KTr (global key order). wq's first
            # half prefetches during the K loop (DMA slack).
            # =========================================================
            wBa = tc.alloc_tile_pool(name="wBa", bufs=1)
            wqAh = wBa.tile([P, DC // 4, D], F8, tag="wh")
            wqAl = wBa.tile([P, DC // 4, D], F8, tag="wl")
            wqh_ap = wqh_d[:, :].rearrange("(dc p) n -> p dc n", p=P)
            wql_ap = wql_d[:, :].rearrange("(dc p) n -> p dc n", p=P)
            lnpK = tc.alloc_tile_pool(name="lnpK", bufs=1)
            kw_t = bcast_vec(lnpK, kw_d, KVD)
            kb_t = bcast_vec(lnpK, kb_d, KVD)
            wAk = tc.alloc_tile_pool(name="wAk", bufs=1)
            wkh_sb = wAk.tile([P, DC, KVD], F8, tag="wh")
            wkl_sb = wAk.tile([P, DC, KVD], F8, tag="wl")
            wkh_ap = wkvh_d[:, :KVD].rearrange("(dc p) n -> p dc n", p=P)
            wkl_ap = wkvl_d[:, :KVD].rearrange("(dc p) n -> p dc n", p=P)
            # queue order: 2 wk chunks, first x tile, rest of wk, wq quarter
            for dc in range(2):
                nc.sync.dma_start(out=wkh_sb[:, dc, :], in_=wkh_ap[:, dc, :])
                nc.sync.dma_start(out=wkl_sb[:, dc, :], in_=wkl_ap[:, dc, :])
            xt_first = load_x(0, name="xt_firstk")
            for dc in range(2, DC):
                nc.sync.dma_start(out=wkh_sb[:, dc, :], in_=wkh_ap[:, dc, :])
                nc.sync.dma_start(out=wkl_sb[:, dc, :], in_=wkl_ap[:, dc, :])
            nc.sync.dma_start(
                out=cosq_t,
                in_=cosq_d[:, :].rearrange("(t p) f -> p t f", p=P))
            nc.sync.dma_start(
                out=sinq_t,
                in_=sinq_d[:, :].rearrange("(t p) f -> p t f", p=P))
            for dc in range(DC // 4):
                nc.sync.dma_start(out=wqAh[:, dc, :], in_=wqh_ap[:, dc, :])
                nc.sync.dma_start(out=wqAl[:, dc, :], in_=wql_ap[:, dc, :])
            kns = []
            for tci in range(TCL):
                tok0 = tci * P
                if tci == 0:
                    xh_t, xl_t = xt_first
                else:
                    xh_t, xl_t = load_x(tok0)
                k_ps = [psA.tile([P, NQ], F32, tag=f"acc{n}", name=f"kps{n}",
                                 bufs=2) for n in range(2)]
                dr_proj(k_ps, xh_t, xl_t, wkh_sb, wkl_sb,
                        [slice(n * NQ, (n + 1) * NQ) for n in range(2)])
                kn = work.tile([P, KVD], F32, tag="work")
                for n in range(2):
                    nc.scalar.copy(out=kn[:, n * NQ:(n + 1) * NQ],
                                   in_=k_ps[n][:])
                ln_apply_sb(kn, 2, kw_t, kb_t, nc.vector)
                _rope_inplace(nc, ropep, kn, KVH, cosq_t[:, tci, :],
                              sinq_t[:, tci, :], nc.gpsimd)
                # transposes trail by two chunks so PE never waits on the
                # LN/rope chain (its latency exceeds one PE iteration)
                if tci > 1:
                    transpose_to(kns[tci - 2], KVH, KTr, tok0 - 2 * P)
                kns.append(kn)
            for i in (TCL - 2, TCL - 1):
                transpose_to(kns[i], KVH, KTr, i * P)
            nc.gpsimd.dma_start(out=kin[:, :, :], in_=KTr[:, :, 0:S_LOC])
            nc.gpsimd.collective_compute(
                "AllGather", ALU.bypass, replica_groups=REP_GROUPS,
                ins=[kin[:, :, :].opt()], outs=[kout[:, :, :, :].opt()])
            wAk.release()
            lnpK.release()

            # =========================================================
            # Stage V: local-half V proj, streamed to the bounce buffers
            # per token chunk; two V-half gathers follow the K gather.
            # =========================================================
            wAv = tc.alloc_tile_pool(name="wAv", bufs=1)
            wC = tc.alloc_tile_pool(name="wC", bufs=1)
            vst = tc.alloc_tile_pool(name="vstream", bufs=2)
            wvh_sb = wAv.tile([P, DC, KVD], F8, tag="wh")
            wvl_sb = wAv.tile([P, DC, KVD], F8, tag="wl")
            wvh_ap = wkvh_d[:, KVD:].rearrange("(dc p) n -> p dc n", p=P)
            wvl_ap = wkvl_d[:, KVD:].rearrange("(dc p) n -> p dc n", p=P)
            for dc in range(2):
                nc.sync.dma_start(out=wvh_sb[:, dc, :], in_=wvh_ap[:, dc, :])
                nc.sync.dma_start(out=wvl_sb[:, dc, :], in_=wvl_ap[:, dc, :])
            xt_firstv = load_x(0, name="xt_firstv")
            for dc in range(2, DC):
                nc.sync.dma_start(out=wvh_sb[:, dc, :], in_=wvh_ap[:, dc, :])
                nc.sync.dma_start(out=wvl_sb[:, dc, :], in_=wvl_ap[:, dc, :])
            # y-projection weights stream during the V loop; y-proj runs at
            # the end of this stage so attention can start right after Q
            wkvy_sb = wC.tile([P, YDC, 2 * KVD], BF, tag="w")
            wkvy_ap = wkvy_d[:, :].rearrange("(dc p) n -> p dc n", p=P)
            yt_t = xs.tile([P, YDC, YL], BF, tag="yt", bufs=1)
            nc.sync.dma_start(
                out=yt_t, in_=yT[:, :].rearrange("(dc p) s -> p dc s", p=P))
            for dc in range(YDC):
                nc.sync.dma_start(out=wkvy_sb[:, dc, :], in_=wkvy_ap[:, dc, :])
            for tci in range(TCL):
                tok0 = tci * P
                if tci == 0:
                    xh_t, xl_t = xt_firstv
                else:
                    xh_t, xl_t = load_x(tok0)
                v_ps = [psA.tile([P, NQ], F32, tag=f"acc{n}", name=f"vps{n}",
                                 bufs=2) for n in range(2)]
                dr_proj(v_ps, xh_t, xl_t, wvh_sb, wvl_sb,
                        [slice(n * NQ, (n + 1) * NQ) for n in range(2)])
                vt = vst.tile([P, KVD], BF, tag="vt", bufs=4)
                for n in range(2):
                    nc.scalar.activation(
                        out=vt[:, n * NQ:(n + 1) * NQ], in_=v_ps[n][:],
                        func=AF.Identity, scale=1.0 / WSCALE)
                nc.gpsimd.dma_start(out=vin1[:, tci, :],
                                     in_=vt[:, 0:KVD // 2])
                nc.gpsimd.dma_start(out=vin2[:, tci, :],
                                    in_=vt[:, KVD // 2:KVD])
            nc.gpsimd.collective_compute(
                "AllGather", ALU.bypass, replica_groups=REP_GROUPS,
                ins=[vin1[:, :, :].opt()], outs=[vout1[:, :, :, :].opt()])
            nc.gpsimd.collective_compute(
                "AllGather", ALU.bypass, replica_groups=REP_GROUPS,
                ins=[vin2[:, :, :].opt()], outs=[vout2[:, :, :, :].opt()])
            # ---- y projections -> YKT (LN, no rope), YV ----
            vst.release()
            lnpY = tc.alloc_tile_pool(name="lnpY", bufs=1)
            kyw_t = bcast_vec(lnpY, kyw_d, KVD, dt=BF)
            kyb_t = bcast_vec(lnpY, kyb_d, KVD, dt=BF)
            yk_ps = [psA.tile([P, NQ], F32, tag=f"acc{n}", name=f"ykps{n}",
                              bufs=2) for n in range(2)]
            yv_ps = [psA.tile([P, NQ], F32, tag=f"acc{n+2}", name=f"yvps{n}",
                              bufs=1) for n in range(2)]
            for dc in range(YDC):
                for n in range(2):
                    nc.tensor.matmul(
                        yk_ps[n][:], lhsT=yt_t[:, dc, :],
                        rhs=wkvy_sb[:, dc, n * NQ:(n + 1) * NQ],
                        start=(dc == 0), stop=(dc == YDC - 1))
                for n in range(2):
                    nc.tensor.matmul(
                        yv_ps[n][:], lhsT=yt_t[:, dc, :],
                        rhs=wkvy_sb[:, dc, KVD + n * NQ:KVD + (n + 1) * NQ],
                        start=(dc == 0), stop=(dc == YDC - 1))
            for n in range(2):
                nc.scalar.copy(
                    out=YV[:, 4 * n:4 * (n + 1), :], in_=yv_ps[n][:])
            ykn = work.tile([P, KVD], F32, tag="work")
            for n in range(2):
                nc.scalar.copy(out=ykn[:, n * NQ:(n + 1) * NQ],
                               in_=yk_ps[n][:])
            ln_apply_sb(ykn, 2, kyw_t, kyb_t, nc.vector)
            for hg in range(2):
                tp = psA.tile([P, 4, P], F32, tag="tr", bufs=2)
                for j in range(4):
                    kv = hg * 4 + j
                    nc.tensor.transpose(
                        tp[:, j, :], ykn[:, kv * HD:(kv + 1) * HD], ident)
                nc.scalar.copy(
                    out=YKT[:, hg * 4:(hg + 1) * 4, :], in_=tp)
            lnpY.release()
            wC.release()
            wAv.release()

            # =========================================================
            # Stage Q: Q proj + LN + rope + transpose -> QT. First wq
            # half is already resident; second half streams in now.
            # =========================================================
            lnpQ = tc.alloc_tile_pool(name="lnpQ", bufs=1)
            qw_t = bcast_vec(lnpQ, qw_d, D)
            qb_t = bcast_vec(lnpQ, qb_d, D)
            wBb = tc.alloc_tile_pool(name="wBb", bufs=1)
            wqBh = wBb.tile([P, 3 * DC // 4, D], F8, tag="wh")
            wqBl = wBb.tile([P, 3 * DC // 4, D], F8, tag="wl")
            for dc in range(3 * DC // 4):
                nc.sync.dma_start(out=wqBh[:, dc, :],
                                  in_=wqh_ap[:, DC // 4 + dc, :])
                nc.sync.dma_start(out=wqBl[:, dc, :],
                                  in_=wql_ap[:, DC // 4 + dc, :])
            qns = []
            for tcl in range(TCL):
                tok0 = tcl * P
                xh_t, xl_t = load_x(tok0)
                q_ps = [psA.tile([P, NQ], F32,
                                 tag=f"acc{n}", name=f"qps{n}",
                                 bufs=(2 if n < 2 else 1)) for n in range(4)]
                npair = DC // 2
                for dcp in range(npair):
                    if dcp < DC // 8:
                        wh, wl = wqAh, wqAl
                        sl = slice(2 * dcp, 2 * dcp + 2)
                    else:
                        wh, wl = wqBh, wqBl
                        sl = slice(2 * dcp - DC // 4, 2 * dcp - DC // 4 + 2)
                    xsl = slice(2 * dcp, 2 * dcp + 2)
                    for n in range(4):
                        for ti, (lt, rt) in enumerate(
                                ((xh_t, wh), (xh_t, wl), (xl_t, wh))):
                            nc.tensor.matmul(
                                q_ps[n][:], lhsT=lt[:, xsl, :],
                                rhs=rt[:, sl, n * NQ:(n + 1) * NQ],
                                start=(dcp == 0 and ti == 0),
                                stop=(dcp == npair - 1 and ti == 2),
                                perf_mode=DR)
                qn = work.tile([P, D], F32, tag="work")
                for n in range(4):
                    nc.scalar.copy(out=qn[:, n * NQ:(n + 1) * NQ],
                                   in_=q_ps[n][:])
                ln_apply_sb(qn, 4, qw_t, qb_t, nc.vector, nc.gpsimd)
                for hh in range(2):
                    _rope_inplace(nc, ropep, qn[:, hh * KVD:(hh + 1) * KVD],
                                  H // 2, cosq_t[:, tcl, :],
                                  sinq_t[:, tcl, :], nc.gpsimd)
                if tcl > 1:
                    transpose_to(qns[tcl - 2], H, QT, tok0 - 2 * P)
                qns.append(qn)
            for i in (TCL - 2, TCL - 1):
                transpose_to(qns[i], H, QT, i * P)
            # land K slots via the Pool queue (slot r = token half r),
            # deprioritized so the scheduler orders them after the Q-stage
            # Pool work (they wait on the K gather; anything queued behind
            # them would stall with them)
            for kv in range(KVH):
                nc.gpsimd.dma_start(out=KTr[:, kv, 0:S_LOC],
                                    in_=kout[0, :, kv, :])
                nc.gpsimd.dma_start(out=KTr[:, kv, S_LOC:S],
                                    in_=kout[1, :, kv, :])
            wBb.release()
            lnpQ.release()
            wBa.release()

            statp.release()
            ropep.release()
            work.release()
            xs.release()
            psA.release()

            # =========================================================
            # Stage D+E: pipelined attention + interleaved out-proj.
            # V lands via the otherwise-idle Pool queue (the landing DMAs
            # wait on the V gathers; nothing else queues behind them).
            # =========================================================
            vrp = tc.alloc_tile_pool(name="vrpool", bufs=1)
            Vr = vrp.tile([P, TC, KVD], BF)       # token-major V, global order
            for vo, c0 in ((vout1, 0), (vout2, KVD // 2)):
                nc.gpsimd.dma_start(out=Vr[:, 0:TCL, c0:c0 + KVD // 2],
                                    in_=vo[0, :, :, :])
                nc.gpsimd.dma_start(out=Vr[:, TCL:TC, c0:c0 + KVD // 2],
                                    in_=vo[1, :, :, :])
            ep = tc.alloc_tile_pool(name="escores", bufs=2)
            esp = tc.alloc_tile_pool(name="espairs", bufs=4)
            esq = tc.alloc_tile_pool(name="esquads", bufs=8)
            eyp = tc.alloc_tile_pool(name="eyscores", bufs=2)
            rcp = tc.alloc_tile_pool(name="recips", bufs=2)
            wop = tc.alloc_tile_pool(name="wostream", bufs=2)
            outp = tc.alloc_tile_pool(name="outtiles", bufs=3)
            psD = tc.alloc_tile_pool(name="psD", bufs=1, space="PSUM")

            wo_ap = wo_d[:, :].rearrange("(hc p) n -> p hc n", p=P)

            def load_wo8(j):
                t = wop.tile([P, DC, 2 * P], BF, tag="wo8", bufs=2,
                             name=f"wo8_{j}")
                nc.sync.dma_start(out=t, in_=wo_ap[:, :, j * 256:(j + 1) * 256])
                return t

            def attn_scores(h, qc):
                kv = h // NREP
                q0 = qc * NQ
                qt_t = QT[:, h, q0:q0 + NQ]
                # self-attention scores for all 16 key chunks
                E_t = ep.tile([P, TC, NQ], BF, tag="E", bufs=2)
                quads = []
                octs = []
                pairs = []
                for kp in range(TC // 2):
                    s_ps = psD.tile([P, 2, NQ], F32, tag="s", bufs=2)
                    for j in range(2):
                        kc = kp * 2 + j
                        nc.tensor.matmul(
                            s_ps[:, j, :],
                            lhsT=KTr[:, kv, kc * P:(kc + 1) * P],
                            rhs=qt_t, start=True, stop=True,
                            skip_group_check=True)
                    nc.scalar.activation(
                        out=E_t[:, kp * 2:kp * 2 + 2, :], in_=s_ps[:],
                        func=AF.Exp, scale=SCALE)
                    pr = esp.tile([P, NQ], BF, tag="esp", bufs=4,
                                  name="espair")
                    nc.vector.tensor_add(
                        out=pr, in0=E_t[:, kp * 2, :],
                        in1=E_t[:, kp * 2 + 1, :])
                    pairs.append(pr)
                    if kp % 2 == 1:
                        qd = esq.tile([P, NQ], BF, tag="esq", bufs=6,
                                      name="esquad")
                        nc.vector.tensor_add(
                            out=qd, in0=pairs[-2], in1=pairs[-1])
                        quads.append(qd)
                        if len(quads) % 2 == 0:
                            oc8 = esq.tile([P, NQ], BF, tag="eso", bufs=4,
                                           name="esoct")
                            nc.vector.tensor_add(
                                out=oc8, in0=quads[-2], in1=quads[-1])
                            octs.append(oc8)
                # cross-attention (needs y-stage outputs, so issued after
                # the self scores): sy -> Ey -> dy -> oy -> t1
                sy_ps = psD.tile([P, NQ], F32, tag="cross", bufs=1,
                                 name="sy_ps")
                nc.tensor.matmul(
                    sy_ps[:], lhsT=YKT[:, kv, :], rhs=qt_t,
                    start=True, stop=True, skip_group_check=True)
                Ey_t = eyp.tile([P, NQ], BF, tag="Ey", bufs=2)
                nc.scalar.activation(
                    out=Ey_t, in_=sy_ps[:], func=AF.Exp, scale=SCALE,
                    bias=ymb_t)
                dy_ps = psD.tile([P, NQ], F32, tag="cross", bufs=1,
                                 name="dy_ps")
                nc.tensor.matmul(
                    dy_ps[:], lhsT=ones_t, rhs=Ey_t,
                    start=True, stop=True, skip_group_check=True)
                rec_y = rcp.tile([P, NQ], F32, tag="recy", bufs=2)
                nc.vector.reciprocal(out=rec_y, in_=dy_ps[:])
                oy_ps = psD.tile([P, NQ], F32, tag="cross", bufs=1,
                                 name="oy_ps")
                nc.tensor.matmul(
                    oy_ps[:], lhsT=YV[:, kv, :], rhs=Ey_t,
                    start=True, stop=True, skip_group_check=True)
                t1 = rcp.tile([P, NQ], F32, tag="t1", bufs=3)
                nc.vector.scalar_tensor_tensor(
                    out=t1, in0=oy_ps[:], scalar=gates_t[:, h:h + 1],
                    in1=rec_y, op0=ALU.mult, op1=ALU.mult)
                return dict(h=h, kv=kv, q0=q0, E_t=E_t, quads=octs, t1=t1)

            def attn_out(st):
                h, kv, q0 = st["h"], st["kv"], st["q0"]
                E_t, quads, t1 = st["E_t"], st["quads"], st["t1"]
                d_ps = psD.tile([P, NQ], F32, tag="d", bufs=1, name="d_ps")
                for i, qd in enumerate(quads):
                    nc.tensor.matmul(
                        d_ps[:], lhsT=ones_t, rhs=qd,
                        start=(i == 0), stop=(i == len(quads) - 1),
                        skip_group_check=True)
                o_ps = psD.tile([P, NQ], F32, tag="o", bufs=1, name="o_ps")
                for kc in range(TC):
                    nc.tensor.matmul(
                        o_ps[:], lhsT=Vr[:, kc, kv * HD:(kv + 1) * HD],
                        rhs=E_t[:, kc, :],
                        start=(kc == 0), stop=(kc == TC - 1),
                        skip_group_check=True)
                rec = rcp.tile([P, NQ], F32, tag="rec", bufs=2)
                nc.vector.reciprocal(out=rec, in_=d_ps[:])
                t0 = rcp.tile([P, NQ], F32, tag="t0", bufs=2)
                nc.vector.tensor_mul(out=t0, in0=o_ps[:], in1=rec)
                # merged output aliases into the (now dead) Q^T slice
                nc.vector.tensor_add(
                    out=QT[:, h, q0:q0 + NQ], in0=t0, in1=t1)

            def outproj(qc, oc, wo8_t):
                q0 = qc * NQ
                out_ps = psD.tile([P, NQ], F32, tag="oout", bufs=1,
                                  name="out_ps")
                sub = (oc % 2) * P
                for hc in range(DC):
                    nc.tensor.matmul(
                        out_ps[:],
                        lhsT=wo8_t[:, hc, sub:sub + P],
                        rhs=QT[:, hc, q0:q0 + NQ],
                        start=(hc == 0), stop=(hc == DC - 1),
                        skip_group_check=True)
                out_t = outp.tile([P, NQ], F32, tag="outt")
                nc.vector.tensor_copy(out=out_t, in_=out_ps[:])
                nc.sync.dma_start(
                    out=outT[oc * P:(oc + 1) * P, q0:q0 + NQ],
                    in_=out_t)

            pending = []
            wo8_t = None
            for qc in range(QCN):
                for h in range(H):
                    st = attn_scores(h, qc)
                    pending.append(st)
                    if len(pending) > 1:
                        attn_out(pending.pop(0))
                    if qc == 1:
                        oc = h
                        if oc % 2 == 0:
                            wo8_t = load_wo8(oc // 2)
                        outproj(0, oc, wo8_t)
            for st in pending:
                attn_out(st)
            for oc in range(DC):
                if oc % 2 == 0:
                    wo8_t = load_wo8(oc // 2)
                outproj(1, oc, wo8_t)

            psD.release()
            outp.release()
            wop.release()
            rcp.release()
            eyp.release()
            esq.release()
            esp.release()
            ep.release()
            vrp.release()

    _split_dma_waits(nc)
    return nc


def _prep_shared(x, y, freqs_cos, freqs_sin, y_mask, wq, wk, wv, wk_y, wv_y,
                 wo, q_w, q_b, k_w, k_b, ky_w, ky_b, gate):
    f32 = np.float32

    def hilo(a, scale):
        s = np.asarray(a, f32) * scale
        hi = s.astype(E4)
        lo = (s - hi.astype(f32)).astype(E4)
        return np.ascontiguousarray(hi), np.ascontiguousarray(lo)

    wq64 = hilo(wq, WSCALE)
    wkv64 = hilo(np.concatenate([np.asarray(wk, f32), np.asarray(wv, f32)],
                                axis=1), WSCALE)
    shared = {
        "wqh": wq64[0], "wql": wq64[1],
        "wkvh": wkv64[0], "wkvl": wkv64[1],
        "wkvy": np.ascontiguousarray(
            np.concatenate([np.asarray(wk_y, f32), np.asarray(wv_y, f32)],
                           axis=1).astype(BF16)),
        "wo": np.ascontiguousarray(np.asarray(wo, f32).astype(BF16)),
        "qw": np.ascontiguousarray(np.asarray(q_w, f32)),
        "qb": np.ascontiguousarray(np.asarray(q_b, f32)),
        "kw": np.ascontiguousarray(np.asarray(k_w, f32)),
        "kb": np.ascontiguousarray(np.asarray(k_b, f32)),
        "kyw": np.ascontiguousarray(np.asarray(ky_w, f32).astype(BF16)),
        "kyb": np.ascontiguousarray(np.asarray(ky_b, f32).astype(BF16)),
        "gates": np.ascontiguousarray(np.tanh(np.asarray(gate, f32))),
    }
    per_core = []
    for c in range(8):
        b, hf = c // 2, c % 2
        sl = slice(hf * S_LOC, (hf + 1) * S_LOC)
        xTb = np.asarray(x[b], f32).T
        m = dict(shared)
        m["xh"], m["xl"] = hilo(xTb[:, sl], 1.0)
        m["yT"] = np.ascontiguousarray(np.asarray(y[b], f32).T.astype(BF16))
        m["cosq"] = np.ascontiguousarray(np.asarray(freqs_cos, f32)[sl])
        m["sinq"] = np.ascontiguousarray(np.asarray(freqs_sin, f32)[sl])
        m["ymb"] = np.where(np.asarray(y_mask[b]), 0.0, -1e9).astype(f32)
        per_core.append(m)
    return per_core


def kernel(**inputs):
    if "nc" not in _CACHED:
        _CACHED["nc"] = build_program()
    nc = _CACHED["nc"]
    in_maps = _prep_shared(
        inputs["x"], inputs["y"], inputs["freqs_cos"], inputs["freqs_sin"],
        inputs["y_mask"], inputs["wq"], inputs["wk"], inputs["wv"],
        inputs["wk_y"], inputs["wv_y"], inputs["wo"], inputs["q_w"],
        inputs["q_b"], inputs["k_w"], inputs["k_b"], inputs["ky_w"],
        inputs["ky_b"], inputs["gate"])
    res = run_bass_kernel_spmd(nc, in_maps, core_ids=list(range(8)))
    global LAST_EXEC_NS
    LAST_EXEC_NS = res.exec_time_ns
    out = np.zeros((B, S, D), np.float32)
    for c in range(8):
        b, hf = c // 2, c % 2
        out[b, hf * S_LOC:(hf + 1) * S_LOC, :] = res.results[c]["outT"].T
    return out


if __name__ == "__main__":
    nc = build_program()
    print("program built OK")


# revision 7
# speedup vs baseline: 1.3560x; 1.0302x over previous
"""Trainium2 Bass kernel for nn_Attention_35734127903400 — v2.

Token-sharded (core c: batch c//2, seq half c%2) with pair-AllGather K/V
dedup: each core projects K/V only for its local 1024 tokens, exchanges
halves with its pair core via two HBM AllGathers (K first, then V) that
run on the collective cores, overlapped with Q/y projection. Key order
in KTr/Vr is global (gather slot r = token half r); on odd cores the
slot-0 DMA overwrites the locally-projected half with the peer's data
and the local half lands in the upper columns — same program on every
core, no divergence.

K^T/V/Q^T stay SBUF-resident (no DRAM spill). Attention is software-
pipelined: out-matmuls trail scores by one head (tolerating the late V
gather); softmax denominators fold 4-wide on DVE into 4 ones-matmuls;
output projection for q-chunk 0 interleaves with attention q-chunk 1,
with wo streamed in eighths; merged output aliases into dead Q^T
slices. Collectives issue from Pool (gpsimd), whose sequencer blocks on
the bounce-DMA waits — so LN bias-adds run on DVE/Pool split such that
nothing downstream queues behind a blocked Pool sequencer.
"""

import numpy as np
import ml_dtypes

import concourse.bass as bass
import concourse.mybir as mybir
import concourse.tile as tile
from concourse.bass_utils import run_bass_kernel_spmd
from concourse.masks import make_identity

BF16 = ml_dtypes.bfloat16
E4 = ml_dtypes.float8_e4m3fn
F32 = mybir.dt.float32
BF = mybir.dt.bfloat16
F8 = mybir.dt.float8e4
WSCALE = 64.0
DR = mybir.MatmulPerfMode.DoubleRow

P = 128
B, S, D = 4, 2048, 2048
H, KVH = 16, 8
HD = 128
NREP = 2
YL, YD = 128, 1024
EPS = 1e-5
S_LOC = S // 2
DC = D // P          # 16 contraction chunks for D
YDC = YD // P        # 8
TC = S // P          # 16 key chunks (full seq)
TCL = S_LOC // P     # 8 local token chunks
NQ = 512             # q-free chunk (one PSUM bank of f32)
QCN = S_LOC // NQ    # 2
KVD = KVH * HD       # 1024
SCALE = 1.0 / float(np.sqrt(np.float32(HD)))
AF = mybir.ActivationFunctionType
ALU = mybir.AluOpType
REP_GROUPS = [[0, 1], [2, 3], [4, 5], [6, 7]]

_CACHED = {}
LAST_EXEC_NS = None


def _ln_stats(nc, statp, ps_chunks):
    """bn_stats over a list of [P, 512] chunks -> mv [P, 2] (mean, var)."""
    nchunks = len(ps_chunks)
    stats = statp.tile([P, nchunks, 6], F32, tag="bnstats")
    for i, ps in enumerate(ps_chunks):
        nc.vector.bn_stats(out=stats[:, i, :], in_=ps[:])
    mv = statp.tile([P, 2], F32, tag="bnaggr")
    nc.vector.bn_aggr(out=mv, in_=stats)
    return mv


def _rope_inplace(nc, ropep, zn, nheads, cos_t, sin_t, mul_eng):
    """In-place rope on zn [P, nheads*HD] f32; cos/sin [P, 64] f32.
    The three products run on mul_eng (DVE or Pool), sub/add on DVE."""
    zv = zn.rearrange("p (h f two) -> p h f two", h=nheads, two=2)
    re = zv[:, :, :, 0]
    im = zv[:, :, :, 1]
    shp = (P, nheads, HD // 2)
    cb = cos_t[:, None, :].to_broadcast(shp)
    sb = sin_t[:, None, :].to_broadcast(shp)
    t1 = ropep.tile([P, nheads, HD // 2], F32, tag="rp1")
    t2 = ropep.tile([P, nheads, HD // 2], F32, tag="rp2")
    t3 = ropep.tile([P, nheads, HD // 2], F32, tag="rp3")
    mul_eng.tensor_mul(out=t1, in0=re, in1=cb)     # re*c
    mul_eng.tensor_mul(out=t2, in0=re, in1=sb)     # re*s
    nc.vector.tensor_mul(out=t3, in0=im, in1=sb)   # im*s
    nc.vector.tensor_sub(out=re, in0=t1, in1=t3)   # re' = re*c - im*s
    mul_eng.tensor_mul(out=t3, in0=im, in1=cb)     # im*c
    nc.vector.tensor_add(out=im, in0=t2, in1=t3)   # im' = re*s + im*c


def _split_dma_waits(nc, max_waits=1):
    """Hoist excess sync waits onto preceding same-engine single-wait NoOps
    (walrus per-instruction structs have 1-2 wait slots)."""
    n_split = 0
    for f in nc.m.functions:
        for blk in f.blocks:
            insts = list(blk.instructions)
            out = []
            changed = False
            for ins in insts:
                si = ins.sync_info
                if (si is not None and si.on_wait
                        and len(si.on_wait) > max_waits):
                    waits = list(si.on_wait)
                    for wi, w in enumerate(waits[:-max_waits]):
                        out.append(mybir.InstNoOp(
                            name=f"{ins.name}-wsplit{wi}", engine=ins.engine,
                            sync_info=mybir.SyncInfo(on_wait=[w],
                                                     on_update=[])))
                    ins.sync_info = mybir.SyncInfo(
                        on_wait=waits[-max_waits:],
                        on_update=list(si.on_update))
                    changed = True
                    n_split += 1
                out.append(ins)
            if changed:
                blk.instructions = out
    return n_split


def build_program():
    nc = bass.Bass()

    # ---- I/O (all per-core local; key order handled host-side) ----
    xh_d = nc.declare_dram_parameter("xh", [D, S_LOC], F8, isOutput=False)
    xl_d = nc.declare_dram_parameter("xl", [D, S_LOC], F8, isOutput=False)
    yT = nc.declare_dram_parameter("yT", [YD, YL], BF, isOutput=False)
    wqh_d = nc.declare_dram_parameter("wqh", [D, D], F8, isOutput=False)
    wql_d = nc.declare_dram_parameter("wql", [D, D], F8, isOutput=False)
    wkvh_d = nc.declare_dram_parameter("wkvh", [D, 2 * KVD], F8, isOutput=False)
    wkvl_d = nc.declare_dram_parameter("wkvl", [D, 2 * KVD], F8, isOutput=False)
    wkvy_d = nc.declare_dram_parameter("wkvy", [YD, 2 * KVD], BF, isOutput=False)
    wo_d = nc.declare_dram_parameter("wo", [D, D], BF, isOutput=False)
    qw_d = nc.declare_dram_parameter("qw", [D], F32, isOutput=False)
    qb_d = nc.declare_dram_parameter("qb", [D], F32, isOutput=False)
    kw_d = nc.declare_dram_parameter("kw", [KVD], F32, isOutput=False)
    kb_d = nc.declare_dram_parameter("kb", [KVD], F32, isOutput=False)
    kyw_d = nc.declare_dram_parameter("kyw", [KVD], BF, isOutput=False)
    kyb_d = nc.declare_dram_parameter("kyb", [KVD], BF, isOutput=False)
    cosq_d = nc.declare_dram_parameter("cosq", [S_LOC, HD // 2], F32, isOutput=False)
    sinq_d = nc.declare_dram_parameter("sinq", [S_LOC, HD // 2], F32, isOutput=False)
    gates_d = nc.declare_dram_parameter("gates", [H], F32, isOutput=False)
    ymb_d = nc.declare_dram_parameter("ymb", [YL], F32, isOutput=False)
    outT = nc.declare_dram_parameter("outT", [D, S_LOC], F32, isOutput=True)

    with tile.TileContext(nc) as tc:
        from contextlib import ExitStack
        with ExitStack() as ctx:
            # ---- persistent pools ----
            cpool = ctx.enter_context(tc.tile_pool(name="consts", bufs=1))
            yp = ctx.enter_context(tc.tile_pool(name="ypool", bufs=1))
            ktp = ctx.enter_context(tc.tile_pool(name="ktpool", bufs=1))
            qtp = ctx.enter_context(tc.tile_pool(name="qtpool", bufs=1))
            dramp = ctx.enter_context(
                tc.tile_pool(name="dscratch", bufs=1, space="DRAM"))

            KTr = ktp.tile([P, KVH, S], BF)       # key-major K^T, global order
            QT = qtp.tile([P, H, S_LOC], BF)      # Q^T; merged aliases in later
            YKT = yp.tile([P, KVH, YL], BF)
            YV = yp.tile([P, KVH, HD], BF)

            # one K gather; V gather split by feature half (kv-heads 0-3 /
            # 4-7) so each half lands just ahead of the heads needing it
            kin_a = dramp.tile([P, KVH, S_LOC // 2], BF)
            kin_b = dramp.tile([P, KVH, S_LOC // 2], BF)
            kout_a = dramp.tile([2, P, KVH, S_LOC // 2], BF)
            kout_b = dramp.tile([2, P, KVH, S_LOC // 2], BF)
            vin1 = dramp.tile([P, TCL, KVD // 2], BF)
            vin2 = dramp.tile([P, TCL, KVD // 2], BF)
            vout1 = dramp.tile([2, P, TCL, KVD // 2], BF)
            vout2 = dramp.tile([2, P, TCL, KVD // 2], BF)

            # projection-phase transient pools
            xs = tc.alloc_tile_pool(name="xstream", bufs=3)
            work = tc.alloc_tile_pool(name="work", bufs=3)
            ropep = tc.alloc_tile_pool(name="rope", bufs=1)
            statp = tc.alloc_tile_pool(name="stats", bufs=3)
            psA = tc.alloc_tile_pool(name="psA", bufs=1, space="PSUM")

            # ---- constants ----
            ident = cpool.tile([P, P], F32)
            make_identity(nc, ident)
            ones_t = cpool.tile([P, P], BF)
            nc.vector.memset(ones_t, 1.0)
            eps_t = cpool.tile([P, 1], F32)
            nc.vector.memset(eps_t, EPS)
            gates_t = cpool.tile([P, H], F32)
            nc.gpsimd.dma_start(
                out=gates_t,
                in_=bass.AP(tensor=gates_d, offset=0, ap=[[0, P], [1, H]]))
            ymb_t = cpool.tile([P, 1], F32)
            nc.gpsimd.dma_start(
                out=ymb_t,
                in_=bass.AP(tensor=ymb_d, offset=0, ap=[[1, P], [0, 1]]))

            cosq_t = ropep.tile([P, TCL, HD // 2], F32, tag="costab", bufs=1)
            sinq_t = ropep.tile([P, TCL, HD // 2], F32, tag="sintab", bufs=1)

            def bcast_vec(pool, dram_h, n, dt=F32):
                t = pool.tile([P, n], dt, tag=f"lnp_{dram_h.name}", bufs=1)
                nc.gpsimd.dma_start(
                    out=t, in_=bass.AP(tensor=dram_h, offset=0, ap=[[0, P], [1, n]]))
                return t

            def rstd_from_mv(mv):
                r = statp.tile([P, 1], F32, tag="rstd")
                nc.scalar.activation(out=r, in_=mv[:, 1:2], func=AF.Sqrt,
                                     bias=eps_t, scale=1.0)
                nc.vector.reciprocal(out=r, in_=r)
                return r

            def transpose_to(zn, nheads, sb_dst, tok0):
                """PE-transpose zn's heads into head-major bf16 dst."""
                for hg in range(nheads // 4):
                    tp = psA.tile([P, 4, P], F32, tag="tr", bufs=2)
                    for j in range(4):
                        hh = hg * 4 + j
                        nc.tensor.transpose(
                            tp[:, j, :], zn[:, hh * HD:(hh + 1) * HD], ident)
                    nc.scalar.copy(
                        out=sb_dst[:, hg * 4:(hg + 1) * 4, tok0:tok0 + P],
                        in_=tp)

            def ln_apply_mv(dst, nchunks, mv, w_t, b_t, badd_eng,
                            wmul_eng=None):
                wmul_eng = wmul_eng or nc.vector
                rstd = rstd_from_mv(mv)
                negmr = statp.tile([P, 1], F32, tag="negmr")
                nc.vector.tensor_scalar(
                    out=negmr, in0=mv[:, 0:1], scalar1=rstd, scalar2=-1.0,
                    op0=ALU.mult, op1=ALU.mult)
                n_tot = nchunks * NQ
                nc.scalar.activation(
                    out=dst[:, :n_tot], in_=dst[:, :n_tot], func=AF.Identity,
                    scale=rstd, bias=negmr)
                nc.vector.tensor_mul(out=dst[:, :n_tot], in0=dst[:, :n_tot],
                                     in1=w_t)
                badd_eng.tensor_add(out=dst[:, :n_tot], in0=dst[:, :n_tot],
                                    in1=b_t)

            def ln_apply_sb(dst, nchunks, w_t, b_t, badd_eng,
                            wmul_eng=None):
                mv = _ln_stats(nc, statp,
                               [dst[:, n * NQ:(n + 1) * NQ]
                                for n in range(nchunks)])
                ln_apply_mv(dst, nchunks, mv, w_t, b_t, badd_eng)

            xh_ap = xh_d[:, :].rearrange("(dc p) s -> p dc s", p=P)
            xl_ap = xl_d[:, :].rearrange("(dc p) s -> p dc s", p=P)

            def load_x(tok0, name=None):
                xh_t = xs.tile([P, DC, P], F8, tag="xh",
                               name=name and name + "h")
                xl_t = xs.tile([P, DC, P], F8, tag="xl",
                               name=name and name + "l")
                nc.sync.dma_start(out=xh_t, in_=xh_ap[:, :, tok0:tok0 + P])
                nc.sync.dma_start(out=xl_t, in_=xl_ap[:, :, tok0:tok0 + P])
                return xh_t, xl_t

            def dr_proj(ps_banks, xh_t, xl_t, wh, wl, nslices):
                """3-term fp8 DoubleRow accumulation over all DC chunks."""
                npair = DC // 2
                for dcp in range(npair):
                    sl = slice(2 * dcp, 2 * dcp + 2)
                    for bi, nsl in enumerate(nslices):
                        for ti, (lt, rt) in enumerate(
                                ((xh_t, wh), (xh_t, wl), (xl_t, wh))):
                            nc.tensor.matmul(
                                ps_banks[bi][:], lhsT=lt[:, sl, :],
                                rhs=rt[:, sl, nsl],
                                start=(dcp == 0 and ti == 0),
                                stop=(dcp == npair - 1 and ti == 2),
                                perf_mode=DR)

            # =========================================================
            # Stage K: local-half K proj + LN + rope + transpose, then
            # one pair-AllGather into KTr (global key order). wq's first
            # half prefetches during the K loop (DMA slack).
            # =========================================================
            wBa = tc.alloc_tile_pool(name="wBa", bufs=1)
            wqAh = wBa.tile([P, DC // 4, D], F8, tag="wh")
            wqAl = wBa.tile([P, DC // 4, D], F8, tag="wl")
            wqh_ap = wqh_d[:, :].rearrange("(dc p) n -> p dc n", p=P)
            wql_ap = wql_d[:, :].rearrange("(dc p) n -> p dc n", p=P)
            lnpK = tc.alloc_tile_pool(name="lnpK", bufs=1)
            kw_t = bcast_vec(lnpK, kw_d, KVD)
            kb_t = bcast_vec(lnpK, kb_d, KVD)
            wAk = tc.alloc_tile_pool(name="wAk", bufs=1)
            wkh_sb = wAk.tile([P, DC, KVD], F8, tag="wh")
            wkl_sb = wAk.tile([P, DC, KVD], F8, tag="wl")
            wkh_ap = wkvh_d[:, :KVD].rearrange("(dc p) n -> p dc n", p=P)
            wkl_ap = wkvl_d[:, :KVD].rearrange("(dc p) n -> p dc n", p=P)
            # queue order: 2 wk chunks, first x tile, rest of wk, wq quarter
            for dc in range(2):
                nc.sync.dma_start(out=wkh_sb[:, dc, :], in_=wkh_ap[:, dc, :])
                nc.sync.dma_start(out=wkl_sb[:, dc, :], in_=wkl_ap[:, dc, :])
            xt_first = load_x(0, name="xt_firstk")
            for dc in range(2, DC):
                nc.sync.dma_start(out=wkh_sb[:, dc, :], in_=wkh_ap[:, dc, :])
                nc.sync.dma_start(out=wkl_sb[:, dc, :], in_=wkl_ap[:, dc, :])
            nc.sync.dma_start(
                out=cosq_t,
                in_=cosq_d[:, :].rearrange("(t p) f -> p t f", p=P))
            nc.sync.dma_start(
                out=sinq_t,
                in_=sinq_d[:, :].rearrange("(t p) f -> p t f", p=P))
            for dc in range(DC // 4):
                nc.sync.dma_start(out=wqAh[:, dc, :], in_=wqh_ap[:, dc, :])
                nc.sync.dma_start(out=wqAl[:, dc, :], in_=wql_ap[:, dc, :])
            kns = []
            for tci in range(TCL):
                tok0 = tci * P
                if tci == 0:
                    xh_t, xl_t = xt_first
                else:
                    xh_t, xl_t = load_x(tok0)
                k_ps = [psA.tile([P, NQ], F32, tag=f"acc{n}", name=f"kps{n}",
                                 bufs=2) for n in range(2)]
                dr_proj(k_ps, xh_t, xl_t, wkh_sb, wkl_sb,
                        [slice(n * NQ, (n + 1) * NQ) for n in range(2)])
                kn = work.tile([P, KVD], F32, tag="work")
                for n in range(2):
                    nc.scalar.copy(out=kn[:, n * NQ:(n + 1) * NQ],
                                   in_=k_ps[n][:])
                ln_apply_sb(kn, 2, kw_t, kb_t, nc.vector)
                _rope_inplace(nc, ropep, kn, KVH, cosq_t[:, tci, :],
                              sinq_t[:, tci, :], nc.gpsimd)
                # transposes trail by two chunks so PE never waits on the
                # LN/rope chain (its latency exceeds one PE iteration)
                if tci > 1:
                    transpose_to(kns[tci - 2], KVH, KTr, tok0 - 2 * P)
                    if tci == TCL // 2 + 1:
                        # first token half done: kick off its gather early
                        nc.gpsimd.dma_start(
                            out=kin_a[:, :, :],
                            in_=KTr[:, :, 0:S_LOC // 2])
                kns.append(kn)
            for i in (TCL - 2, TCL - 1):
                transpose_to(kns[i], KVH, KTr, i * P)
            nc.gpsimd.dma_start(out=kin_b[:, :, :],
                                in_=KTr[:, :, S_LOC // 2:S_LOC])
            nc.gpsimd.collective_compute(
                "AllGather", ALU.bypass, replica_groups=REP_GROUPS,
                ins=[kin_a[:, :, :].opt()], outs=[kout_a[:, :, :, :].opt()])
            nc.gpsimd.collective_compute(
                "AllGather", ALU.bypass, replica_groups=REP_GROUPS,
                ins=[kin_b[:, :, :].opt()], outs=[kout_b[:, :, :, :].opt()])
            wAk.release()
            lnpK.release()

            # =========================================================
            # Stage V: local-half V proj, streamed to the bounce buffers
            # per token chunk; two V-half gathers follow the K gather.
            # =========================================================
            wAv = tc.alloc_tile_pool(name="wAv", bufs=1)
            wC = tc.alloc_tile_pool(name="wC", bufs=1)
            vst = tc.alloc_tile_pool(name="vstream", bufs=2)
            wvh_sb = wAv.tile([P, DC, KVD], F8, tag="wh")
            wvl_sb = wAv.tile([P, DC, KVD], F8, tag="wl")
            wvh_ap = wkvh_d[:, KVD:].rearrange("(dc p) n -> p dc n", p=P)
            wvl_ap = wkvl_d[:, KVD:].rearrange("(dc p) n -> p dc n", p=P)
            for dc in range(2):
                nc.sync.dma_start(out=wvh_sb[:, dc, :], in_=wvh_ap[:, dc, :])
                nc.sync.dma_start(out=wvl_sb[:, dc, :], in_=wvl_ap[:, dc, :])
            xt_firstv = load_x(0, name="xt_firstv")
            for dc in range(2, DC):
                nc.sync.dma_start(out=wvh_sb[:, dc, :], in_=wvh_ap[:, dc, :])
                nc.sync.dma_start(out=wvl_sb[:, dc, :], in_=wvl_ap[:, dc, :])
            # y-projection weights stream during the V loop; y-proj runs at
            # the end of this stage so attention can start right after Q
            wkvy_sb = wC.tile([P, YDC, 2 * KVD], BF, tag="w")
            wkvy_ap = wkvy_d[:, :].rearrange("(dc p) n -> p dc n", p=P)
            yt_t = xs.tile([P, YDC, YL], BF, tag="yt", bufs=1)
            nc.sync.dma_start(
                out=yt_t, in_=yT[:, :].rearrange("(dc p) s -> p dc s", p=P))
            for dc in range(YDC):
                nc.sync.dma_start(out=wkvy_sb[:, dc, :], in_=wkvy_ap[:, dc, :])
            for tci in range(TCL):
                tok0 = tci * P
                if tci == 0:
                    xh_t, xl_t = xt_firstv
                else:
                    xh_t, xl_t = load_x(tok0)
                v_ps = [psA.tile([P, NQ], F32, tag=f"acc{n}", name=f"vps{n}",
                                 bufs=2) for n in range(2)]
                dr_proj(v_ps, xh_t, xl_t, wvh_sb, wvl_sb,
                        [slice(n * NQ, (n + 1) * NQ) for n in range(2)])
                vt = vst.tile([P, KVD], BF, tag="vt", bufs=4)
                for n in range(2):
                    nc.scalar.activation(
                        out=vt[:, n * NQ:(n + 1) * NQ], in_=v_ps[n][:],
                        func=AF.Identity, scale=1.0 / WSCALE)
                nc.gpsimd.dma_start(out=vin1[:, tci, :],
                                     in_=vt[:, 0:KVD // 2])
                nc.gpsimd.dma_start(out=vin2[:, tci, :],
                                    in_=vt[:, KVD // 2:KVD])
            nc.gpsimd.collective_compute(
                "AllGather", ALU.bypass, replica_groups=REP_GROUPS,
                ins=[vin1[:, :, :].opt()], outs=[vout1[:, :, :, :].opt()])
            nc.gpsimd.collective_compute(
                "AllGather", ALU.bypass, replica_groups=REP_GROUPS,
                ins=[vin2[:, :, :].opt()], outs=[vout2[:, :, :, :].opt()])
            # ---- y projections -> YKT (LN, no rope), YV ----
            vst.release()
            lnpY = tc.alloc_tile_pool(name="lnpY", bufs=1)
            kyw_t = bcast_vec(lnpY, kyw_d, KVD, dt=BF)
            kyb_t = bcast_vec(lnpY, kyb_d, KVD, dt=BF)
            yk_ps = [psA.tile([P, NQ], F32, tag=f"acc{n}", name=f"ykps{n}",
                              bufs=2) for n in range(2)]
            yv_ps = [psA.tile([P, NQ], F32, tag=f"acc{n+2}", name=f"yvps{n}",
                              bufs=1) for n in range(2)]
            for dc in range(YDC):
                for n in range(2):
                    nc.tensor.matmul(
                        yk_ps[n][:], lhsT=yt_t[:, dc, :],
                        rhs=wkvy_sb[:, dc, n * NQ:(n + 1) * NQ],
                        start=(dc == 0), stop=(dc == YDC - 1))
                for n in range(2):
                    nc.tensor.matmul(
                        yv_ps[n][:], lhsT=yt_t[:, dc, :],
                        rhs=wkvy_sb[:, dc, KVD + n * NQ:KVD + (n + 1) * NQ],
                        start=(dc == 0), stop=(dc == YDC - 1))
            for n in range(2):
                nc.scalar.copy(
                    out=YV[:, 4 * n:4 * (n + 1), :], in_=yv_ps[n][:])
            ykn = work.tile([P, KVD], F32, tag="work")
            for n in range(2):
                nc.scalar.copy(out=ykn[:, n * NQ:(n + 1) * NQ],
                               in_=yk_ps[n][:])
            ln_apply_sb(ykn, 2, kyw_t, kyb_t, nc.vector)
            for hg in range(2):
                tp = psA.tile([P, 4, P], F32, tag="tr", bufs=2)
                for j in range(4):
                    kv = hg * 4 + j
                    nc.tensor.transpose(
                        tp[:, j, :], ykn[:, kv * HD:(kv + 1) * HD], ident)
                nc.scalar.copy(
                    out=YKT[:, hg * 4:(hg + 1) * 4, :], in_=tp)
            lnpY.release()
            wC.release()
            wAv.release()

            # =========================================================
            # Stage Q: Q proj + LN + rope + transpose -> QT. First wq
            # half is already resident; second half streams in now.
            # =========================================================
            lnpQ = tc.alloc_tile_pool(name="lnpQ", bufs=1)
            qw_t = bcast_vec(lnpQ, qw_d, D)
            qb_t = bcast_vec(lnpQ, qb_d, D)
            wBb = tc.alloc_tile_pool(name="wBb", bufs=1)
            wqBh = wBb.tile([P, 3 * DC // 4, D], F8, tag="wh")
            wqBl = wBb.tile([P, 3 * DC // 4, D], F8, tag="wl")
            for dc in range(3 * DC // 4):
                nc.sync.dma_start(out=wqBh[:, dc, :],
                                  in_=wqh_ap[:, DC // 4 + dc, :])
                nc.sync.dma_start(out=wqBl[:, dc, :],
                                  in_=wql_ap[:, DC // 4 + dc, :])
            qns = []
            for tcl in range(TCL):
                tok0 = tcl * P
                xh_t, xl_t = load_x(tok0)
                q_ps = [psA.tile([P, NQ], F32,
                                 tag=f"acc{n}", name=f"qps{n}",
                                 bufs=(2 if n < 2 else 1)) for n in range(4)]
                npair = DC // 2
                for dcp in range(npair):
                    if dcp < DC // 8:
                        wh, wl = wqAh, wqAl
                        sl = slice(2 * dcp, 2 * dcp + 2)
                    else:
                        wh, wl = wqBh, wqBl
                        sl = slice(2 * dcp - DC // 4, 2 * dcp - DC // 4 + 2)
                    xsl = slice(2 * dcp, 2 * dcp + 2)
                    for n in range(4):
                        for ti, (lt, rt) in enumerate(
                                ((xh_t, wh), (xh_t, wl), (xl_t, wh))):
                            nc.tensor.matmul(
                                q_ps[n][:], lhsT=lt[:, xsl, :],
                                rhs=rt[:, sl, n * NQ:(n + 1) * NQ],
                                start=(dcp == 0 and ti == 0),
                                stop=(dcp == npair - 1 and ti == 2),
                                perf_mode=DR)
                qn = work.tile([P, D], F32, tag="work")
                for n in range(4):
                    nc.scalar.copy(out=qn[:, n * NQ:(n + 1) * NQ],
                                   in_=q_ps[n][:])
                ln_apply_sb(qn, 4, qw_t, qb_t, nc.vector, nc.gpsimd)
                for hh in range(2):
                    _rope_inplace(nc, ropep, qn[:, hh * KVD:(hh + 1) * KVD],
                                  H // 2, cosq_t[:, tcl, :],
                                  sinq_t[:, tcl, :], nc.gpsimd)
                if tcl > 1:
                    transpose_to(qns[tcl - 2], H, QT, tok0 - 2 * P)
                qns.append(qn)
            for i in (TCL - 2, TCL - 1):
                transpose_to(qns[i], H, QT, i * P)
            # land K slots via the Pool queue (slot r = token half r),
            # deprioritized so the scheduler orders them after the Q-stage
            # Pool work (they wait on the K gather; anything queued behind
            # them would stall with them)
            HSL = S_LOC // 2
            for kv in range(KVH):
                for r in range(2):
                    nc.gpsimd.dma_start(
                        out=KTr[:, kv, r * S_LOC:r * S_LOC + HSL],
                        in_=kout_a[r, :, kv, :])
                    nc.gpsimd.dma_start(
                        out=KTr[:, kv, r * S_LOC + HSL:(r + 1) * S_LOC],
                        in_=kout_b[r, :, kv, :])
            wBb.release()
            lnpQ.release()
            wBa.release()

            statp.release()
            ropep.release()
            work.release()
            xs.release()
            psA.release()

            # =========================================================
            # Stage D+E: pipelined attention + interleaved out-proj.
            # V lands via the otherwise-idle Pool queue (the landing DMAs
            # wait on the V gathers; nothing else queues behind them).
            # =========================================================
            vrp = tc.alloc_tile_pool(name="vrpool", bufs=1)
            Vr = vrp.tile([P, TC, KVD], BF)       # token-major V, global order
            for vo, c0 in ((vout1, 0), (vout2, KVD // 2)):
                nc.gpsimd.dma_start(out=Vr[:, 0:TCL, c0:c0 + KVD // 2],
                                    in_=vo[0, :, :, :])
                nc.gpsimd.dma_start(out=Vr[:, TCL:TC, c0:c0 + KVD // 2],
                                    in_=vo[1, :, :, :])
            ep = tc.alloc_tile_pool(name="escores", bufs=2)
            esp = tc.alloc_tile_pool(name="espairs", bufs=4)
            esq = tc.alloc_tile_pool(name="esquads", bufs=8)
            eyp = tc.alloc_tile_pool(name="eyscores", bufs=2)
            rcp = tc.alloc_tile_pool(name="recips", bufs=2)
            wop = tc.alloc_tile_pool(name="wostream", bufs=2)
            outp = tc.alloc_tile_pool(name="outtiles", bufs=3)
            psD = tc.alloc_tile_pool(name="psD", bufs=1, space="PSUM")

            wo_ap = wo_d[:, :].rearrange("(hc p) n -> p hc n", p=P)

            def load_wo8(j):
                t = wop.tile([P, DC, 2 * P], BF, tag="wo8", bufs=2,
                             name=f"wo8_{j}")
                nc.sync.dma_start(out=t, in_=wo_ap[:, :, j * 256:(j + 1) * 256])
                return t

            def attn_scores(h, qc):
                kv = h // NREP
                q0 = qc * NQ
                qt_t = QT[:, h, q0:q0 + NQ]
                # self-attention scores for all 16 key chunks
                E_t = ep.tile([P, TC, NQ], BF, tag="E", bufs=2)
                quads = []
                octs = []
                pairs = []
                for kp in range(TC // 2):
                    s_ps = psD.tile([P, 2, NQ], F32, tag="s", bufs=2)
                    for j in range(2):
                        kc = kp * 2 + j
                        nc.tensor.matmul(
                            s_ps[:, j, :],
                            lhsT=KTr[:, kv, kc * P:(kc + 1) * P],
                            rhs=qt_t, start=True, stop=True,
                            skip_group_check=True)
                    nc.scalar.activation(
                        out=E_t[:, kp * 2:kp * 2 + 2, :], in_=s_ps[:],
                        func=AF.Exp, scale=SCALE)
                    pr = esp.tile([P, NQ], BF, tag="esp", bufs=4,
                                  name="espair")
                    nc.vector.tensor_add(
                        out=pr, in0=E_t[:, kp * 2, :],
                        in1=E_t[:, kp * 2 + 1, :])
                    pairs.append(pr)
                    if kp % 2 == 1:
                        qd = esq.tile([P, NQ], BF, tag="esq", bufs=6,
                                      name="esquad")
                        nc.vector.tensor_add(
                            out=qd, in0=pairs[-2], in1=pairs[-1])
                        quads.append(qd)
                        if len(quads) % 2 == 0:
                            oc8 = esq.tile([P, NQ], BF, tag="eso", bufs=4,
                                           name="esoct")
                            nc.vector.tensor_add(
                                out=oc8, in0=quads[-2], in1=quads[-1])
                            octs.append(oc8)
                # cross-attention (needs y-stage outputs, so issued after
                # the self scores): sy -> Ey -> dy -> oy -> t1
                sy_ps = psD.tile([P, NQ], F32, tag="cross", bufs=1,
                                 name="sy_ps")
                nc.tensor.matmul(
                    sy_ps[:], lhsT=YKT[:, kv, :], rhs=qt_t,
                    start=True, stop=True, skip_group_check=True)
                Ey_t = eyp.tile([P, NQ], BF, tag="Ey", bufs=2)
                nc.scalar.activation(
                    out=Ey_t, in_=sy_ps[:], func=AF.Exp, scale=SCALE,
                    bias=ymb_t)
                dy_ps = psD.tile([P, NQ], F32, tag="cross", bufs=1,
                                 name="dy_ps")
                nc.tensor.matmul(
                    dy_ps[:], lhsT=ones_t, rhs=Ey_t,
                    start=True, stop=True, skip_group_check=True)
                rec_y = rcp.tile([P, NQ], F32, tag="recy", bufs=2)
                nc.vector.reciprocal(out=rec_y, in_=dy_ps[:])
                oy_ps = psD.tile([P, NQ], F32, tag="cross", bufs=1,
                                 name="oy_ps")
                nc.tensor.matmul(
                    oy_ps[:], lhsT=YV[:, kv, :], rhs=Ey_t,
                    start=True, stop=True, skip_group_check=True)
                t1 = rcp.tile([P, NQ], F32, tag="t1", bufs=3)
                nc.vector.scalar_tensor_tensor(
                    out=t1, in0=oy_ps[:], scalar=gates_t[:, h:h + 1],
                    in1=rec_y, op0=ALU.mult, op1=ALU.mult)
                return dict(h=h, kv=kv, q0=q0, E_t=E_t, quads=octs, t1=t1)

            def attn_out(st):
                h, kv, q0 = st["h"], st["kv"], st["q0"]
                E_t, quads, t1 = st["E_t"], st["quads"], st["t1"]
                d_ps = psD.tile([P, NQ], F32, tag="d", bufs=1, name="d_ps")
                for i, qd in enumerate(quads):
                    nc.tensor.matmul(
                        d_ps[:], lhsT=ones_t, rhs=qd,
                        start=(i == 0), stop=(i == len(quads) - 1),
                        skip_group_check=True)
                o_ps = psD.tile([P, NQ], F32, tag="o", bufs=1, name="o_ps")
                for kc in range(TC):
                    nc.tensor.matmul(
                        o_ps[:], lhsT=Vr[:, kc, kv * HD:(kv + 1) * HD],
                        rhs=E_t[:, kc, :],
                        start=(kc == 0), stop=(kc == TC - 1),
                        skip_group_check=True)
                rec = rcp.tile([P, NQ], F32, tag="rec", bufs=2)
                nc.vector.reciprocal(out=rec, in_=d_ps[:])
                t0 = rcp.tile([P, NQ], F32, tag="t0", bufs=2)
                nc.vector.tensor_mul(out=t0, in0=o_ps[:], in1=rec)
                # merged output aliases into the (now dead) Q^T slice
                nc.vector.tensor_add(
                    out=QT[:, h, q0:q0 + NQ], in0=t0, in1=t1)

            def outproj(qc, oc, wo8_t):
                q0 = qc * NQ
                out_ps = psD.tile([P, NQ], F32, tag="oout", bufs=1,
                                  name="out_ps")
                sub = (oc % 2) * P
                for hc in range(DC):
                    nc.tensor.matmul(
                        out_ps[:],
                        lhsT=wo8_t[:, hc, sub:sub + P],
                        rhs=QT[:, hc, q0:q0 + NQ],
                        start=(hc == 0), stop=(hc == DC - 1),
                        skip_group_check=True)
                out_t = outp.tile([P, NQ], F32, tag="outt")
                nc.vector.tensor_copy(out=out_t, in_=out_ps[:])
                nc.sync.dma_start(
                    out=outT[oc * P:(oc + 1) * P, q0:q0 + NQ],
                    in_=out_t)

            pending = []
            wo8_t = None
            for qc in range(QCN):
                for h in range(H):
                    st = attn_scores(h, qc)
                    pending.append(st)
                    if len(pending) > 1:
                        attn_out(pending.pop(0))
                    if qc == 1:
                        oc = h
                        if oc % 2 == 0:
                            wo8_t = load_wo8(oc // 2)
                        outproj(0, oc, wo8_t)
            for st in pending:
                attn_out(st)
            for oc in range(DC):
                if oc % 2 == 0:
                    wo8_t = load_wo8(oc // 2)
                outproj(1, oc, wo8_t)

            psD.release()
            outp.release()
            wop.release()
            rcp.release()
            eyp.release()
            esq.release()
            esp.release()
            ep.release()
            vrp.release()

    _split_dma_waits(nc)
    return nc


def _prep_shared(x, y, freqs_cos, freqs_sin, y_mask, wq, wk, wv, wk_y, wv_y,
                 wo, q_w, q_b, k_w, k_b, ky_w, ky_b, gate):
    f32 = np.float32

    def hilo(a, scale):
        s = np.asarray(a, f32) * scale
        hi = s.astype(E4)
        lo = (s - hi.astype(f32)).astype(E4)
        return np.ascontiguousarray(hi), np.ascontiguousarray(lo)

    wq64 = hilo(wq, WSCALE)
    wkv64 = hilo(np.concatenate([np.asarray(wk, f32), np.asarray(wv, f32)],
                                axis=1), WSCALE)
    shared = {
        "wqh": wq64[0], "wql": wq64[1],
        "wkvh": wkv64[0], "wkvl": wkv64[1],
        "wkvy": np.ascontiguousarray(
            np.concatenate([np.asarray(wk_y, f32), np.asarray(wv_y, f32)],
                           axis=1).astype(BF16)),
        "wo": np.ascontiguousarray(np.asarray(wo, f32).astype(BF16)),
        "qw": np.ascontiguousarray(np.asarray(q_w, f32)),
        "qb": np.ascontiguousarray(np.asarray(q_b, f32)),
        "kw": np.ascontiguousarray(np.asarray(k_w, f32)),
        "kb": np.ascontiguousarray(np.asarray(k_b, f32)),
        "kyw": np.ascontiguousarray(np.asarray(ky_w, f32).astype(BF16)),
        "kyb": np.ascontiguousarray(np.asarray(ky_b, f32).astype(BF16)),
        "gates": np.ascontiguousarray(np.tanh(np.asarray(gate, f32))),
    }
    per_core = []
    for c in range(8):
        b, hf = c // 2, c % 2
        sl = slice(hf * S_LOC, (hf + 1) * S_LOC)
        xTb = np.asarray(x[b], f32).T
        m = dict(shared)
        m["xh"], m["xl"] = hilo(xTb[:, sl], 1.0)
        m["yT"] = np.ascontiguousarray(np.asarray(y[b], f32).T.astype(BF16))
        m["cosq"] = np.ascontiguousarray(np.asarray(freqs_cos, f32)[sl])
        m["sinq"] = np.ascontiguousarray(np.asarray(freqs_sin, f32)[sl])
        m["ymb"] = np.where(np.asarray(y_mask[b]), 0.0, -1e9).astype(f32)
        per_core.append(m)
    return per_core


def kernel(**inputs):
    if "nc" not in _CACHED:
        _CACHED["nc"] = build_program()
    nc = _CACHED["nc"]
    in_maps = _prep_shared(
        inputs["x"], inputs["y"], inputs["freqs_cos"], inputs["freqs_sin"],
        inputs["y_mask"], inputs["wq"], inputs["wk"], inputs["wv"],
        inputs["wk_y"], inputs["wv_y"], inputs["wo"], inputs["q_w"],
        inputs["q_b"], inputs["k_w"], inputs["k_b"], inputs["ky_w"],
        inputs["ky_b"], inputs["gate"])
    res = run_bass_kernel_spmd(nc, in_maps, core_ids=list(range(8)))
    global LAST_EXEC_NS
    LAST_EXEC_NS = res.exec_time_ns
    out = np.zeros((B, S, D), np.float32)
    for c in range(8):
        b, hf = c // 2, c % 2
        out[b, hf * S_LOC:(hf + 1) * S_LOC, :] = res.results[c]["outT"].T
    return out


if __name__ == "__main__":
    nc = build_program()
    print("program built OK")


# revision 8
# speedup vs baseline: 1.3566x; 1.0005x over previous
"""Trainium2 Bass kernel for nn_Attention_35734127903400 — v2.

Token-sharded (core c: batch c//2, seq half c%2) with pair-AllGather K/V
dedup: each core projects K/V only for its local 1024 tokens, exchanges
halves with its pair core via two HBM AllGathers (K first, then V) that
run on the collective cores, overlapped with Q/y projection. Key order
in KTr/Vr is global (gather slot r = token half r); on odd cores the
slot-0 DMA overwrites the locally-projected half with the peer's data
and the local half lands in the upper columns — same program on every
core, no divergence.

K^T/V/Q^T stay SBUF-resident (no DRAM spill). Attention is software-
pipelined: out-matmuls trail scores by one head (tolerating the late V
gather); softmax denominators fold 4-wide on DVE into 4 ones-matmuls;
output projection for q-chunk 0 interleaves with attention q-chunk 1,
with wo streamed in eighths; merged output aliases into dead Q^T
slices. Collectives issue from Pool (gpsimd), whose sequencer blocks on
the bounce-DMA waits — so LN bias-adds run on DVE/Pool split such that
nothing downstream queues behind a blocked Pool sequencer.
"""

import numpy as np
import ml_dtypes

import concourse.bass as bass
import concourse.mybir as mybir
import concourse.tile as tile
from concourse.bass_utils import run_bass_kernel_spmd
from concourse.masks import make_identity

BF16 = ml_dtypes.bfloat16
E4 = ml_dtypes.float8_e4m3fn
F32 = mybir.dt.float32
BF = mybir.dt.bfloat16
F8 = mybir.dt.float8e4
WSCALE = 64.0
DR = mybir.MatmulPerfMode.DoubleRow

P = 128
B, S, D = 4, 2048, 2048
H, KVH = 16, 8
HD = 128
NREP = 2
YL, YD = 128, 1024
EPS = 1e-5
S_LOC = S // 2
DC = D // P          # 16 contraction chunks for D
YDC = YD // P        # 8
TC = S // P          # 16 key chunks (full seq)
TCL = S_LOC // P     # 8 local token chunks
NQ = 512             # q-free chunk (one PSUM bank of f32)
QCN = S_LOC // NQ    # 2
KVD = KVH * HD       # 1024
SCALE = 1.0 / float(np.sqrt(np.float32(HD)))
AF = mybir.ActivationFunctionType
ALU = mybir.AluOpType
REP_GROUPS = [[0, 1], [2, 3], [4, 5], [6, 7]]

_CACHED = {}
LAST_EXEC_NS = None


def _ln_stats(nc, statp, ps_chunks):
    """bn_stats over a list of [P, 512] chunks -> mv [P, 2] (mean, var)."""
    nchunks = len(ps_chunks)
    stats = statp.tile([P, nchunks, 6], F32, tag="bnstats")
    for i, ps in enumerate(ps_chunks):
        nc.vector.bn_stats(out=stats[:, i, :], in_=ps[:])
    mv = statp.tile([P, 2], F32, tag="bnaggr")
    nc.vector.bn_aggr(out=mv, in_=stats)
    return mv


def _rope_inplace(nc, ropep, zn, nheads, cos_t, sin_t, mul_eng):
    """In-place rope on zn [P, nheads*HD] f32; cos/sin [P, 64] f32.
    The three products run on mul_eng (DVE or Pool), sub/add on DVE."""
    zv = zn.rearrange("p (h f two) -> p h f two", h=nheads, two=2)
    re = zv[:, :, :, 0]
    im = zv[:, :, :, 1]
    shp = (P, nheads, HD // 2)
    cb = cos_t[:, None, :].to_broadcast(shp)
    sb = sin_t[:, None, :].to_broadcast(shp)
    t1 = ropep.tile([P, nheads, HD // 2], F32, tag="rp1")
    t2 = ropep.tile([P, nheads, HD // 2], F32, tag="rp2")
    t3 = ropep.tile([P, nheads, HD // 2], F32, tag="rp3")
    mul_eng.tensor_mul(out=t1, in0=re, in1=cb)     # re*c
    mul_eng.tensor_mul(out=t2, in0=re, in1=sb)     # re*s
    nc.vector.tensor_mul(out=t3, in0=im, in1=sb)   # im*s
    nc.vector.tensor_sub(out=re, in0=t1, in1=t3)   # re' = re*c - im*s
    mul_eng.tensor_mul(out=t3, in0=im, in1=cb)     # im*c
    nc.vector.tensor_add(out=im, in0=t2, in1=t3)   # im' = re*s + im*c


def _split_dma_waits(nc, max_waits=1):
    """Hoist excess sync waits onto preceding same-engine single-wait NoOps
    (walrus per-instruction structs have 1-2 wait slots)."""
    n_split = 0
    for f in nc.m.functions:
        for blk in f.blocks:
            insts = list(blk.instructions)
            out = []
            changed = False
            for ins in insts:
                si = ins.sync_info
                if (si is not None and si.on_wait
                        and len(si.on_wait) > max_waits):
                    waits = list(si.on_wait)
                    for wi, w in enumerate(waits[:-max_waits]):
                        out.append(mybir.InstNoOp(
                            name=f"{ins.name}-wsplit{wi}", engine=ins.engine,
                            sync_info=mybir.SyncInfo(on_wait=[w],
                                                     on_update=[])))
                    ins.sync_info = mybir.SyncInfo(
                        on_wait=waits[-max_waits:],
                        on_update=list(si.on_update))
                    changed = True
                    n_split += 1
                out.append(ins)
            if changed:
                blk.instructions = out
    return n_split


def build_program():
    nc = bass.Bass()

    # ---- I/O (all per-core local; key order handled host-side) ----
    xh_d = nc.declare_dram_parameter("xh", [D, S_LOC], F8, isOutput=False)
    xl_d = nc.declare_dram_parameter("xl", [D, S_LOC], F8, isOutput=False)
    yT = nc.declare_dram_parameter("yT", [YD, YL], BF, isOutput=False)
    wqh_d = nc.declare_dram_parameter("wqh", [D, D], F8, isOutput=False)
    wql_d = nc.declare_dram_parameter("wql", [D, D], F8, isOutput=False)
    wkvh_d = nc.declare_dram_parameter("wkvh", [D, 2 * KVD], F8, isOutput=False)
    wkvl_d = nc.declare_dram_parameter("wkvl", [D, 2 * KVD], F8, isOutput=False)
    wkvy_d = nc.declare_dram_parameter("wkvy", [YD, 2 * KVD], BF, isOutput=False)
    wo_d = nc.declare_dram_parameter("wo", [D, D], BF, isOutput=False)
    qw_d = nc.declare_dram_parameter("qw", [D], F32, isOutput=False)
    qb_d = nc.declare_dram_parameter("qb", [D], F32, isOutput=False)
    kw_d = nc.declare_dram_parameter("kw", [KVD], F32, isOutput=False)
    kb_d = nc.declare_dram_parameter("kb", [KVD], F32, isOutput=False)
    kyw_d = nc.declare_dram_parameter("kyw", [KVD], BF, isOutput=False)
    kyb_d = nc.declare_dram_parameter("kyb", [KVD], BF, isOutput=False)
    cosq_d = nc.declare_dram_parameter("cosq", [S_LOC, HD // 2], F32, isOutput=False)
    sinq_d = nc.declare_dram_parameter("sinq", [S_LOC, HD // 2], F32, isOutput=False)
    gates_d = nc.declare_dram_parameter("gates", [H], F32, isOutput=False)
    ymb_d = nc.declare_dram_parameter("ymb", [YL], F32, isOutput=False)
    outT = nc.declare_dram_parameter("outT", [D, S_LOC], F32, isOutput=True)

    with tile.TileContext(nc) as tc:
        from contextlib import ExitStack
        with ExitStack() as ctx:
            # ---- persistent pools ----
            cpool = ctx.enter_context(tc.tile_pool(name="consts", bufs=1))
            yp = ctx.enter_context(tc.tile_pool(name="ypool", bufs=1))
            ktp = ctx.enter_context(tc.tile_pool(name="ktpool", bufs=1))
            qtp = ctx.enter_context(tc.tile_pool(name="qtpool", bufs=1))
            dramp = ctx.enter_context(
                tc.tile_pool(name="dscratch", bufs=1, space="DRAM"))

            KTr = ktp.tile([P, KVH, S], BF)       # key-major K^T, global order
            QT = qtp.tile([P, H, S_LOC], BF)      # Q^T; merged aliases in later
            YKT = yp.tile([P, KVH, YL], BF)
            YV = yp.tile([P, KVH, HD], BF)

            # one K gather; V gather split by feature half (kv-heads 0-3 /
            # 4-7) so each half lands just ahead of the heads needing it
            kin_a = dramp.tile([P, KVH, S_LOC // 2], BF)
            kin_b = dramp.tile([P, KVH, S_LOC // 2], BF)
            kout_a = dramp.tile([2, P, KVH, S_LOC // 2], BF)
            kout_b = dramp.tile([2, P, KVH, S_LOC // 2], BF)
            vin1 = dramp.tile([P, TCL, KVD // 2], BF)
            vin2 = dramp.tile([P, TCL, KVD // 2], BF)
            vout1 = dramp.tile([2, P, TCL, KVD // 2], BF)
            vout2 = dramp.tile([2, P, TCL, KVD // 2], BF)

            # projection-phase transient pools
            xs = tc.alloc_tile_pool(name="xstream", bufs=3)
            work = tc.alloc_tile_pool(name="work", bufs=3)
            ropep = tc.alloc_tile_pool(name="rope", bufs=1)
            statp = tc.alloc_tile_pool(name="stats", bufs=3)
            psA = tc.alloc_tile_pool(name="psA", bufs=1, space="PSUM")

            # ---- constants ----
            ident = cpool.tile([P, P], F32)
            make_identity(nc, ident)
            ones_t = cpool.tile([P, P], BF)
            nc.vector.memset(ones_t, 1.0)
            eps_t = cpool.tile([P, 1], F32)
            nc.vector.memset(eps_t, EPS)
            gates_t = cpool.tile([P, H], F32)
            nc.gpsimd.dma_start(
                out=gates_t,
                in_=bass.AP(tensor=gates_d, offset=0, ap=[[0, P], [1, H]]))
            ymb_t = cpool.tile([P, 1], F32)
            nc.gpsimd.dma_start(
                out=ymb_t,
                in_=bass.AP(tensor=ymb_d, offset=0, ap=[[1, P], [0, 1]]))

            cosq_t = ropep.tile([P, TCL, HD // 2], F32, tag="costab", bufs=1)
            sinq_t = ropep.tile([P, TCL, HD // 2], F32, tag="sintab", bufs=1)

            def bcast_vec(pool, dram_h, n, dt=F32):
                t = pool.tile([P, n], dt, tag=f"lnp_{dram_h.name}", bufs=1)
                nc.gpsimd.dma_start(
                    out=t, in_=bass.AP(tensor=dram_h, offset=0, ap=[[0, P], [1, n]]))
                return t

            def rstd_from_mv(mv):
                r = statp.tile([P, 1], F32, tag="rstd")
                nc.scalar.activation(out=r, in_=mv[:, 1:2], func=AF.Sqrt,
                                     bias=eps_t, scale=1.0)
                nc.vector.reciprocal(out=r, in_=r)
                return r

            def transpose_to(zn, nheads, sb_dst, tok0):
                """PE-transpose zn's heads into head-major bf16 dst."""
                for hg in range(nheads // 4):
                    tp = psA.tile([P, 4, P], F32, tag="tr", bufs=2)
                    for j in range(4):
                        hh = hg * 4 + j
                        nc.tensor.transpose(
                            tp[:, j, :], zn[:, hh * HD:(hh + 1) * HD], ident)
                    nc.scalar.copy(
                        out=sb_dst[:, hg * 4:(hg + 1) * 4, tok0:tok0 + P],
                        in_=tp)

            def ln_apply_mv(dst, nchunks, mv, w_t, b_t, badd_eng,
                            wmul_eng=None):
                wmul_eng = wmul_eng or nc.vector
                rstd = rstd_from_mv(mv)
                negmr = statp.tile([P, 1], F32, tag="negmr")
                nc.vector.tensor_scalar(
                    out=negmr, in0=mv[:, 0:1], scalar1=rstd, scalar2=-1.0,
                    op0=ALU.mult, op1=ALU.mult)
                n_tot = nchunks * NQ
                nc.scalar.activation(
                    out=dst[:, :n_tot], in_=dst[:, :n_tot], func=AF.Identity,
                    scale=rstd, bias=negmr)
                nc.vector.tensor_mul(out=dst[:, :n_tot], in0=dst[:, :n_tot],
                                     in1=w_t)
                badd_eng.tensor_add(out=dst[:, :n_tot], in0=dst[:, :n_tot],
                                    in1=b_t)

            def ln_apply_sb(dst, nchunks, w_t, b_t, badd_eng,
                            wmul_eng=None):
                mv = _ln_stats(nc, statp,
                               [dst[:, n * NQ:(n + 1) * NQ]
                                for n in range(nchunks)])
                ln_apply_mv(dst, nchunks, mv, w_t, b_t, badd_eng)

            xh_ap = xh_d[:, :].rearrange("(dc p) s -> p dc s", p=P)
            xl_ap = xl_d[:, :].rearrange("(dc p) s -> p dc s", p=P)

            def load_x(tok0, name=None):
                xh_t = xs.tile([P, DC, P], F8, tag="xh",
                               name=name and name + "h")
                xl_t = xs.tile([P, DC, P], F8, tag="xl",
                               name=name and name + "l")
                nc.sync.dma_start(out=xh_t, in_=xh_ap[:, :, tok0:tok0 + P])
                nc.sync.dma_start(out=xl_t, in_=xl_ap[:, :, tok0:tok0 + P])
                return xh_t, xl_t

            def dr_proj(ps_banks, xh_t, xl_t, wh, wl, nslices):
                """3-term fp8 DoubleRow accumulation over all DC chunks."""
                npair = DC // 2
                for dcp in range(npair):
                    sl = slice(2 * dcp, 2 * dcp + 2)
                    for bi, nsl in enumerate(nslices):
                        for ti, (lt, rt) in enumerate(
                                ((xh_t, wh), (xh_t, wl), (xl_t, wh))):
                            nc.tensor.matmul(
                                ps_banks[bi][:], lhsT=lt[:, sl, :],
                                rhs=rt[:, sl, nsl],
                                start=(dcp == 0 and ti == 0),
                                stop=(dcp == npair - 1 and ti == 2),
                                perf_mode=DR)

            # =========================================================
            # Stage K: local-half K proj + LN + rope + transpose, then
            # one pair-AllGather into KTr (global key order). wq's first
            # half prefetches during the K loop (DMA slack).
            # =========================================================
            wBa = tc.alloc_tile_pool(name="wBa", bufs=1)
            wqAh = wBa.tile([P, DC // 4, D], F8, tag="wh")
            wqAl = wBa.tile([P, DC // 4, D], F8, tag="wl")
            wqh_ap = wqh_d[:, :].rearrange("(dc p) n -> p dc n", p=P)
            wql_ap = wql_d[:, :].rearrange("(dc p) n -> p dc n", p=P)
            lnpK = tc.alloc_tile_pool(name="lnpK", bufs=1)
            kw_t = bcast_vec(lnpK, kw_d, KVD)
            kb_t = bcast_vec(lnpK, kb_d, KVD)
            wAk = tc.alloc_tile_pool(name="wAk", bufs=1)
            wkh_sb = wAk.tile([P, DC, KVD], F8, tag="wh")
            wkl_sb = wAk.tile([P, DC, KVD], F8, tag="wl")
            wkh_ap = wkvh_d[:, :KVD].rearrange("(dc p) n -> p dc n", p=P)
            wkl_ap = wkvl_d[:, :KVD].rearrange("(dc p) n -> p dc n", p=P)
            # queue order: 2 wk chunks, first x tile, rest of wk, wq quarter
            for dc in range(2):
                nc.sync.dma_start(out=wkh_sb[:, dc, :], in_=wkh_ap[:, dc, :])
                nc.sync.dma_start(out=wkl_sb[:, dc, :], in_=wkl_ap[:, dc, :])
            xt_first = load_x(0, name="xt_firstk")
            for dc in range(2, DC):
                nc.sync.dma_start(out=wkh_sb[:, dc, :], in_=wkh_ap[:, dc, :])
                nc.sync.dma_start(out=wkl_sb[:, dc, :], in_=wkl_ap[:, dc, :])
            nc.sync.dma_start(
                out=cosq_t,
                in_=cosq_d[:, :].rearrange("(t p) f -> p t f", p=P))
            nc.sync.dma_start(
                out=sinq_t,
                in_=sinq_d[:, :].rearrange("(t p) f -> p t f", p=P))
            for dc in range(DC // 4):
                nc.sync.dma_start(out=wqAh[:, dc, :], in_=wqh_ap[:, dc, :])
                nc.sync.dma_start(out=wqAl[:, dc, :], in_=wql_ap[:, dc, :])
            kns = []
            for tci in range(TCL):
                tok0 = tci * P
                if tci == 0:
                    xh_t, xl_t = xt_first
                else:
                    xh_t, xl_t = load_x(tok0)
                k_ps = [psA.tile([P, NQ], F32, tag=f"acc{n}", name=f"kps{n}",
                                 bufs=2) for n in range(2)]
                dr_proj(k_ps, xh_t, xl_t, wkh_sb, wkl_sb,
                        [slice(n * NQ, (n + 1) * NQ) for n in range(2)])
                kn = work.tile([P, KVD], F32, tag="work")
                for n in range(2):
                    nc.scalar.copy(out=kn[:, n * NQ:(n + 1) * NQ],
                                   in_=k_ps[n][:])
                ln_apply_sb(kn, 2, kw_t, kb_t, nc.vector)
                _rope_inplace(nc, ropep, kn, KVH, cosq_t[:, tci, :],
                              sinq_t[:, tci, :], nc.gpsimd)
                # transposes trail by two chunks so PE never waits on the
                # LN/rope chain (its latency exceeds one PE iteration)
                if tci > 1:
                    transpose_to(kns[tci - 2], KVH, KTr, tok0 - 2 * P)
                    if tci == TCL // 2 + 1:
                        # first token half done: kick off its gather early
                        nc.gpsimd.dma_start(
                            out=kin_a[:, :, :],
                            in_=KTr[:, :, 0:S_LOC // 2])
                kns.append(kn)
            for i in (TCL - 2, TCL - 1):
                transpose_to(kns[i], KVH, KTr, i * P)
            nc.gpsimd.dma_start(out=kin_b[:, :, :],
                                in_=KTr[:, :, S_LOC // 2:S_LOC])
            nc.gpsimd.collective_compute(
                "AllGather", ALU.bypass, replica_groups=REP_GROUPS,
                ins=[kin_a[:, :, :].opt()], outs=[kout_a[:, :, :, :].opt()])
            nc.gpsimd.collective_compute(
                "AllGather", ALU.bypass, replica_groups=REP_GROUPS,
                ins=[kin_b[:, :, :].opt()], outs=[kout_b[:, :, :, :].opt()])
            wAk.release()
            lnpK.release()

            # =========================================================
            # Stage V: local-half V proj, streamed to the bounce buffers
            # per token chunk; two V-half gathers follow the K gather.
            # =========================================================
            wAv = tc.alloc_tile_pool(name="wAv", bufs=1)
            wC = tc.alloc_tile_pool(name="wC", bufs=1)
            vst = tc.alloc_tile_pool(name="vstream", bufs=2)
            wvh_sb = wAv.tile([P, DC, KVD], F8, tag="wh")
            wvl_sb = wAv.tile([P, DC, KVD], F8, tag="wl")
            wvh_ap = wkvh_d[:, KVD:].rearrange("(dc p) n -> p dc n", p=P)
            wvl_ap = wkvl_d[:, KVD:].rearrange("(dc p) n -> p dc n", p=P)
            for dc in range(2):
                nc.sync.dma_start(out=wvh_sb[:, dc, :], in_=wvh_ap[:, dc, :])
                nc.sync.dma_start(out=wvl_sb[:, dc, :], in_=wvl_ap[:, dc, :])
            xt_firstv = load_x(0, name="xt_firstv")
            for dc in range(2, DC):
                nc.sync.dma_start(out=wvh_sb[:, dc, :], in_=wvh_ap[:, dc, :])
                nc.sync.dma_start(out=wvl_sb[:, dc, :], in_=wvl_ap[:, dc, :])
            # y-projection weights stream during the V loop; y-proj runs at
            # the end of this stage so attention can start right after Q
            wkvy_sb = wC.tile([P, YDC, 2 * KVD], BF, tag="w")
            wkvy_ap = wkvy_d[:, :].rearrange("(dc p) n -> p dc n", p=P)
            yt_t = xs.tile([P, YDC, YL], BF, tag="yt", bufs=1)
            nc.sync.dma_start(
                out=yt_t, in_=yT[:, :].rearrange("(dc p) s -> p dc s", p=P))
            for dc in range(YDC):
                nc.sync.dma_start(out=wkvy_sb[:, dc, :], in_=wkvy_ap[:, dc, :])
            for tci in range(TCL):
                tok0 = tci * P
                if tci == 0:
                    xh_t, xl_t = xt_firstv
                else:
                    xh_t, xl_t = load_x(tok0)
                v_ps = [psA.tile([P, NQ], F32, tag=f"acc{n}", name=f"vps{n}",
                                 bufs=2) for n in range(2)]
                dr_proj(v_ps, xh_t, xl_t, wvh_sb, wvl_sb,
                        [slice(n * NQ, (n + 1) * NQ) for n in range(2)])
                vt = vst.tile([P, KVD], BF, tag="vt", bufs=4)
                for n in range(2):
                    nc.scalar.activation(
                        out=vt[:, n * NQ:(n + 1) * NQ], in_=v_ps[n][:],
                        func=AF.Identity, scale=1.0 / WSCALE)
                nc.gpsimd.dma_start(out=vin1[:, tci, :],
                                     in_=vt[:, 0:KVD // 2])
                nc.gpsimd.dma_start(out=vin2[:, tci, :],
                                    in_=vt[:, KVD // 2:KVD])
            nc.gpsimd.collective_compute(
                "AllGather", ALU.bypass, replica_groups=REP_GROUPS,
                ins=[vin1[:, :, :].opt()], outs=[vout1[:, :, :, :].opt()])
            nc.gpsimd.collective_compute(
                "AllGather", ALU.bypass, replica_groups=REP_GROUPS,
                ins=[vin2[:, :, :].opt()], outs=[vout2[:, :, :, :].opt()])
            # ---- y projections -> YKT (LN, no rope), YV ----
            vst.release()
            lnpY = tc.alloc_tile_pool(name="lnpY", bufs=1)
            kyw_t = bcast_vec(lnpY, kyw_d, KVD, dt=BF)
            kyb_t = bcast_vec(lnpY, kyb_d, KVD, dt=BF)
            yk_ps = [psA.tile([P, NQ], F32, tag=f"acc{n}", name=f"ykps{n}",
                              bufs=2) for n in range(2)]
            yv_ps = [psA.tile([P, NQ], F32, tag=f"acc{n+2}", name=f"yvps{n}",
                              bufs=1) for n in range(2)]
            for dc in range(YDC):
                for n in range(2):
                    nc.tensor.matmul(
                        yk_ps[n][:], lhsT=yt_t[:, dc, :],
                        rhs=wkvy_sb[:, dc, n * NQ:(n + 1) * NQ],
                        start=(dc == 0), stop=(dc == YDC - 1))
                for n in range(2):
                    nc.tensor.matmul(
                        yv_ps[n][:], lhsT=yt_t[:, dc, :],
                        rhs=wkvy_sb[:, dc, KVD + n * NQ:KVD + (n + 1) * NQ],
                        start=(dc == 0), stop=(dc == YDC - 1))
            for n in range(2):
                nc.scalar.copy(
                    out=YV[:, 4 * n:4 * (n + 1), :], in_=yv_ps[n][:])
            ykn = work.tile([P, KVD], F32, tag="work")
            for n in range(2):
                nc.scalar.copy(out=ykn[:, n * NQ:(n + 1) * NQ],
                               in_=yk_ps[n][:])
            ln_apply_sb(ykn, 2, kyw_t, kyb_t, nc.vector)
            for hg in range(2):
                tp = psA.tile([P, 4, P], F32, tag="tr", bufs=2)
                for j in range(4):
                    kv = hg * 4 + j
                    nc.tensor.transpose(
                        tp[:, j, :], ykn[:, kv * HD:(kv + 1) * HD], ident)
                nc.scalar.copy(
                    out=YKT[:, hg * 4:(hg + 1) * 4, :], in_=tp)
            lnpY.release()
            wC.release()
            wAv.release()

            # =========================================================
            # Stage Q: Q proj + LN + rope + transpose -> QT. First wq
            # half is already resident; second half streams in now.
            # =========================================================
            lnpQ = tc.alloc_tile_pool(name="lnpQ", bufs=1)
            qw_t = bcast_vec(lnpQ, qw_d, D)
            qb_t = bcast_vec(lnpQ, qb_d, D)
            wBb = tc.alloc_tile_pool(name="wBb", bufs=1)
            wqBh = wBb.tile([P, 3 * DC // 4, D], F8, tag="wh")
            wqBl = wBb.tile([P, 3 * DC // 4, D], F8, tag="wl")
            for dc in range(3 * DC // 4):
                nc.sync.dma_start(out=wqBh[:, dc, :],
                                  in_=wqh_ap[:, DC // 4 + dc, :])
                nc.sync.dma_start(out=wqBl[:, dc, :],
                                  in_=wql_ap[:, DC // 4 + dc, :])
            qns = []
            for tcl in range(TCL):
                tok0 = tcl * P
                xh_t, xl_t = load_x(tok0)
                q_ps = [psA.tile([P, NQ], F32,
                                 tag=f"acc{n}", name=f"qps{n}",
                                 bufs=(2 if n < 2 else 1)) for n in range(4)]
                npair = DC // 2
                for dcp in range(npair):
                    if dcp < DC // 8:
                        wh, wl = wqAh, wqAl
                        sl = slice(2 * dcp, 2 * dcp + 2)
                    else:
                        wh, wl = wqBh, wqBl
                        sl = slice(2 * dcp - DC // 4, 2 * dcp - DC // 4 + 2)
                    xsl = slice(2 * dcp, 2 * dcp + 2)
                    for n in range(4):
                        for ti, (lt, rt) in enumerate(
                                ((xh_t, wh), (xh_t, wl), (xl_t, wh))):
                            nc.tensor.matmul(
                                q_ps[n][:], lhsT=lt[:, xsl, :],
                                rhs=rt[:, sl, n * NQ:(n + 1) * NQ],
                                start=(dcp == 0 and ti == 0),
                                stop=(dcp == npair - 1 and ti == 2),
                                perf_mode=DR)
                qn = work.tile([P, D], F32, tag="work")
                for n in range(4):
                    nc.scalar.copy(out=qn[:, n * NQ:(n + 1) * NQ],
                                   in_=q_ps[n][:])
                ln_apply_sb(qn, 4, qw_t, qb_t, nc.vector, nc.gpsimd)
                for hh in range(2):
                    _rope_inplace(nc, ropep, qn[:, hh * KVD:(hh + 1) * KVD],
                                  H // 2, cosq_t[:, tcl, :],
                                  sinq_t[:, tcl, :], nc.gpsimd)
                if tcl > 1:
                    transpose_to(qns[tcl - 2], H, QT, tok0 - 2 * P)
                qns.append(qn)
            for i in (TCL - 2, TCL - 1):
                transpose_to(qns[i], H, QT, i * P)
            # land K slots via the Pool queue (slot r = token half r),
            # deprioritized so the scheduler orders them after the Q-stage
            # Pool work (they wait on the K gather; anything queued behind
            # them would stall with them)
            HSL = S_LOC // 2
            for kv in range(KVH):
                for r in range(2):
                    nc.gpsimd.dma_start(
                        out=KTr[:, kv, r * S_LOC:r * S_LOC + HSL],
                        in_=kout_a[r, :, kv, :])
                    nc.gpsimd.dma_start(
                        out=KTr[:, kv, r * S_LOC + HSL:(r + 1) * S_LOC],
                        in_=kout_b[r, :, kv, :])
            wBb.release()
            lnpQ.release()
            wBa.release()

            statp.release()
            ropep.release()
            work.release()
            xs.release()
            psA.release()

            # =========================================================
            # Stage D+E: pipelined attention + interleaved out-proj.
            # V lands via the otherwise-idle Pool queue (the landing DMAs
            # wait on the V gathers; nothing else queues behind them).
            # =========================================================
            vrp = tc.alloc_tile_pool(name="vrpool", bufs=1)
            Vr = vrp.tile([P, TC, KVD], BF)       # token-major V, global order
            for vo, c0 in ((vout1, 0), (vout2, KVD // 2)):
                nc.gpsimd.dma_start(out=Vr[:, 0:TCL, c0:c0 + KVD // 2],
                                    in_=vo[0, :, :, :])
                nc.gpsimd.dma_start(out=Vr[:, TCL:TC, c0:c0 + KVD // 2],
                                    in_=vo[1, :, :, :])
            ep = tc.alloc_tile_pool(name="escores", bufs=2)
            esp = tc.alloc_tile_pool(name="espairs", bufs=4)
            esq = tc.alloc_tile_pool(name="esquads", bufs=8)
            eyp = tc.alloc_tile_pool(name="eyscores", bufs=2)
            rcp = tc.alloc_tile_pool(name="recips", bufs=2)
            wop = tc.alloc_tile_pool(name="wostream", bufs=2)
            outp = tc.alloc_tile_pool(name="outtiles", bufs=3)
            psD = tc.alloc_tile_pool(name="psD", bufs=1, space="PSUM")

            wo_ap = wo_d[:, :].rearrange("(hc p) n -> p hc n", p=P)

            def load_wo8(j):
                t = wop.tile([P, DC, 2 * P], BF, tag="wo8", bufs=2,
                             name=f"wo8_{j}")
                nc.sync.dma_start(out=t, in_=wo_ap[:, :, j * 256:(j + 1) * 256])
                return t

            def attn_scores(h, qc):
                kv = h // NREP
                q0 = qc * NQ
                qt_t = QT[:, h, q0:q0 + NQ]
                # self-attention scores for all 16 key chunks
                E_t = ep.tile([P, TC, NQ], BF, tag="E", bufs=3)
                quads = []
                octs = []
                pairs = []
                for kp in range(TC // 2):
                    s_ps = psD.tile([P, 2, NQ], F32, tag="s", bufs=2)
                    for j in range(2):
                        kc = kp * 2 + j
                        nc.tensor.matmul(
                            s_ps[:, j, :],
                            lhsT=KTr[:, kv, kc * P:(kc + 1) * P],
                            rhs=qt_t, start=True, stop=True,
                            skip_group_check=True)
                    nc.scalar.activation(
                        out=E_t[:, kp * 2:kp * 2 + 2, :], in_=s_ps[:],
                        func=AF.Exp, scale=SCALE)
                    pr = esp.tile([P, NQ], BF, tag="esp", bufs=4,
                                  name="espair")
                    nc.vector.tensor_add(
                        out=pr, in0=E_t[:, kp * 2, :],
                        in1=E_t[:, kp * 2 + 1, :])
                    pairs.append(pr)
                    if kp % 2 == 1:
                        qd = esq.tile([P, NQ], BF, tag="esq", bufs=6,
                                      name="esquad")
                        nc.vector.tensor_add(
                            out=qd, in0=pairs[-2], in1=pairs[-1])
                        quads.append(qd)
                        if len(quads) % 2 == 0:
                            oc8 = esq.tile([P, NQ], BF, tag="eso", bufs=4,
                                           name="esoct")
                            nc.vector.tensor_add(
                                out=oc8, in0=quads[-2], in1=quads[-1])
                            octs.append(oc8)
                # cross-attention (needs y-stage outputs, so issued after
                # the self scores): sy -> Ey -> dy -> oy -> t1
                sy_ps = psD.tile([P, NQ], F32, tag="cross", bufs=1,
                                 name="sy_ps")
                nc.tensor.matmul(
                    sy_ps[:], lhsT=YKT[:, kv, :], rhs=qt_t,
                    start=True, stop=True, skip_group_check=True)
                Ey_t = eyp.tile([P, NQ], BF, tag="Ey", bufs=2)
                nc.scalar.activation(
                    out=Ey_t, in_=sy_ps[:], func=AF.Exp, scale=SCALE,
                    bias=ymb_t)
                dy_ps = psD.tile([P, NQ], F32, tag="cross", bufs=1,
                                 name="dy_ps")
                nc.tensor.matmul(
                    dy_ps[:], lhsT=ones_t, rhs=Ey_t,
                    start=True, stop=True, skip_group_check=True)
                rec_y = rcp.tile([P, NQ], F32, tag="recy", bufs=2)
                nc.vector.reciprocal(out=rec_y, in_=dy_ps[:])
                oy_ps = psD.tile([P, NQ], F32, tag="cross", bufs=1,
                                 name="oy_ps")
                nc.tensor.matmul(
                    oy_ps[:], lhsT=YV[:, kv, :], rhs=Ey_t,
                    start=True, stop=True, skip_group_check=True)
                t1 = rcp.tile([P, NQ], F32, tag="t1", bufs=3)
                nc.vector.scalar_tensor_tensor(
                    out=t1, in0=oy_ps[:], scalar=gates_t[:, h:h + 1],
                    in1=rec_y, op0=ALU.mult, op1=ALU.mult)
                return dict(h=h, kv=kv, q0=q0, E_t=E_t, quads=octs, t1=t1)

            def attn_out(st):
                h, kv, q0 = st["h"], st["kv"], st["q0"]
                E_t, quads, t1 = st["E_t"], st["quads"], st["t1"]
                d_ps = psD.tile([P, NQ], F32, tag="d", bufs=1, name="d_ps")
                for i, qd in enumerate(quads):
                    nc.tensor.matmul(
                        d_ps[:], lhsT=ones_t, rhs=qd,
                        start=(i == 0), stop=(i == len(quads) - 1),
                        skip_group_check=True)
                o_ps = psD.tile([P, NQ], F32, tag="o", bufs=1, name="o_ps")
                for kc in range(TC):
                    nc.tensor.matmul(
                        o_ps[:], lhsT=Vr[:, kc, kv * HD:(kv + 1) * HD],
                        rhs=E_t[:, kc, :],
                        start=(kc == 0), stop=(kc == TC - 1),
                        skip_group_check=True)
                rec = rcp.tile([P, NQ], F32, tag="rec", bufs=2)
                nc.vector.reciprocal(out=rec, in_=d_ps[:])
                t0 = rcp.tile([P, NQ], F32, tag="t0", bufs=2)
                nc.vector.tensor_mul(out=t0, in0=o_ps[:], in1=rec)
                # merged output aliases into the (now dead) Q^T slice
                nc.vector.tensor_add(
                    out=QT[:, h, q0:q0 + NQ], in0=t0, in1=t1)

            def outproj(qc, oc, wo8_t):
                q0 = qc * NQ
                out_ps = psD.tile([P, NQ], F32, tag="oout", bufs=1,
                                  name="out_ps")
                sub = (oc % 2) * P
                for hc in range(DC):
                    nc.tensor.matmul(
                        out_ps[:],
                        lhsT=wo8_t[:, hc, sub:sub + P],
                        rhs=QT[:, hc, q0:q0 + NQ],
                        start=(hc == 0), stop=(hc == DC - 1),
                        skip_group_check=True)
                out_t = outp.tile([P, NQ], F32, tag="outt")
                nc.vector.tensor_copy(out=out_t, in_=out_ps[:])
                nc.sync.dma_start(
                    out=outT[oc * P:(oc + 1) * P, q0:q0 + NQ],
                    in_=out_t)

            pending = []
            wo8_t = None
            for qc in range(QCN):
                for h in range(H):
                    st = attn_scores(h, qc)
                    pending.append(st)
                    if len(pending) > 1:
                        attn_out(pending.pop(0))
                    if qc == 1:
                        oc = h
                        if oc % 2 == 0:
                            wo8_t = load_wo8(oc // 2)
                        outproj(0, oc, wo8_t)
            for st in pending:
                attn_out(st)
            for oc in range(DC):
                if oc % 2 == 0:
                    wo8_t = load_wo8(oc // 2)
                outproj(1, oc, wo8_t)

            psD.release()
            outp.release()
            wop.release()
            rcp.release()
            eyp.release()
            esq.release()
            esp.release()
            ep.release()
            vrp.release()

    _split_dma_waits(nc)
    return nc


def _prep_shared(x, y, freqs_cos, freqs_sin, y_mask, wq, wk, wv, wk_y, wv_y,
                 wo, q_w, q_b, k_w, k_b, ky_w, ky_b, gate):
    f32 = np.float32

    def hilo(a, scale):
        s = np.asarray(a, f32) * scale
        hi = s.astype(E4)
        lo = (s - hi.astype(f32)).astype(E4)
        return np.ascontiguousarray(hi), np.ascontiguousarray(lo)

    wq64 = hilo(wq, WSCALE)
    wkv64 = hilo(np.concatenate([np.asarray(wk, f32), np.asarray(wv, f32)],
                                axis=1), WSCALE)
    shared = {
        "wqh": wq64[0], "wql": wq64[1],
        "wkvh": wkv64[0], "wkvl": wkv64[1],
        "wkvy": np.ascontiguousarray(
            np.concatenate([np.asarray(wk_y, f32), np.asarray(wv_y, f32)],
                           axis=1).astype(BF16)),
        "wo": np.ascontiguousarray(np.asarray(wo, f32).astype(BF16)),
        "qw": np.ascontiguousarray(np.asarray(q_w, f32)),
        "qb": np.ascontiguousarray(np.asarray(q_b, f32)),
        "kw": np.ascontiguousarray(np.asarray(k_w, f32)),
        "kb": np.ascontiguousarray(np.asarray(k_b, f32)),
        "kyw": np.ascontiguousarray(np.asarray(ky_w, f32).astype(BF16)),
        "kyb": np.ascontiguousarray(np.asarray(ky_b, f32).astype(BF16)),
        "gates": np.ascontiguousarray(np.tanh(np.asarray(gate, f32))),
    }
    per_core = []
    for c in range(8):
        b, hf = c // 2, c % 2
        sl = slice(hf * S_LOC, (hf + 1) * S_LOC)
        xTb = np.asarray(x[b], f32).T
        m = dict(shared)
        m["xh"], m["xl"] = hilo(xTb[:, sl], 1.0)
        m["yT"] = np.ascontiguousarray(np.asarray(y[b], f32).T.astype(BF16))
        m["cosq"] = np.ascontiguousarray(np.asarray(freqs_cos, f32)[sl])
        m["sinq"] = np.ascontiguousarray(np.asarray(freqs_sin, f32)[sl])
        m["ymb"] = np.where(np.asarray(y_mask[b]), 0.0, -1e9).astype(f32)
        per_core.append(m)
    return per_core


def kernel(**inputs):
    if "nc" not in _CACHED:
        _CACHED["nc"] = build_program()
    nc = _CACHED["nc"]
    in_maps = _prep_shared(
        inputs["x"], inputs["y"], inputs["freqs_cos"], inputs["freqs_sin"],
        inputs["y_mask"], inputs["wq"], inputs["wk"], inputs["wv"],
        inputs["wk_y"], inputs["wv_y"], inputs["wo"], inputs["q_w"],
        inputs["q_b"], inputs["k_w"], inputs["k_b"], inputs["ky_w"],
        inputs["ky_b"], inputs["gate"])
    res = run_bass_kernel_spmd(nc, in_maps, core_ids=list(range(8)))
    global LAST_EXEC_NS
    LAST_EXEC_NS = res.exec_time_ns
    out = np.zeros((B, S, D), np.float32)
    for c in range(8):
        b, hf = c // 2, c % 2
        out[b, hf * S_LOC:(hf + 1) * S_LOC, :] = res.results[c]["outT"].T
    return out


if __name__ == "__main__":
    nc = build_program()
    print("program built OK")
